# revision 18
# baseline (speedup 1.0000x reference)
"""PowerSpectrumModel Trainium2 kernel (8 NeuronCores, SPMD).

Strategy (data-parallel over atoms, structures disjoint per shard):
 - Host: cut the atom axis at structure boundaries into 8 balanced shards;
   pre-TRANSPOSE ps to feature-major [128, nT, 8, TILE] (so every tile
   load is one contiguous stride per partition — no on-device transpose
   descriptors); fp32 PSUM accumulation on device; replicate the small
   weight matrices.
 - Precision (fp8mix mode): the big h1 GEMM runs fp8e4m3 DoubleRow (2x
   PE rate; w1-quant + ps-quant contribute only ~6e-3 rel err because
   errors decorrelate across atoms); h2/psnn stay fp16 (their weight
   quantization biases the per-atom energy coherently across a structure
   -> 3.6e-2 if fp8); psl gets hi+lo fp8 splits of both ps and W_psl
   (or an extra fp16 psT copy, PSL_MODE).
 - Device, per 512-atom tile:
     psT   <- contiguous loads [128, 8, 512]                      [DMA]
     h1    = W_h1 @ psT  (fp8 DoubleRow k-pairs / fp16 k-chunks)  [PE]
     psl   = W_psl row, M=1 matmuls -> e partial rows 0/32/64/96  [PE]
     sil1  = silu(h1)                                             [ACT]
     h2    = W_h2 @ sil1; sil2 = silu(h2)                         [PE/ACT]
     psnn  M=1 matmuls accumulated onto the psl PSUM rows         [PE]
     per 128-chunk: e column via K=97 ones matmul, + species
     energy, one-hot(struct) matmul accumulates into a [1,256]
     PSUM row holding this core's per-structure energies.         [PE/DVE]
 - Host: slice per-core structure ranges, concat -> [2000, 1].
"""

import numpy as np
import ml_dtypes

N_ATOMS = 200000
N_FEAT = 1024
N_SPECIES = 4
N_STRUCT = 2000
H1 = 256
H2 = 256
SCALE = 1.0
N_CORES = 8
TILE = 512
CHUNK = 128
SMAX = 256  # per-core structure capacity (PSUM row)

MODE = "fp8mix"  # "fp16" | "fp8mix"
PSL_MODE = "fp16"  # "e3" | "hilo8" | "fp16"
SEG_WINDOWED = True  # bake per-chunk structure windows into the seg matmul

_BUILD_CACHE = {}
TRACE = False
LAST_EXEC_NS = None
LAST_RESULTS = None

F8 = ml_dtypes.float8_e4m3


def _split_waits(nc, mybir, maxw=1):
    """walrus on this build rejects >1 sync wait per instruction; move
    overflow waits onto preceding same-engine NoOps."""
    cnt = 0
    for f in nc.m.functions:
        for blk in f.blocks:
            if not hasattr(blk, "instructions"):
                continue
            out = []
            changed = False
            for inst in blk.instructions:
                si = getattr(inst, "sync_info", None)
                if si is not None and si.on_wait and len(si.on_wait) > maxw:
                    waits = list(si.on_wait)
                    keep = waits[-maxw:]
                    extra = waits[:-maxw]
                    while extra:
                        chunk, extra = extra[:maxw], extra[maxw:]
                        cnt += 1
                        out.append(
                            mybir.InstNoOp(
                                name=f"waitfix-{cnt}",
                                engine=inst.engine,
                                text_hint="waitfix",
                                bass_nofuse=True,
                                ins=[],
                                outs=[],
                                sync_info=mybir.SyncInfo(on_wait=chunk, on_update=[]),
                            )
                        )
                    si.on_wait = keep
                    changed = True
                out.append(inst)
            if changed:
                blk.instructions[:] = out
    return cnt


def _build(Ta, C, poly, mode, psl_mode, segw=None):
    import concourse.bass as bass
    import concourse.tile as tile
    import concourse.mybir as mybir
    from contextlib import ExitStack

    fp8 = mode == "fp8mix"
    f8 = mybir.dt.float8e4
    f16 = mybir.dt.float16
    f32 = mybir.dt.float32
    fd = f8 if fp8 else f16  # h1 GEMM dtype
    AF = mybir.ActivationFunctionType
    ALU = mybir.AluOpType
    DR = mybir.MatmulPerfMode.DoubleRow if fp8 else None
    PSUM = bass.MemorySpace.PSUM
    nT = Ta // TILE
    c0, c1, c2, c3 = (float(x) for x in poly)
    KG = 4 if fp8 else 8  # h1 k-groups (DoubleRow contracts pairs)
    KS = 2 if fp8 else 1  # k-chunks per h1 matmul

    nc = bass.Bass("TRN2", target_bir_lowering=False, debug=False)

    # pre-transposed ps: [p, t, k, a] = ps[t*512+a, k*128+p]
    psT_d = nc.dram_tensor("psT", [128, nT, 8, TILE], fd, kind="ExternalInput").ap()
    f8e3 = mybir.dt.float8e3
    psl2_d = None
    if fp8:
        d2 = {"hilo8": f8, "e3": f8e3, "fp16": f16}[psl_mode]
        psl2_d = nc.dram_tensor("psT2", [128, nT, 8, TILE], d2, kind="ExternalInput").ap()
    relb_d = nc.dram_tensor("relb", [CHUNK, C], f32, kind="ExternalInput").ap()
    nums_d = nc.dram_tensor("nums", [CHUNK, C], f32, kind="ExternalInput").ap()
    # w1: [p, j, m, i, c] = W_h1[m*128+c, (KS*j+i)*128+p]
    w1_d = nc.dram_tensor("w1", [128, KG, 2, KS, 128], fd, kind="ExternalInput").ap()
    # w2: [p, m, i, c] = W_h2[m*128+c, i*128+p]  (fp16)
    w2_d = nc.dram_tensor("w2", [128, 2, 2, 128], f16, kind="ExternalInput").ap()
    # wout: [p, i] = W_out[0, i*128+p]  (fp16)
    wout_d = nc.dram_tensor("wout", [128, 2, 1], f16, kind="ExternalInput").ap()
    # psl weights
    if fp8 and psl_mode == "hilo8":
        # k-subtile dim padded to step 16 (ISA: dual-fp8 LW step%16==0)
        wpa_d = nc.dram_tensor("wpsl_hi", [128, 4, 2, 16], f8, kind="ExternalInput").ap()
        wpb_d = nc.dram_tensor("wpsl_lo", [128, 4, 2, 16], f8, kind="ExternalInput").ap()
    elif fp8 and psl_mode == "e3":
        wpa_d = nc.dram_tensor("wpsl", [128, 8, 1, 1], f8e3, kind="ExternalInput").ap()
        wpb_d = None
    else:
        wpa_d = nc.dram_tensor("wpsl", [128, 8, 1, 1], f16, kind="ExternalInput").ap()
        wpb_d = None
    ones_d = nc.dram_tensor("ones", [97, 1], f16, kind="ExternalInput").ap()
    iota_d = nc.dram_tensor("iota", [128, SMAX], f16, kind="ExternalInput").ap()
    out_d = nc.dram_tensor("out", [1, SMAX], f32, kind="ExternalOutput").ap()

    with tile.TileContext(nc) as tc, ExitStack() as ctx:
        const = ctx.enter_context(tc.tile_pool(name="const", bufs=1))
        psTp = ctx.enter_context(tc.tile_pool(name="psT", bufs=6))
        psTp2 = ctx.enter_context(tc.tile_pool(name="psT2", bufs=6)) if fp8 else None
        silp = ctx.enter_context(tc.tile_pool(name="sil", bufs=4))
        rowp = ctx.enter_context(tc.tile_pool(name="row", bufs=4))
        pp_h1 = ctx.enter_context(tc.tile_pool(name="pph1", bufs=1, space=PSUM))
        pp_h2 = ctx.enter_context(tc.tile_pool(name="pph2", bufs=1, space=PSUM))
        pp_e = ctx.enter_context(tc.tile_pool(name="ppe", bufs=1, space=PSUM))
        pp_ec = ctx.enter_context(tc.tile_pool(name="ppec", bufs=1, space=PSUM))
        pp_seg = ctx.enter_context(tc.tile_pool(name="ppseg", bufs=1, space=PSUM))

        # ---- constants ----
        w1_sb = const.tile([128, KG, 2, KS, 128], fd, tag="w1")
        nc.sync.dma_start(w1_sb[:], w1_d[:])
        w2_sb = const.tile([128, 2, 2, 128], f16, tag="w2")
        nc.sync.dma_start(w2_sb[:], w2_d[:])
        wout_sb = const.tile([128, 2, 1], f16, tag="wout")
        nc.sync.dma_start(wout_sb[:], wout_d[:])
        if fp8 and psl_mode == "hilo8":
            wpa_sb = const.tile([128, 4, 2, 16], f8, tag="wpa")
            nc.sync.dma_start(wpa_sb[:], wpa_d[:])
            wpb_sb = const.tile([128, 4, 2, 16], f8, tag="wpb")
            nc.sync.dma_start(wpb_sb[:], wpb_d[:])
        else:
            wpa_sb = const.tile([128, 8, 1, 1], f8e3 if (fp8 and psl_mode == "e3") else f16, tag="wpa")
            nc.sync.dma_start(wpa_sb[:], wpa_d[:])
            wpb_sb = None
        iota_sb = const.tile([128, SMAX], f16, tag="iota")
        nc.sync.dma_start(iota_sb[:], iota_d[:])
        relb_sb = const.tile([CHUNK, C], f32, tag="relb")
        nc.sync.dma_start(relb_sb[:], relb_d[:])
        nums_sb = const.tile([CHUNK, C], f32, tag="nums")
        nc.sync.dma_start(nums_sb[:], nums_d[:])
        ones_sb = const.tile([97, 1], f16, tag="ones")
        nc.sync.dma_start(ones_sb[:], ones_d[:])

        # species energy per atom: cubic through W_comp[0, 0..3]
        # comp = (c1*n + c0) + n*n*(c3*n + c2)
        t_n2 = const.tile([CHUNK, C], f32, tag="t_n2")
        nc.vector.tensor_mul(t_n2[:], nums_sb[:], nums_sb[:])
        t_a = const.tile([CHUNK, C], f32, tag="t_a")
        nc.vector.tensor_scalar(t_a[:], nums_sb[:], c3, c2, ALU.mult, ALU.add)
        t_b = const.tile([CHUNK, C], f32, tag="t_b")
        nc.vector.tensor_mul(t_b[:], t_n2[:], t_a[:])
        t_c = const.tile([CHUNK, C], f32, tag="t_c")
        nc.vector.tensor_scalar(t_c[:], nums_sb[:], c1, c0, ALU.mult, ALU.add)
        comp_sb = const.tile([CHUNK, C], f32, tag="comp")
        nc.vector.tensor_add(comp_sb[:], t_b[:], t_c[:])

        seg_ps = pp_seg.tile([1, SMAX], f32, tag="seg")
        nc.vector.memset(seg_ps[:], 0.0)
        # e-partials banks (double-buffered so tile t+1's psl does not
        # wait on tile t's e_row copy): partial rows 0/32/64/96; the rows
        # in between stay 0 from this one-time clear, so a K=97
        # ones-matmul sums the partials.
        e_bufs = []
        for i in range(2):
            eb = pp_e.tile([128, TILE], f32, tag=f"e{i}", name=f"e{i}")
            nc.vector.memset(eb[:], 0.0)
            e_bufs.append(eb)
        # precompute every chunk's one-hot window (constants only) so the
        # seg matmul never waits on the DVE mid-tile
        oh_all = None
        if segw is not None:
            W = segw[0]
            oh_all = const.tile([128, C, W], f16, tag="oh_all")
            for ch in range(C):
                nc.vector.tensor_scalar(
                    oh_all[:, ch], iota_sb[:, 0:W], relb_sb[:, ch : ch + 1],
                    None, ALU.is_equal,
                )

        for t in range(nT):
            e_ps = e_bufs[t % 2]
            # ---- contiguous loads: [128, 8, 512]
            big = psTp.tile([128, 8, TILE], fd, tag="psT", name=f"psT{t}")
            nc.sync.dma_start(big[:], psT_d[:, t])
            if fp8:
                d2 = {"hilo8": f8, "e3": f8e3, "fp16": f16}[psl_mode]
                big2 = psTp2.tile([128, 8, TILE], d2, tag="psT2", name=f"psT2_{t}")
                nc.sync.dma_start(big2[:], psl2_d[:, t])

            # ---- h1: KG k-groups x 2 M-chunks
            h1ps = [pp_h1.tile([128, TILE], f32, tag=f"h1m{m}", name=f"h1ps{t}_{m}") for m in range(2)]
            for j in range(KG):
                for m in range(2):
                    nc.tensor.matmul(
                        h1ps[m][:],
                        w1_sb[:, j, m],
                        big[:, KS * j : KS * (j + 1)],
                        start=(j == 0),
                        stop=(j == KG - 1),
                        perf_mode=DR,
                    )
            # ---- psl: M=1 matmuls in column groups, partial sums landing
            # on e_ps rows 0/32/64/96.
            if fp8 and psl_mode == "hilo8":
                # hi*w_hi + lo*w_hi + hi*w_lo, all fp8 DoubleRow.
                # DoubleRow forbids col-group tiling (col_grp must be 0xf)
                # so every partial accumulates onto e_ps row 0.
                for cn, (wp, mv) in enumerate(
                    [(wpa_sb, big), (wpa_sb, big2), (wpb_sb, big)]
                ):
                    for j in range(4):
                        nc.tensor.matmul(
                            e_ps[0:1, :],
                            wp[:, j, :, 0:1],
                            mv[:, 2 * j : 2 * j + 2],
                            start=(cn == 0 and j == 0),
                            stop=False,
                            perf_mode=DR,
                        )
            elif fp8:
                # psl from the scaled-e3m4/fp16 psT copy: fast M=1 matmuls
                # in column groups 0/1, partials on e_ps rows 0/32 (the
                # 1/(s*t) unscale rides in the ones vector rows 0:33)
                for k in range(8):
                    g = 32 * (k % 2)
                    nc.tensor.matmul(
                        e_ps[g : g + 1, :],
                        wpa_sb[:, k],
                        big2[:, k],
                        start=(k < 2),
                        stop=False,
                        tile_position=(0, g),
                    )
            else:
                for k in range(8):
                    g = 32 * (k % 4)
                    nc.tensor.matmul(
                        e_ps[g : g + 1, :],
                        wpa_sb[:, k],
                        big[:, k],
                        start=(k < 4),
                        stop=False,
                        tile_position=(0, g),
                    )

            sil1 = silp.tile([128, 2, TILE], f16, tag="sil1")
            for m in range(2):
                nc.scalar.activation(sil1[:, m], h1ps[m][:], AF.Silu)

            # ---- h2 (fp16)
            h2ps = [pp_h2.tile([128, TILE], f32, tag=f"h2m{m}", name=f"h2ps{t}_{m}") for m in range(2)]
            for kj in range(2):
                for m in range(2):
                    nc.tensor.matmul(
                        h2ps[m][:],
                        w2_sb[:, m, kj],
                        sil1[:, kj],
                        start=(kj == 0),
                        stop=(kj == 1),
                    )
            sil2 = silp.tile([128, 2, TILE], f16, tag="sil2")
            for m in range(2):
                nc.scalar.activation(sil2[:, m], h2ps[m][:], AF.Silu)

            # ---- psnn (fp16). In fp8 modes psl only occupies rows 0(/32)
            # so psnn gets its own fresh rows 64/96 (unit-scaled in the
            # ones vector); in fp16 mode it accumulates onto the psl rows.
            for kj in range(2):
                g = (64 + 32 * kj) if fp8 else (32 * kj)
                nc.tensor.matmul(
                    e_ps[g : g + 1, :],
                    wout_sb[:, kj],
                    sil2[:, kj],
                    start=fp8,
                    stop=(kj == 1),
                    tile_position=(0, g),
                )
            # partial rows 0/32/64/96 (zeros between) -> SBUF in one copy
            e_row = rowp.tile([97, TILE], f16, tag="erow")
            nc.vector.tensor_copy(e_row[:], e_ps[0:97, :])

            # ---- per-chunk: column-ize (K=97 sums the partials), add
            # species energy, segment matmul
            for cc in range(4):
                ch = t * 4 + cc
                ec_ps = pp_ec.tile([128, 1], f32, tag="ec")
                nc.tensor.matmul(
                    ec_ps[:],
                    e_row[0:97, cc * 128 : (cc + 1) * 128],
                    ones_sb[:],
                    start=True,
                    stop=True,
                )
                e_col = rowp.tile([128, 1], f16, tag="ecol")
                nc.vector.tensor_add(e_col[:], ec_ps[:], comp_sb[:, ch : ch + 1])
                if segw is not None:
                    # host guarantees this chunk's structures fall inside
                    # [w0, w0+W); relb is pre-shifted by -w0 per chunk
                    W, w0s = segw
                    w0 = w0s[ch]
                    nc.tensor.matmul(
                        seg_ps[0:1, w0 : w0 + W], e_col[:], oh_all[:, ch],
                        start=False, stop=(ch == C - 1), skip_group_check=True,
                    )
                else:
                    oh = rowp.tile([128, SMAX], f16, tag="oh")
                    nc.vector.tensor_scalar(
                        oh[:], iota_sb[:], relb_sb[:, ch : ch + 1], None, ALU.is_equal
                    )
                    nc.tensor.matmul(
                        seg_ps[:], e_col[:], oh[:], start=(ch == 0), stop=(ch == C - 1)
                    )

        out_sb = rowp.tile([1, SMAX], f32, tag="outsb")
        nc.scalar.activation(out_sb[:], seg_ps[:], AF.Copy)
        nc.sync.dma_start(out_d[:], out_sb[:])

    _split_waits(nc, mybir)
    return nc


def _install_ntff_hook():
    """Register the axon NTFF profile hook (missing antenv.axon_hooks in
    this image) so run_bass_kernel_spmd(trace=True) can report exec_time_ns."""
    import sys
    import types

    try:
        import antenv.axon_hooks  # noqa: F401

        return
    except ImportError:
        pass
    from trn_agent_boot.trn_boot import _ntff_profile_via_ctypes

    hook = _ntff_profile_via_ctypes("/opt/axon/libaxon_pjrt.so")
    mod = types.ModuleType("antenv.axon_hooks")
    mod.get_axon_ntff_profile_hook = lambda: hook
    mod.set_axon_ntff_profile_hook = lambda h: None
    sys.modules["antenv.axon_hooks"] = mod
    import antenv

    antenv.axon_hooks = mod
    import concourse.bass_utils as bu

    bu.upload_artifacts = lambda tmpdir: tmpdir


def _to_psT(pss, Ta):
    """[Ta, 1024] -> [128, nT, 8, TILE] with [p,t,k,a] = pss[t*TILE+a, k*128+p]"""
    nT = Ta // TILE
    return np.ascontiguousarray(pss.reshape(nT, TILE, 8, 128).transpose(3, 0, 2, 1))


def kernel(ps, numbers, batch, W_comp, W_psl, W_h1, W_h2, W_out):
    global LAST_EXEC_NS, LAST_RESULTS
    from concourse.bass_utils import run_bass_kernel_spmd

    if TRACE:
        _install_ntff_hook()

    fp8 = MODE == "fp8mix"
    DT = F8 if fp8 else np.float16
    KG = 4 if fp8 else 8
    KS = 2 if fp8 else 1

    ps = np.asarray(ps)
    numbers = np.asarray(numbers)
    batch = np.asarray(batch)
    W_comp = np.asarray(W_comp, dtype=np.float32)
    W_psl = np.asarray(W_psl, dtype=np.float32)
    W_h1 = np.asarray(W_h1, dtype=np.float32)
    W_h2 = np.asarray(W_h2, dtype=np.float32)
    W_out = np.asarray(W_out, dtype=np.float32)

    counts = np.bincount(batch, minlength=N_STRUCT)
    cum = np.zeros(N_STRUCT + 1, dtype=np.int64)
    np.cumsum(counts, out=cum[1:])

    # equal-structure shard cuts (atoms balance to ~1-2% by CLT; keeps
    # per-core structure count fixed at N_STRUCT/N_CORES <= SMAX)
    s_cut = [i * N_STRUCT // N_CORES for i in range(N_CORES + 1)]

    shards = []
    for i in range(N_CORES):
        s_lo, s_hi = s_cut[i], s_cut[i + 1]
        a_lo, a_hi = int(cum[s_lo]), int(cum[s_hi])
        n_at, n_st = a_hi - a_lo, s_hi - s_lo
        assert n_st <= SMAX, f"shard {i}: {n_st} structs > {SMAX}"
        shards.append((s_lo, s_hi, a_lo, a_hi, n_at, n_st))

    Ta = max(s[4] for s in shards)
    Ta = (Ta + TILE - 1) // TILE * TILE
    nT = Ta // TILE
    C = Ta // CHUNK

    # replicated weights: w1[p, j, m, i, c] = W_h1[m*128+c, (KS*j+i)*128+p]
    w1 = np.ascontiguousarray(
        W_h1.T.reshape(KG, KS, 128, 2, 128).transpose(2, 0, 3, 1, 4)
    ).astype(DT)
    # w2[p, m, i, c] = W_h2[m*128+c, i*128+p]  (fp16)
    w2 = np.ascontiguousarray(
        W_h2.T.reshape(2, 128, 2, 128).transpose(1, 2, 0, 3)
    ).astype(np.float16)
    # wout[p, i] = W_out[0, i*128+p]  (fp16)
    wout = np.ascontiguousarray(
        W_out[0].reshape(2, 128).T.reshape(128, 2, 1)
    ).astype(np.float16)
    # psl weights
    if fp8 and PSL_MODE == "hilo8":
        wp32 = W_psl[0].astype(np.float32)
        wp_hi8 = wp32.astype(F8)
        wp_lo8 = (wp32 - wp_hi8.astype(np.float32)).astype(F8)
        def pack(w):
            out = np.zeros((128, 4, 2, 16), dtype=F8)
            out[..., 0] = np.asarray(w).reshape(4, 2, 128).transpose(2, 0, 1)
            return out

        wpsl_hi, wpsl_lo = pack(wp_hi8), pack(wp_lo8)
    elif fp8 and PSL_MODE == "e3":
        t_w = 15.5 / max(float(np.abs(W_psl).max()), 1e-30)
        _WPSL_T = [t_w]
        wpsl16 = np.asarray(
            np.clip(W_psl[0].astype(np.float32) * t_w, -15.5, 15.5)
            .reshape(8, 128).T.reshape(128, 8, 1, 1),
            dtype=ml_dtypes.float8_e3m4,
        ).copy()
    else:
        wpsl16 = np.ascontiguousarray(
            W_psl[0].reshape(8, 128).T.reshape(128, 8, 1, 1)
        ).astype(np.float16)
    iota = np.ascontiguousarray(
        np.tile(np.arange(SMAX, dtype=np.float16), (128, 1))
    )
    # exact cubic through the 4 species energies
    V = np.vander(np.arange(N_SPECIES, dtype=np.float64), 4, increasing=True)
    poly = np.linalg.solve(V, W_comp[0, :N_SPECIES].astype(np.float64))

    # per-shard relative batch ids (padding = -1)
    rb_all = []
    for s_lo, s_hi, a_lo, a_hi, n_at, n_st in shards:
        rb = np.full(Ta, -1.0, dtype=np.float32)
        rb[:n_at] = (batch[a_lo:a_hi] - s_lo).astype(np.float32)
        rb_all.append(rb)

    segw = None
    if SEG_WINDOWED:
        # shared per-chunk structure windows: batch is sorted so each
        # 128-atom chunk only spans a few structures; bake [w0, w0+W)
        m0 = np.full(C, np.inf)
        m1 = np.full(C, -np.inf)
        for rb in rb_all:
            r2 = rb.reshape(C, CHUNK)
            mask = r2 >= 0
            has = mask.any(axis=1)
            lo = np.where(has, np.where(mask, r2, np.inf).min(axis=1), np.inf)
            hi = np.where(has, np.where(mask, r2, -np.inf).max(axis=1), -np.inf)
            m0 = np.minimum(m0, lo)
            m1 = np.maximum(m1, hi)
        w0 = np.where(np.isfinite(m0), m0, 0.0)
        span = np.where(np.isfinite(m1), m1 - w0 + 1, 1.0)
        W = int(max(1, span.max()))
        W = min((W + 3) // 4 * 4, SMAX)
        w0 = np.clip(w0, 0, SMAX - W).astype(np.int64)
        segw = (W, tuple(int(x) for x in w0))

    key = (Ta, C, tuple(np.round(poly, 12)), MODE, PSL_MODE, segw)
    if key not in _BUILD_CACHE:
        _BUILD_CACHE.clear()
        _BUILD_CACHE[key] = _build(Ta, C, poly, MODE, PSL_MODE, segw)
    nc = _BUILD_CACHE[key]

    psq = np.asarray(ps, dtype=DT)  # quantize once, slice per shard
    ones = np.ones((97, 1), dtype=np.float16)
    if fp8 and PSL_MODE == "hilo8":
        ps2 = (ps.astype(np.float32) - psq.astype(np.float32)).astype(F8)
    elif fp8 and PSL_MODE == "e3":
        # scale ps and wpsl into e3m4's normal range (subnormals would
        # cost ~8% error on small values); the 1/(s*t) unscale rides in
        # the colize ones vector rows 0:33 (psl partial rows 0/32)
        s_ps = 15.5 / max(float(np.abs(ps).max()), 1e-30)
        ps2 = np.asarray(
            np.clip(ps.astype(np.float32) * s_ps, -15.5, 15.5),
            dtype=ml_dtypes.float8_e3m4,
        )
        ones[0:33] = np.float16(1.0 / (s_ps * _WPSL_T[0]))
    elif fp8:
        ps2 = ps.astype(np.float16)

    in_maps = []
    for si, (s_lo, s_hi, a_lo, a_hi, n_at, n_st) in enumerate(shards):
        pss = np.zeros((Ta, N_FEAT), dtype=DT)
        pss[:n_at] = psq[a_lo:a_hi]
        rb = rb_all[si]
        if segw is not None:
            rb = rb.reshape(C, CHUNK) - np.asarray(segw[1], dtype=np.float32)[:, None]
            rb = rb.reshape(Ta)
        nums = np.zeros(Ta, dtype=np.float32)
        nums[:n_at] = numbers[a_lo:a_hi].astype(np.float32)
        im = {
            "psT": _to_psT(pss, Ta),
            "relb": np.ascontiguousarray(rb.reshape(C, CHUNK).T),
            "nums": np.ascontiguousarray(nums.reshape(C, CHUNK).T),
            "w1": w1,
            "w2": w2,
            "wout": wout,
            "ones": ones,
            "iota": iota,
        }
        if fp8 and PSL_MODE == "hilo8":
            pss2 = np.zeros((Ta, N_FEAT), dtype=F8)
            pss2[:n_at] = ps2[a_lo:a_hi]
            im["psT2"] = _to_psT(pss2, Ta)
            im["wpsl_hi"] = wpsl_hi
            im["wpsl_lo"] = wpsl_lo
        elif fp8:
            pss2 = np.zeros((Ta, N_FEAT), dtype=ps2.dtype)
            pss2[:n_at] = ps2[a_lo:a_hi]
            im["psT2"] = _to_psT(pss2, Ta)
            im["wpsl"] = wpsl16
        else:
            im["wpsl"] = wpsl16
        in_maps.append(im)

    res = run_bass_kernel_spmd(nc, in_maps, list(range(N_CORES)), trace=TRACE)
    LAST_EXEC_NS = res.exec_time_ns
    LAST_RESULTS = res

    out = np.zeros((N_STRUCT, 1), dtype=np.float32)
    for i, (s_lo, s_hi, a_lo, a_hi, n_at, n_st) in enumerate(shards):
        vals = res.results[i]["out"][0, :n_st].astype(np.float32)
        empty = counts[s_lo:s_hi] == 0
        if empty.any():
            vals = np.where(empty, 0.0, vals)
        out[s_lo:s_hi, 0] = vals
    return out


# revision 19
# speedup vs baseline: 1.0577x; 1.0577x over previous
"""PowerSpectrumModel Trainium2 kernel (8 NeuronCores, SPMD).

Strategy (data-parallel over atoms, structures disjoint per shard):
 - Host: cut the atom axis at structure boundaries into 8 balanced shards;
   pre-TRANSPOSE ps to feature-major [128, nT, 8, TILE] (so every tile
   load is one contiguous stride per partition — no on-device transpose
   descriptors); fp32 PSUM accumulation on device; replicate the small
   weight matrices.
 - Precision (fp8mix mode): the big h1 GEMM runs fp8e4m3 DoubleRow (2x
   PE rate; w1-quant + ps-quant contribute only ~6e-3 rel err because
   errors decorrelate across atoms); h2/psnn stay fp16 (their weight
   quantization biases the per-atom energy coherently across a structure
   -> 3.6e-2 if fp8); psl gets hi+lo fp8 splits of both ps and W_psl
   (or an extra fp16 psT copy, PSL_MODE).
 - Device, per 512-atom tile:
     psT   <- contiguous loads [128, 8, 512]                      [DMA]
     h1    = W_h1 @ psT  (fp8 DoubleRow k-pairs / fp16 k-chunks)  [PE]
     psl   = W_psl row, M=1 matmuls -> e partial rows 0/32/64/96  [PE]
     sil1  = silu(h1)                                             [ACT]
     h2    = W_h2 @ sil1; sil2 = silu(h2)                         [PE/ACT]
     psnn  M=1 matmuls accumulated onto the psl PSUM rows         [PE]
     per 128-chunk: e column via K=97 ones matmul, + species
     energy, one-hot(struct) matmul accumulates into a [1,256]
     PSUM row holding this core's per-structure energies.         [PE/DVE]
 - Host: slice per-core structure ranges, concat -> [2000, 1].
"""

import numpy as np
import ml_dtypes

N_ATOMS = 200000
N_FEAT = 1024
N_SPECIES = 4
N_STRUCT = 2000
H1 = 256
H2 = 256
SCALE = 1.0
N_CORES = 8
TILE = 512
CHUNK = 128
SMAX = 256  # per-core structure capacity (PSUM row)

MODE = "fp8mix"  # "fp16" | "fp8mix"
PSL_MODE = "fp16"  # "e3" | "hilo8" | "fp16"
SEG_WINDOWED = True  # bake per-chunk structure windows into the seg matmul

_BUILD_CACHE = {}
TRACE = False
LAST_EXEC_NS = None
LAST_RESULTS = None

F8 = ml_dtypes.float8_e4m3


def _split_waits(nc, mybir, maxw=1):
    """walrus on this build rejects >1 sync wait per instruction; move
    overflow waits onto preceding same-engine NoOps."""
    cnt = 0
    for f in nc.m.functions:
        for blk in f.blocks:
            if not hasattr(blk, "instructions"):
                continue
            out = []
            changed = False
            for inst in blk.instructions:
                si = getattr(inst, "sync_info", None)
                if si is not None and si.on_wait and len(si.on_wait) > maxw:
                    waits = list(si.on_wait)
                    keep = waits[-maxw:]
                    extra = waits[:-maxw]
                    while extra:
                        chunk, extra = extra[:maxw], extra[maxw:]
                        cnt += 1
                        out.append(
                            mybir.InstNoOp(
                                name=f"waitfix-{cnt}",
                                engine=inst.engine,
                                text_hint="waitfix",
                                bass_nofuse=True,
                                ins=[],
                                outs=[],
                                sync_info=mybir.SyncInfo(on_wait=chunk, on_update=[]),
                            )
                        )
                    si.on_wait = keep
                    changed = True
                out.append(inst)
            if changed:
                blk.instructions[:] = out
    return cnt


def _build(Ta, C, poly, mode, psl_mode, segw=None):
    import concourse.bass as bass
    import concourse.tile as tile
    import concourse.mybir as mybir
    from contextlib import ExitStack

    fp8 = mode == "fp8mix"
    f8 = mybir.dt.float8e4
    f16 = mybir.dt.float16
    f32 = mybir.dt.float32
    fd = f8 if fp8 else f16  # h1 GEMM dtype
    AF = mybir.ActivationFunctionType
    ALU = mybir.AluOpType
    DR = mybir.MatmulPerfMode.DoubleRow if fp8 else None
    PSUM = bass.MemorySpace.PSUM
    nT = Ta // TILE
    c0, c1, c2, c3 = (float(x) for x in poly)
    KG = 4 if fp8 else 8  # h1 k-groups (DoubleRow contracts pairs)
    KS = 2 if fp8 else 1  # k-chunks per h1 matmul

    nc = bass.Bass("TRN2", target_bir_lowering=False, debug=False)

    # pre-transposed ps: [p, t, k, a] = ps[t*512+a, k*128+p]
    psT_d = nc.dram_tensor("psT", [128, nT, 8, TILE], fd, kind="ExternalInput").ap()
    f8e3 = mybir.dt.float8e3
    psl2_d = None
    if fp8:
        d2 = {"hilo8": f8, "e3": f8e3, "fp16": f16}[psl_mode]
        psl2_d = nc.dram_tensor("psT2", [128, nT, 8, TILE], d2, kind="ExternalInput").ap()
    relb_d = nc.dram_tensor("relb", [CHUNK, C], f32, kind="ExternalInput").ap()
    nums_d = nc.dram_tensor("nums", [CHUNK, C], f32, kind="ExternalInput").ap()
    # w1: [p, j, m, i, c] = W_h1[m*128+c, (KS*j+i)*128+p]
    w1_d = nc.dram_tensor("w1", [128, KG, 2, KS, 128], fd, kind="ExternalInput").ap()
    # w2: [p, m, i, c] = W_h2[m*128+c, i*128+p]  (fp16)
    w2_d = nc.dram_tensor("w2", [128, 2, 2, 128], f16, kind="ExternalInput").ap()
    # wout: [p, i] = W_out[0, i*128+p]  (fp16)
    wout_d = nc.dram_tensor("wout", [128, 2, 1], f16, kind="ExternalInput").ap()
    # psl weights
    if fp8 and psl_mode == "hilo8":
        # k-subtile dim padded to step 16 (ISA: dual-fp8 LW step%16==0)
        wpa_d = nc.dram_tensor("wpsl_hi", [128, 4, 2, 16], f8, kind="ExternalInput").ap()
        wpb_d = nc.dram_tensor("wpsl_lo", [128, 4, 2, 16], f8, kind="ExternalInput").ap()
    elif fp8 and psl_mode == "e3":
        wpa_d = nc.dram_tensor("wpsl", [128, 8, 1, 1], f8e3, kind="ExternalInput").ap()
        wpb_d = None
    else:
        wpa_d = nc.dram_tensor("wpsl", [128, 8, 1, 1], f16, kind="ExternalInput").ap()
        wpb_d = None
    ones_d = nc.dram_tensor("ones", [97, 1], f16, kind="ExternalInput").ap()
    iota_d = nc.dram_tensor("iota", [128, SMAX], f16, kind="ExternalInput").ap()
    out_d = nc.dram_tensor("out", [1, SMAX], f32, kind="ExternalOutput").ap()

    with tile.TileContext(nc) as tc, ExitStack() as ctx:
        const = ctx.enter_context(tc.tile_pool(name="const", bufs=1))
        psTp = ctx.enter_context(tc.tile_pool(name="psT", bufs=6))
        psTp2 = ctx.enter_context(tc.tile_pool(name="psT2", bufs=6)) if fp8 else None
        silp = ctx.enter_context(tc.tile_pool(name="sil", bufs=4))
        rowp = ctx.enter_context(tc.tile_pool(name="row", bufs=4))
        pp_h1 = ctx.enter_context(tc.tile_pool(name="pph1", bufs=1, space=PSUM))
        pp_e = ctx.enter_context(tc.tile_pool(name="ppe", bufs=1, space=PSUM))
        pp_ec = ctx.enter_context(tc.tile_pool(name="ppec", bufs=2, space=PSUM))
        pp_seg = ctx.enter_context(tc.tile_pool(name="ppseg", bufs=1, space=PSUM))

        # ---- constants ----
        w1_sb = const.tile([128, KG, 2, KS, 128], fd, tag="w1")
        nc.sync.dma_start(w1_sb[:], w1_d[:])
        w2_sb = const.tile([128, 2, 2, 128], f16, tag="w2")
        nc.sync.dma_start(w2_sb[:], w2_d[:])
        wout_sb = const.tile([128, 2, 1], f16, tag="wout")
        nc.sync.dma_start(wout_sb[:], wout_d[:])
        if fp8 and psl_mode == "hilo8":
            wpa_sb = const.tile([128, 4, 2, 16], f8, tag="wpa")
            nc.sync.dma_start(wpa_sb[:], wpa_d[:])
            wpb_sb = const.tile([128, 4, 2, 16], f8, tag="wpb")
            nc.sync.dma_start(wpb_sb[:], wpb_d[:])
        else:
            wpa_sb = const.tile([128, 8, 1, 1], f8e3 if (fp8 and psl_mode == "e3") else f16, tag="wpa")
            nc.sync.dma_start(wpa_sb[:], wpa_d[:])
            wpb_sb = None
        iota_sb = const.tile([128, SMAX], f16, tag="iota")
        nc.sync.dma_start(iota_sb[:], iota_d[:])
        relb_sb = const.tile([CHUNK, C], f32, tag="relb")
        nc.sync.dma_start(relb_sb[:], relb_d[:])
        nums_sb = const.tile([CHUNK, C], f32, tag="nums")
        nc.sync.dma_start(nums_sb[:], nums_d[:])
        ones_sb = const.tile([97, 1], f16, tag="ones")
        nc.sync.dma_start(ones_sb[:], ones_d[:])

        # species energy per atom: cubic through W_comp[0, 0..3]
        # comp = (c1*n + c0) + n*n*(c3*n + c2)
        t_n2 = const.tile([CHUNK, C], f32, tag="t_n2")
        nc.vector.tensor_mul(t_n2[:], nums_sb[:], nums_sb[:])
        t_a = const.tile([CHUNK, C], f32, tag="t_a")
        nc.vector.tensor_scalar(t_a[:], nums_sb[:], c3, c2, ALU.mult, ALU.add)
        t_b = const.tile([CHUNK, C], f32, tag="t_b")
        nc.vector.tensor_mul(t_b[:], t_n2[:], t_a[:])
        t_c = const.tile([CHUNK, C], f32, tag="t_c")
        nc.vector.tensor_scalar(t_c[:], nums_sb[:], c1, c0, ALU.mult, ALU.add)
        comp_sb = const.tile([CHUNK, C], f32, tag="comp")
        nc.vector.tensor_add(comp_sb[:], t_b[:], t_c[:])

        seg_ps = pp_seg.tile([1, SMAX], f32, tag="seg")
        nc.vector.memset(seg_ps[:], 0.0)
        # e-partials banks (double-buffered so tile t+1's psl does not
        # wait on tile t's e_row copy): partial rows 0/32/64/96; the rows
        # in between stay 0 from this one-time clear, so a K=97
        # ones-matmul sums the partials.
        e_bufs = []
        for i in range(2):
            eb = pp_e.tile([128, TILE], f32, tag=f"e{i}", name=f"e{i}")
            nc.vector.memset(eb[:], 0.0)
            e_bufs.append(eb)
        # one-hot windows are generated a few tiles ahead of use (OH_PRE)
        # so the seg matmul never waits on the DVE mid-tile, without
        # clogging the in-order DVE queue upfront
        oh_all = None
        OH_PRE = 3
        if segw is not None:
            W = segw[0]
            oh_all = const.tile([128, C, W], f16, tag="oh_all")

        def gen_oh(ch):
            if oh_all is not None and ch < C:
                nc.vector.tensor_scalar(
                    oh_all[:, ch], iota_sb[:, 0:W], relb_sb[:, ch : ch + 1],
                    None, ALU.is_equal,
                )

        for ch in range(4 * OH_PRE):
            gen_oh(ch)

        for t in range(nT):
            e_ps = e_bufs[t % 2]
            for cc in range(4):
                gen_oh((t + OH_PRE) * 4 + cc)
            # ---- contiguous loads: [128, 8, 512]
            big = psTp.tile([128, 8, TILE], fd, tag="psT", name=f"psT{t}")
            nc.sync.dma_start(big[:], psT_d[:, t])
            if fp8:
                d2 = {"hilo8": f8, "e3": f8e3, "fp16": f16}[psl_mode]
                big2 = psTp2.tile([128, 8, TILE], d2, tag="psT2", name=f"psT2_{t}")
                nc.sync.dma_start(big2[:], psl2_d[:, t])

            # ---- h1: KG k-groups x 2 M-chunks
            h1ps = [pp_h1.tile([128, TILE], f32, tag=f"h1m{m}", name=f"h1ps{t}_{m}") for m in range(2)]
            for j in range(KG):
                for m in range(2):
                    nc.tensor.matmul(
                        h1ps[m][:],
                        w1_sb[:, j, m],
                        big[:, KS * j : KS * (j + 1)],
                        start=(j == 0),
                        stop=(j == KG - 1),
                        perf_mode=DR,
                    )
            # ---- psl: M=1 matmuls in column groups, partial sums landing
            # on e_ps rows 0/32/64/96.
            if fp8 and psl_mode == "hilo8":
                # hi*w_hi + lo*w_hi + hi*w_lo, all fp8 DoubleRow.
                # DoubleRow forbids col-group tiling (col_grp must be 0xf)
                # so every partial accumulates onto e_ps row 0.
                for cn, (wp, mv) in enumerate(
                    [(wpa_sb, big), (wpa_sb, big2), (wpb_sb, big)]
                ):
                    for j in range(4):
                        nc.tensor.matmul(
                            e_ps[0:1, :],
                            wp[:, j, :, 0:1],
                            mv[:, 2 * j : 2 * j + 2],
                            start=(cn == 0 and j == 0),
                            stop=False,
                            perf_mode=DR,
                        )
            elif fp8:
                # psl from the scaled-e3m4/fp16 psT copy: fast M=1 matmuls
                # in column groups 0/1, partials on e_ps rows 0/32 (the
                # 1/(s*t) unscale rides in the ones vector rows 0:33)
                for k in range(8):
                    g = 32 * (k % 2)
                    nc.tensor.matmul(
                        e_ps[g : g + 1, :],
                        wpa_sb[:, k],
                        big2[:, k],
                        start=(k < 2),
                        stop=False,
                        tile_position=(0, g),
                    )
            else:
                for k in range(8):
                    g = 32 * (k % 4)
                    nc.tensor.matmul(
                        e_ps[g : g + 1, :],
                        wpa_sb[:, k],
                        big[:, k],
                        start=(k < 4),
                        stop=False,
                        tile_position=(0, g),
                    )

            sil1 = silp.tile([128, 2, TILE], f16, tag="sil1")
            for m in range(2):
                nc.scalar.activation(sil1[:, m], h1ps[m][:], AF.Silu)

            # ---- h2 (fp16)
            h2ps = [pp_h1.tile([128, TILE], f32, tag=f"h1m{m}", name=f"h2ps{t}_{m}") for m in range(2)]
            for kj in range(2):
                for m in range(2):
                    nc.tensor.matmul(
                        h2ps[m][:],
                        w2_sb[:, m, kj],
                        sil1[:, kj],
                        start=(kj == 0),
                        stop=(kj == 1),
                    )
            sil2 = silp.tile([128, 2, TILE], f16, tag="sil2")
            for m in range(2):
                nc.scalar.activation(sil2[:, m], h2ps[m][:], AF.Silu)

            # ---- psnn (fp16). In fp8 modes psl only occupies rows 0(/32)
            # so psnn gets its own fresh rows 64/96 (unit-scaled in the
            # ones vector); in fp16 mode it accumulates onto the psl rows.
            for kj in range(2):
                g = (64 + 32 * kj) if fp8 else (32 * kj)
                nc.tensor.matmul(
                    e_ps[g : g + 1, :],
                    wout_sb[:, kj],
                    sil2[:, kj],
                    start=fp8,
                    stop=(kj == 1),
                    tile_position=(0, g),
                )
            # partial rows 0/32/64/96 (zeros between) -> SBUF in one copy
            e_row = rowp.tile([97, TILE], f16, tag="erow")
            nc.vector.tensor_copy(e_row[:], e_ps[0:97, :])

            # ---- per-chunk: column-ize (K=97 sums the partials), add
            # species energy, segment matmul
            for cc in range(4):
                ch = t * 4 + cc
                ec_ps = pp_ec.tile([128, 1], f32, tag="ec")
                nc.tensor.matmul(
                    ec_ps[:],
                    e_row[0:97, cc * 128 : (cc + 1) * 128],
                    ones_sb[:],
                    start=True,
                    stop=True,
                )
                e_col = rowp.tile([128, 1], f16, tag="ecol")
                nc.vector.tensor_add(e_col[:], ec_ps[:], comp_sb[:, ch : ch + 1])
                if segw is not None:
                    # host guarantees this chunk's structures fall inside
                    # [w0, w0+W); relb is pre-shifted by -w0 per chunk
                    W, w0s = segw
                    w0 = w0s[ch]
                    nc.tensor.matmul(
                        seg_ps[0:1, w0 : w0 + W], e_col[:], oh_all[:, ch],
                        start=False, stop=(ch == C - 1), skip_group_check=True,
                    )
                else:
                    oh = rowp.tile([128, SMAX], f16, tag="oh")
                    nc.vector.tensor_scalar(
                        oh[:], iota_sb[:], relb_sb[:, ch : ch + 1], None, ALU.is_equal
                    )
                    nc.tensor.matmul(
                        seg_ps[:], e_col[:], oh[:], start=(ch == 0), stop=(ch == C - 1)
                    )

        out_sb = rowp.tile([1, SMAX], f32, tag="outsb")
        nc.scalar.activation(out_sb[:], seg_ps[:], AF.Copy)
        nc.sync.dma_start(out_d[:], out_sb[:])

    _split_waits(nc, mybir)
    return nc


def _install_ntff_hook():
    """Register the axon NTFF profile hook (missing antenv.axon_hooks in
    this image) so run_bass_kernel_spmd(trace=True) can report exec_time_ns."""
    import sys
    import types

    try:
        import antenv.axon_hooks  # noqa: F401

        return
    except ImportError:
        pass
    from trn_agent_boot.trn_boot import _ntff_profile_via_ctypes

    hook = _ntff_profile_via_ctypes("/opt/axon/libaxon_pjrt.so")
    mod = types.ModuleType("antenv.axon_hooks")
    mod.get_axon_ntff_profile_hook = lambda: hook
    mod.set_axon_ntff_profile_hook = lambda h: None
    sys.modules["antenv.axon_hooks"] = mod
    import antenv

    antenv.axon_hooks = mod
    import concourse.bass_utils as bu

    bu.upload_artifacts = lambda tmpdir: tmpdir


def _to_psT(pss, Ta):
    """[Ta, 1024] -> [128, nT, 8, TILE] with [p,t,k,a] = pss[t*TILE+a, k*128+p]"""
    nT = Ta // TILE
    return np.ascontiguousarray(pss.reshape(nT, TILE, 8, 128).transpose(3, 0, 2, 1))


def kernel(ps, numbers, batch, W_comp, W_psl, W_h1, W_h2, W_out):
    global LAST_EXEC_NS, LAST_RESULTS
    from concourse.bass_utils import run_bass_kernel_spmd

    if TRACE:
        _install_ntff_hook()

    fp8 = MODE == "fp8mix"
    DT = F8 if fp8 else np.float16
    KG = 4 if fp8 else 8
    KS = 2 if fp8 else 1

    ps = np.asarray(ps)
    numbers = np.asarray(numbers)
    batch = np.asarray(batch)
    W_comp = np.asarray(W_comp, dtype=np.float32)
    W_psl = np.asarray(W_psl, dtype=np.float32)
    W_h1 = np.asarray(W_h1, dtype=np.float32)
    W_h2 = np.asarray(W_h2, dtype=np.float32)
    W_out = np.asarray(W_out, dtype=np.float32)

    counts = np.bincount(batch, minlength=N_STRUCT)
    cum = np.zeros(N_STRUCT + 1, dtype=np.int64)
    np.cumsum(counts, out=cum[1:])

    # equal-structure shard cuts (atoms balance to ~1-2% by CLT; keeps
    # per-core structure count fixed at N_STRUCT/N_CORES <= SMAX)
    s_cut = [i * N_STRUCT // N_CORES for i in range(N_CORES + 1)]

    shards = []
    for i in range(N_CORES):
        s_lo, s_hi = s_cut[i], s_cut[i + 1]
        a_lo, a_hi = int(cum[s_lo]), int(cum[s_hi])
        n_at, n_st = a_hi - a_lo, s_hi - s_lo
        assert n_st <= SMAX, f"shard {i}: {n_st} structs > {SMAX}"
        shards.append((s_lo, s_hi, a_lo, a_hi, n_at, n_st))

    Ta = max(s[4] for s in shards)
    Ta = (Ta + TILE - 1) // TILE * TILE
    nT = Ta // TILE
    C = Ta // CHUNK

    # replicated weights: w1[p, j, m, i, c] = W_h1[m*128+c, (KS*j+i)*128+p]
    w1 = np.ascontiguousarray(
        W_h1.T.reshape(KG, KS, 128, 2, 128).transpose(2, 0, 3, 1, 4)
    ).astype(DT)
    # w2[p, m, i, c] = W_h2[m*128+c, i*128+p]  (fp16)
    w2 = np.ascontiguousarray(
        W_h2.T.reshape(2, 128, 2, 128).transpose(1, 2, 0, 3)
    ).astype(np.float16)
    # wout[p, i] = W_out[0, i*128+p]  (fp16)
    wout = np.ascontiguousarray(
        W_out[0].reshape(2, 128).T.reshape(128, 2, 1)
    ).astype(np.float16)
    # psl weights
    if fp8 and PSL_MODE == "hilo8":
        wp32 = W_psl[0].astype(np.float32)
        wp_hi8 = wp32.astype(F8)
        wp_lo8 = (wp32 - wp_hi8.astype(np.float32)).astype(F8)
        def pack(w):
            out = np.zeros((128, 4, 2, 16), dtype=F8)
            out[..., 0] = np.asarray(w).reshape(4, 2, 128).transpose(2, 0, 1)
            return out

        wpsl_hi, wpsl_lo = pack(wp_hi8), pack(wp_lo8)
    elif fp8 and PSL_MODE == "e3":
        t_w = 15.5 / max(float(np.abs(W_psl).max()), 1e-30)
        _WPSL_T = [t_w]
        wpsl16 = np.asarray(
            np.clip(W_psl[0].astype(np.float32) * t_w, -15.5, 15.5)
            .reshape(8, 128).T.reshape(128, 8, 1, 1),
            dtype=ml_dtypes.float8_e3m4,
        ).copy()
    else:
        wpsl16 = np.ascontiguousarray(
            W_psl[0].reshape(8, 128).T.reshape(128, 8, 1, 1)
        ).astype(np.float16)
    iota = np.ascontiguousarray(
        np.tile(np.arange(SMAX, dtype=np.float16), (128, 1))
    )
    # exact cubic through the 4 species energies
    V = np.vander(np.arange(N_SPECIES, dtype=np.float64), 4, increasing=True)
    poly = np.linalg.solve(V, W_comp[0, :N_SPECIES].astype(np.float64))

    # per-shard relative batch ids (padding = -1)
    rb_all = []
    for s_lo, s_hi, a_lo, a_hi, n_at, n_st in shards:
        rb = np.full(Ta, -1.0, dtype=np.float32)
        rb[:n_at] = (batch[a_lo:a_hi] - s_lo).astype(np.float32)
        rb_all.append(rb)

    segw = None
    if SEG_WINDOWED:
        # shared per-chunk structure windows: batch is sorted so each
        # 128-atom chunk only spans a few structures; bake [w0, w0+W)
        m0 = np.full(C, np.inf)
        m1 = np.full(C, -np.inf)
        for rb in rb_all:
            r2 = rb.reshape(C, CHUNK)
            mask = r2 >= 0
            has = mask.any(axis=1)
            lo = np.where(has, np.where(mask, r2, np.inf).min(axis=1), np.inf)
            hi = np.where(has, np.where(mask, r2, -np.inf).max(axis=1), -np.inf)
            m0 = np.minimum(m0, lo)
            m1 = np.maximum(m1, hi)
        w0 = np.where(np.isfinite(m0), m0, 0.0)
        span = np.where(np.isfinite(m1), m1 - w0 + 1, 1.0)
        W = int(max(1, span.max()))
        W = min((W + 3) // 4 * 4, SMAX)
        w0 = np.clip(w0, 0, SMAX - W).astype(np.int64)
        segw = (W, tuple(int(x) for x in w0))

    key = (Ta, C, tuple(np.round(poly, 12)), MODE, PSL_MODE, segw)
    if key not in _BUILD_CACHE:
        _BUILD_CACHE.clear()
        _BUILD_CACHE[key] = _build(Ta, C, poly, MODE, PSL_MODE, segw)
    nc = _BUILD_CACHE[key]

    psq = np.asarray(ps, dtype=DT)  # quantize once, slice per shard
    ones = np.ones((97, 1), dtype=np.float16)
    if fp8 and PSL_MODE == "hilo8":
        ps2 = (ps.astype(np.float32) - psq.astype(np.float32)).astype(F8)
    elif fp8 and PSL_MODE == "e3":
        # scale ps and wpsl into e3m4's normal range (subnormals would
        # cost ~8% error on small values); the 1/(s*t) unscale rides in
        # the colize ones vector rows 0:33 (psl partial rows 0/32)
        s_ps = 15.5 / max(float(np.abs(ps).max()), 1e-30)
        ps2 = np.asarray(
            np.clip(ps.astype(np.float32) * s_ps, -15.5, 15.5),
            dtype=ml_dtypes.float8_e3m4,
        )
        ones[0:33] = np.float16(1.0 / (s_ps * _WPSL_T[0]))
    elif fp8:
        ps2 = ps.astype(np.float16)

    in_maps = []
    for si, (s_lo, s_hi, a_lo, a_hi, n_at, n_st) in enumerate(shards):
        pss = np.zeros((Ta, N_FEAT), dtype=DT)
        pss[:n_at] = psq[a_lo:a_hi]
        rb = rb_all[si]
        if segw is not None:
            rb = rb.reshape(C, CHUNK) - np.asarray(segw[1], dtype=np.float32)[:, None]
            rb = rb.reshape(Ta)
        nums = np.zeros(Ta, dtype=np.float32)
        nums[:n_at] = numbers[a_lo:a_hi].astype(np.float32)
        im = {
            "psT": _to_psT(pss, Ta),
            "relb": np.ascontiguousarray(rb.reshape(C, CHUNK).T),
            "nums": np.ascontiguousarray(nums.reshape(C, CHUNK).T),
            "w1": w1,
            "w2": w2,
            "wout": wout,
            "ones": ones,
            "iota": iota,
        }
        if fp8 and PSL_MODE == "hilo8":
            pss2 = np.zeros((Ta, N_FEAT), dtype=F8)
            pss2[:n_at] = ps2[a_lo:a_hi]
            im["psT2"] = _to_psT(pss2, Ta)
            im["wpsl_hi"] = wpsl_hi
            im["wpsl_lo"] = wpsl_lo
        elif fp8:
            pss2 = np.zeros((Ta, N_FEAT), dtype=ps2.dtype)
            pss2[:n_at] = ps2[a_lo:a_hi]
            im["psT2"] = _to_psT(pss2, Ta)
            im["wpsl"] = wpsl16
        else:
            im["wpsl"] = wpsl16
        in_maps.append(im)

    res = run_bass_kernel_spmd(nc, in_maps, list(range(N_CORES)), trace=TRACE)
    LAST_EXEC_NS = res.exec_time_ns
    LAST_RESULTS = res

    out = np.zeros((N_STRUCT, 1), dtype=np.float32)
    for i, (s_lo, s_hi, a_lo, a_hi, n_at, n_st) in enumerate(shards):
        vals = res.results[i]["out"][0, :n_st].astype(np.float32)
        empty = counts[s_lo:s_hi] == 0
        if empty.any():
            vals = np.where(empty, 0.0, vals)
        out[s_lo:s_hi, 0] = vals
    return out


# revision 21
# speedup vs baseline: 1.1131x; 1.0524x over previous
"""PowerSpectrumModel Trainium2 kernel (8 NeuronCores, SPMD).

Strategy (data-parallel over atoms, structures disjoint per shard):
 - Host: cut the atom axis at structure boundaries into 8 balanced shards;
   pre-TRANSPOSE ps to feature-major [128, nT, 8, TILE] (so every tile
   load is one contiguous stride per partition — no on-device transpose
   descriptors); fp32 PSUM accumulation on device; replicate the small
   weight matrices.
 - Precision (fp8mix mode): the big h1 GEMM runs fp8e4m3 DoubleRow (2x
   PE rate; w1-quant + ps-quant contribute only ~6e-3 rel err because
   errors decorrelate across atoms); h2/psnn stay fp16 (their weight
   quantization biases the per-atom energy coherently across a structure
   -> 3.6e-2 if fp8); psl gets hi+lo fp8 splits of both ps and W_psl
   (or an extra fp16 psT copy, PSL_MODE).
 - Device, per 512-atom tile:
     psT   <- contiguous loads [128, 8, 512]                      [DMA]
     h1    = W_h1 @ psT  (fp8 DoubleRow k-pairs / fp16 k-chunks)  [PE]
     psl   = W_psl row, M=1 matmuls -> e partial rows 0/32/64/96  [PE]
     sil1  = silu(h1)                                             [ACT]
     h2    = W_h2 @ sil1; sil2 = silu(h2)                         [PE/ACT]
     psnn  M=1 matmuls accumulated onto the psl PSUM rows         [PE]
     per 128-chunk: e column via K=97 ones matmul, + species
     energy, one-hot(struct) matmul accumulates into a [1,256]
     PSUM row holding this core's per-structure energies.         [PE/DVE]
 - Host: slice per-core structure ranges, concat -> [2000, 1].
"""

import numpy as np
import ml_dtypes

N_ATOMS = 200000
N_FEAT = 1024
N_SPECIES = 4
N_STRUCT = 2000
H1 = 256
H2 = 256
SCALE = 1.0
N_CORES = 8
TILE = 512
CHUNK = 128
SMAX = 256  # per-core structure capacity (PSUM row)

MODE = "fp8mix"  # "fp16" | "fp8mix"
PSL_MODE = "fp16"  # "e3" | "hilo8" | "fp16"
SEG_WINDOWED = True  # bake per-chunk structure windows into the seg matmul

_BUILD_CACHE = {}
TRACE = False
LAST_EXEC_NS = None
LAST_RESULTS = None

F8 = ml_dtypes.float8_e4m3


def _split_waits(nc, mybir, maxw=1):
    """walrus on this build rejects >1 sync wait per instruction; move
    overflow waits onto preceding same-engine NoOps."""
    cnt = 0
    for f in nc.m.functions:
        for blk in f.blocks:
            if not hasattr(blk, "instructions"):
                continue
            out = []
            changed = False
            for inst in blk.instructions:
                si = getattr(inst, "sync_info", None)
                if si is not None and si.on_wait and len(si.on_wait) > maxw:
                    waits = list(si.on_wait)
                    keep = waits[-maxw:]
                    extra = waits[:-maxw]
                    while extra:
                        chunk, extra = extra[:maxw], extra[maxw:]
                        cnt += 1
                        out.append(
                            mybir.InstNoOp(
                                name=f"waitfix-{cnt}",
                                engine=inst.engine,
                                text_hint="waitfix",
                                bass_nofuse=True,
                                ins=[],
                                outs=[],
                                sync_info=mybir.SyncInfo(on_wait=chunk, on_update=[]),
                            )
                        )
                    si.on_wait = keep
                    changed = True
                out.append(inst)
            if changed:
                blk.instructions[:] = out
    return cnt


def _build(Ta, C, poly, mode, psl_mode, segw=None):
    import concourse.bass as bass
    import concourse.tile as tile
    import concourse.mybir as mybir
    from contextlib import ExitStack

    fp8 = mode == "fp8mix"
    f8 = mybir.dt.float8e4
    f16 = mybir.dt.float16
    f32 = mybir.dt.float32
    fd = f8 if fp8 else f16  # h1 GEMM dtype
    AF = mybir.ActivationFunctionType
    ALU = mybir.AluOpType
    DR = mybir.MatmulPerfMode.DoubleRow if fp8 else None
    PSUM = bass.MemorySpace.PSUM
    nT = Ta // TILE
    c0, c1, c2, c3 = (float(x) for x in poly)
    KG = 4 if fp8 else 8  # h1 k-groups (DoubleRow contracts pairs)
    KS = 2 if fp8 else 1  # k-chunks per h1 matmul

    nc = bass.Bass("TRN2", target_bir_lowering=False, debug=False)

    # pre-transposed ps: [p, t, k, a] = ps[t*512+a, k*128+p]
    psT_d = nc.dram_tensor("psT", [128, nT, 8, TILE], fd, kind="ExternalInput").ap()
    f8e3 = mybir.dt.float8e3
    psl2_d = None
    if fp8:
        d2 = {"hilo8": f8, "e3": f8e3, "fp16": f16}[psl_mode]
        psl2_d = nc.dram_tensor("psT2", [128, nT, 8, TILE], d2, kind="ExternalInput").ap()
    relb_d = nc.dram_tensor("relb", [CHUNK, C], f32, kind="ExternalInput").ap()
    nums_d = nc.dram_tensor("nums", [CHUNK, C], f32, kind="ExternalInput").ap()
    # w1: [p, j, m, i, c] = W_h1[m*128+c, (KS*j+i)*128+p]
    w1_d = nc.dram_tensor("w1", [128, KG, 2, KS, 128], fd, kind="ExternalInput").ap()
    # w2: [p, m, i, c] = W_h2[m*128+c, i*128+p]  (fp16)
    w2_d = nc.dram_tensor("w2", [128, 2, 2, 128], f16, kind="ExternalInput").ap()
    # wout: [p, i] = W_out[0, i*128+p]  (fp16)
    wout_d = nc.dram_tensor("wout", [128, 2, 1], f16, kind="ExternalInput").ap()
    # psl weights
    if fp8 and psl_mode == "hilo8":
        # k-subtile dim padded to step 16 (ISA: dual-fp8 LW step%16==0)
        wpa_d = nc.dram_tensor("wpsl_hi", [128, 4, 2, 16], f8, kind="ExternalInput").ap()
        wpb_d = nc.dram_tensor("wpsl_lo", [128, 4, 2, 16], f8, kind="ExternalInput").ap()
    elif fp8 and psl_mode == "e3":
        wpa_d = nc.dram_tensor("wpsl", [128, 8, 1, 1], f8e3, kind="ExternalInput").ap()
        wpb_d = None
    else:
        wpa_d = nc.dram_tensor("wpsl", [128, 8, 1, 1], f16, kind="ExternalInput").ap()
        wpb_d = None
    ones_d = nc.dram_tensor("ones", [97, 1], f16, kind="ExternalInput").ap()
    iota_d = nc.dram_tensor("iota", [128, SMAX], f16, kind="ExternalInput").ap()
    out_d = nc.dram_tensor("out", [1, SMAX], f32, kind="ExternalOutput").ap()

    with tile.TileContext(nc) as tc, ExitStack() as ctx:
        const = ctx.enter_context(tc.tile_pool(name="const", bufs=1))
        psTp = ctx.enter_context(tc.tile_pool(name="psT", bufs=6))
        psTp2 = ctx.enter_context(tc.tile_pool(name="psT2", bufs=6)) if fp8 else None
        silp = ctx.enter_context(tc.tile_pool(name="sil", bufs=4))
        rowp = ctx.enter_context(tc.tile_pool(name="row", bufs=4))
        pp_h1 = ctx.enter_context(tc.tile_pool(name="pph1", bufs=1, space=PSUM))
        pp_h2 = ctx.enter_context(tc.tile_pool(name="pph2", bufs=1, space=PSUM))
        pp_e = ctx.enter_context(tc.tile_pool(name="ppe", bufs=1, space=PSUM))
        pp_ec = ctx.enter_context(tc.tile_pool(name="ppec", bufs=2, space=PSUM))
        pp_seg = ctx.enter_context(tc.tile_pool(name="ppseg", bufs=1, space=PSUM))

        # ---- constants ----
        w1_sb = const.tile([128, KG, 2, KS, 128], fd, tag="w1")
        nc.sync.dma_start(w1_sb[:], w1_d[:])
        w2_sb = const.tile([128, 2, 2, 128], f16, tag="w2")
        nc.sync.dma_start(w2_sb[:], w2_d[:])
        wout_sb = const.tile([128, 2, 1], f16, tag="wout")
        nc.sync.dma_start(wout_sb[:], wout_d[:])
        if fp8 and psl_mode == "hilo8":
            wpa_sb = const.tile([128, 4, 2, 16], f8, tag="wpa")
            nc.sync.dma_start(wpa_sb[:], wpa_d[:])
            wpb_sb = const.tile([128, 4, 2, 16], f8, tag="wpb")
            nc.sync.dma_start(wpb_sb[:], wpb_d[:])
        else:
            wpa_sb = const.tile([128, 8, 1, 1], f8e3 if (fp8 and psl_mode == "e3") else f16, tag="wpa")
            nc.sync.dma_start(wpa_sb[:], wpa_d[:])
            wpb_sb = None
        iota_sb = const.tile([128, SMAX], f16, tag="iota")
        nc.sync.dma_start(iota_sb[:], iota_d[:])
        relb_sb = const.tile([CHUNK, C], f32, tag="relb")
        nc.sync.dma_start(relb_sb[:], relb_d[:])
        nums_sb = const.tile([CHUNK, C], f32, tag="nums")
        nc.sync.dma_start(nums_sb[:], nums_d[:])
        ones_sb = const.tile([97, 1], f16, tag="ones")
        nc.sync.dma_start(ones_sb[:], ones_d[:])

        # species energy per atom: cubic through W_comp[0, 0..3]
        # comp = (c1*n + c0) + n*n*(c3*n + c2)
        t_n2 = const.tile([CHUNK, C], f32, tag="t_n2")
        nc.vector.tensor_mul(t_n2[:], nums_sb[:], nums_sb[:])
        t_a = const.tile([CHUNK, C], f32, tag="t_a")
        nc.vector.tensor_scalar(t_a[:], nums_sb[:], c3, c2, ALU.mult, ALU.add)
        t_b = const.tile([CHUNK, C], f32, tag="t_b")
        nc.vector.tensor_mul(t_b[:], t_n2[:], t_a[:])
        t_c = const.tile([CHUNK, C], f32, tag="t_c")
        nc.vector.tensor_scalar(t_c[:], nums_sb[:], c1, c0, ALU.mult, ALU.add)
        comp_sb = const.tile([CHUNK, C], f32, tag="comp")
        nc.vector.tensor_add(comp_sb[:], t_b[:], t_c[:])

        seg_ps = pp_seg.tile([1, SMAX], f32, tag="seg")
        nc.vector.memset(seg_ps[:], 0.0)
        # e-partials bank: psl rows 0/32, psnn rows 64/96, with a +16
        # partition offset on odd tiles (same PSUM quadrant, so the
        # matmul dst stays valid) — in-bank double buffering that breaks
        # the psl(t+1) <- e_row-copy(t) WAR. Rows in between stay 0 from
        # this one-time clear; the K=113 ones-matmul sums the partials.
        e_ps = pp_e.tile([128, TILE], f32, tag="e")
        nc.vector.memset(e_ps[:], 0.0)
        # one-hot windows are generated a few tiles ahead of use (OH_PRE)
        # so the seg matmul never waits on the DVE mid-tile, without
        # clogging the in-order DVE queue upfront
        oh_all = None
        OH_PRE = 3
        if segw is not None:
            W = segw[0]
            oh_all = const.tile([128, C, W], f16, tag="oh_all")

        def gen_oh(ch):
            if oh_all is not None and ch < C:
                nc.vector.tensor_scalar(
                    oh_all[:, ch], iota_sb[:, 0:W], relb_sb[:, ch : ch + 1],
                    None, ALU.is_equal,
                )

        for ch in range(4 * OH_PRE):
            gen_oh(ch)

        for t in range(nT):
            # alternate PSUM quadrant pair per tile: psl+psnn write rows
            # {base, base+32}; the e_row copy reads only that 33-row
            # slice, so tile t+1's partials have no WAR against it
            base = 64 * (t % 2)
            for cc in range(4):
                gen_oh((t + OH_PRE) * 4 + cc)
            # ---- contiguous loads: [128, 8, 512]
            big = psTp.tile([128, 8, TILE], fd, tag="psT", name=f"psT{t}")
            nc.sync.dma_start(big[:], psT_d[:, t])
            if fp8:
                d2 = {"hilo8": f8, "e3": f8e3, "fp16": f16}[psl_mode]
                big2 = psTp2.tile([128, 8, TILE], d2, tag="psT2", name=f"psT2_{t}")
                nc.sync.dma_start(big2[:], psl2_d[:, t])

            # ---- h1: KG k-groups x 2 M-chunks
            h1ps = [pp_h1.tile([128, TILE], f32, tag=f"h1m{m}", name=f"h1ps{t}_{m}") for m in range(2)]
            for j in range(KG):
                for m in range(2):
                    nc.tensor.matmul(
                        h1ps[m][:],
                        w1_sb[:, j, m],
                        big[:, KS * j : KS * (j + 1)],
                        start=(j == 0),
                        stop=(j == KG - 1),
                        perf_mode=DR,
                    )
            # ---- psl: M=1 matmuls in column groups, partial sums landing
            # on e_ps rows 0/32/64/96.
            if fp8 and psl_mode == "hilo8":
                # hi*w_hi + lo*w_hi + hi*w_lo, all fp8 DoubleRow.
                # DoubleRow forbids col-group tiling (col_grp must be 0xf)
                # so every partial accumulates onto e_ps row 0.
                for cn, (wp, mv) in enumerate(
                    [(wpa_sb, big), (wpa_sb, big2), (wpb_sb, big)]
                ):
                    for j in range(4):
                        nc.tensor.matmul(
                            e_ps[0:1, :],
                            wp[:, j, :, 0:1],
                            mv[:, 2 * j : 2 * j + 2],
                            start=(cn == 0 and j == 0),
                            stop=False,
                            perf_mode=DR,
                        )
            elif fp8:
                # psl from the scaled-e3m4/fp16 psT copy: fast M=1 matmuls
                # in column groups 0/1, partials on e_ps rows 0/32 (the
                # 1/(s*t) unscale rides in the ones vector rows 0:33)
                for k in range(8):
                    g = base + 32 * (k % 2)
                    nc.tensor.matmul(
                        e_ps[g : g + 1, :],
                        wpa_sb[:, k],
                        big2[:, k],
                        start=(k < 2),
                        stop=False,
                        tile_position=(0, g),
                    )
            else:
                for k in range(8):
                    g = 32 * (k % 4)
                    nc.tensor.matmul(
                        e_ps[g : g + 1, :],
                        wpa_sb[:, k],
                        big[:, k],
                        start=(k < 4),
                        stop=False,
                        tile_position=(0, g),
                    )

            sil1 = silp.tile([128, 2, TILE], f16, tag="sil1")
            for m in range(2):
                nc.scalar.activation(sil1[:, m], h1ps[m][:], AF.Silu)

            # ---- h2 (fp16)
            h2ps = [pp_h2.tile([128, TILE], f32, tag=f"h2m{m}", name=f"h2ps{t}_{m}") for m in range(2)]
            for kj in range(2):
                for m in range(2):
                    nc.tensor.matmul(
                        h2ps[m][:],
                        w2_sb[:, m, kj],
                        sil1[:, kj],
                        start=(kj == 0),
                        stop=(kj == 1),
                    )
            sil2 = silp.tile([128, 2, TILE], f16, tag="sil2")
            for m in range(2):
                nc.scalar.activation(sil2[:, m], h2ps[m][:], AF.Silu)

            # ---- psnn (fp16) continues the psl chains on rows
            # {base, base+32} (hilo8: fresh rows 64/96; fp16 mode: psl rows)
            for kj in range(2):
                if fp8 and psl_mode == "hilo8":
                    g, st = 64 + 32 * kj, True
                elif fp8:
                    g, st = base + 32 * kj, False
                else:
                    g, st = 32 * kj, False
                nc.tensor.matmul(
                    e_ps[g : g + 1, :],
                    wout_sb[:, kj],
                    sil2[:, kj],
                    start=st,
                    stop=(kj == 1),
                    tile_position=(0, g),
                )
            # partial rows (zeros between) -> SBUF in one copy
            nrow = 33 if (fp8 and psl_mode != "hilo8") else 97
            rbase = base if (fp8 and psl_mode != "hilo8") else 0
            e_row = rowp.tile([nrow, TILE], f16, tag="erow")
            nc.vector.tensor_copy(e_row[:], e_ps[rbase : rbase + nrow, :])

            # ---- per-chunk: column-ize (K=97 sums the partials), add
            # species energy, segment matmul
            for cc in range(4):
                ch = t * 4 + cc
                ec_ps = pp_ec.tile([128, 1], f32, tag="ec")
                nc.tensor.matmul(
                    ec_ps[:],
                    e_row[0:nrow, cc * 128 : (cc + 1) * 128],
                    ones_sb[0:nrow],
                    start=True,
                    stop=True,
                )
                e_col = rowp.tile([128, 1], f16, tag="ecol")
                nc.vector.tensor_add(e_col[:], ec_ps[:], comp_sb[:, ch : ch + 1])
                if segw is not None:
                    # host guarantees this chunk's structures fall inside
                    # [w0, w0+W); relb is pre-shifted by -w0 per chunk
                    W, w0s = segw
                    w0 = w0s[ch]
                    nc.tensor.matmul(
                        seg_ps[0:1, w0 : w0 + W], e_col[:], oh_all[:, ch],
                        start=False, stop=(ch == C - 1), skip_group_check=True,
                    )
                else:
                    oh = rowp.tile([128, SMAX], f16, tag="oh")
                    nc.vector.tensor_scalar(
                        oh[:], iota_sb[:], relb_sb[:, ch : ch + 1], None, ALU.is_equal
                    )
                    nc.tensor.matmul(
                        seg_ps[:], e_col[:], oh[:], start=(ch == 0), stop=(ch == C - 1)
                    )

        out_sb = rowp.tile([1, SMAX], f32, tag="outsb")
        nc.scalar.activation(out_sb[:], seg_ps[:], AF.Copy)
        nc.sync.dma_start(out_d[:], out_sb[:])

    _split_waits(nc, mybir)
    return nc


def _install_ntff_hook():
    """Register the axon NTFF profile hook (missing antenv.axon_hooks in
    this image) so run_bass_kernel_spmd(trace=True) can report exec_time_ns."""
    import sys
    import types

    try:
        import antenv.axon_hooks  # noqa: F401

        return
    except ImportError:
        pass
    from trn_agent_boot.trn_boot import _ntff_profile_via_ctypes

    hook = _ntff_profile_via_ctypes("/opt/axon/libaxon_pjrt.so")
    mod = types.ModuleType("antenv.axon_hooks")
    mod.get_axon_ntff_profile_hook = lambda: hook
    mod.set_axon_ntff_profile_hook = lambda h: None
    sys.modules["antenv.axon_hooks"] = mod
    import antenv

    antenv.axon_hooks = mod
    import concourse.bass_utils as bu

    bu.upload_artifacts = lambda tmpdir: tmpdir


def _to_psT(pss, Ta):
    """[Ta, 1024] -> [128, nT, 8, TILE] with [p,t,k,a] = pss[t*TILE+a, k*128+p]"""
    nT = Ta // TILE
    return np.ascontiguousarray(pss.reshape(nT, TILE, 8, 128).transpose(3, 0, 2, 1))


def kernel(ps, numbers, batch, W_comp, W_psl, W_h1, W_h2, W_out):
    global LAST_EXEC_NS, LAST_RESULTS
    from concourse.bass_utils import run_bass_kernel_spmd

    if TRACE:
        _install_ntff_hook()

    fp8 = MODE == "fp8mix"
    DT = F8 if fp8 else np.float16
    KG = 4 if fp8 else 8
    KS = 2 if fp8 else 1

    ps = np.asarray(ps)
    numbers = np.asarray(numbers)
    batch = np.asarray(batch)
    W_comp = np.asarray(W_comp, dtype=np.float32)
    W_psl = np.asarray(W_psl, dtype=np.float32)
    W_h1 = np.asarray(W_h1, dtype=np.float32)
    W_h2 = np.asarray(W_h2, dtype=np.float32)
    W_out = np.asarray(W_out, dtype=np.float32)

    counts = np.bincount(batch, minlength=N_STRUCT)
    cum = np.zeros(N_STRUCT + 1, dtype=np.int64)
    np.cumsum(counts, out=cum[1:])

    # equal-structure shard cuts (atoms balance to ~1-2% by CLT; keeps
    # per-core structure count fixed at N_STRUCT/N_CORES <= SMAX)
    s_cut = [i * N_STRUCT // N_CORES for i in range(N_CORES + 1)]

    shards = []
    for i in range(N_CORES):
        s_lo, s_hi = s_cut[i], s_cut[i + 1]
        a_lo, a_hi = int(cum[s_lo]), int(cum[s_hi])
        n_at, n_st = a_hi - a_lo, s_hi - s_lo
        assert n_st <= SMAX, f"shard {i}: {n_st} structs > {SMAX}"
        shards.append((s_lo, s_hi, a_lo, a_hi, n_at, n_st))

    Ta = max(s[4] for s in shards)
    Ta = (Ta + TILE - 1) // TILE * TILE
    nT = Ta // TILE
    C = Ta // CHUNK

    # replicated weights: w1[p, j, m, i, c] = W_h1[m*128+c, (KS*j+i)*128+p]
    w1 = np.ascontiguousarray(
        W_h1.T.reshape(KG, KS, 128, 2, 128).transpose(2, 0, 3, 1, 4)
    ).astype(DT)
    # w2[p, m, i, c] = W_h2[m*128+c, i*128+p]  (fp16)
    w2 = np.ascontiguousarray(
        W_h2.T.reshape(2, 128, 2, 128).transpose(1, 2, 0, 3)
    ).astype(np.float16)
    # wout[p, i] = W_out[0, i*128+p]  (fp16)
    wout = np.ascontiguousarray(
        W_out[0].reshape(2, 128).T.reshape(128, 2, 1)
    ).astype(np.float16)
    # psl weights
    if fp8 and PSL_MODE == "hilo8":
        wp32 = W_psl[0].astype(np.float32)
        wp_hi8 = wp32.astype(F8)
        wp_lo8 = (wp32 - wp_hi8.astype(np.float32)).astype(F8)
        def pack(w):
            out = np.zeros((128, 4, 2, 16), dtype=F8)
            out[..., 0] = np.asarray(w).reshape(4, 2, 128).transpose(2, 0, 1)
            return out

        wpsl_hi, wpsl_lo = pack(wp_hi8), pack(wp_lo8)
    elif fp8 and PSL_MODE == "e3":
        t_w = 15.5 / max(float(np.abs(W_psl).max()), 1e-30)
        _WPSL_T = [t_w]
        wpsl16 = np.asarray(
            np.clip(W_psl[0].astype(np.float32) * t_w, -15.5, 15.5)
            .reshape(8, 128).T.reshape(128, 8, 1, 1),
            dtype=ml_dtypes.float8_e3m4,
        ).copy()
    else:
        wpsl16 = np.ascontiguousarray(
            W_psl[0].reshape(8, 128).T.reshape(128, 8, 1, 1)
        ).astype(np.float16)
    iota = np.ascontiguousarray(
        np.tile(np.arange(SMAX, dtype=np.float16), (128, 1))
    )
    # exact cubic through the 4 species energies
    V = np.vander(np.arange(N_SPECIES, dtype=np.float64), 4, increasing=True)
    poly = np.linalg.solve(V, W_comp[0, :N_SPECIES].astype(np.float64))

    # per-shard relative batch ids (padding = -1)
    rb_all = []
    for s_lo, s_hi, a_lo, a_hi, n_at, n_st in shards:
        rb = np.full(Ta, -1.0, dtype=np.float32)
        rb[:n_at] = (batch[a_lo:a_hi] - s_lo).astype(np.float32)
        rb_all.append(rb)

    segw = None
    if SEG_WINDOWED:
        # shared per-chunk structure windows: batch is sorted so each
        # 128-atom chunk only spans a few structures; bake [w0, w0+W)
        m0 = np.full(C, np.inf)
        m1 = np.full(C, -np.inf)
        for rb in rb_all:
            r2 = rb.reshape(C, CHUNK)
            mask = r2 >= 0
            has = mask.any(axis=1)
            lo = np.where(has, np.where(mask, r2, np.inf).min(axis=1), np.inf)
            hi = np.where(has, np.where(mask, r2, -np.inf).max(axis=1), -np.inf)
            m0 = np.minimum(m0, lo)
            m1 = np.maximum(m1, hi)
        w0 = np.where(np.isfinite(m0), m0, 0.0)
        span = np.where(np.isfinite(m1), m1 - w0 + 1, 1.0)
        W = int(max(1, span.max()))
        W = min((W + 3) // 4 * 4, SMAX)
        w0 = np.clip(w0, 0, SMAX - W).astype(np.int64)
        segw = (W, tuple(int(x) for x in w0))

    key = (Ta, C, tuple(np.round(poly, 12)), MODE, PSL_MODE, segw)
    if key not in _BUILD_CACHE:
        _BUILD_CACHE.clear()
        _BUILD_CACHE[key] = _build(Ta, C, poly, MODE, PSL_MODE, segw)
    nc = _BUILD_CACHE[key]

    psq = np.asarray(ps, dtype=DT)  # quantize once, slice per shard
    ones = np.ones((97, 1), dtype=np.float16)
    if fp8 and PSL_MODE == "hilo8":
        ps2 = (ps.astype(np.float32) - psq.astype(np.float32)).astype(F8)
    elif fp8 and PSL_MODE == "e3":
        # scale ps and wpsl into e3m4's normal range (subnormals would
        # cost ~8% error on small values); the 1/(s*t) unscale rides in
        # the colize ones vector rows 0:33 (psl partial rows 0/32)
        s_ps = 15.5 / max(float(np.abs(ps).max()), 1e-30)
        ps2 = np.asarray(
            np.clip(ps.astype(np.float32) * s_ps, -15.5, 15.5),
            dtype=ml_dtypes.float8_e3m4,
        )
        raise NotImplementedError("e3 psl scaling incompatible with shared psl/psnn rows")
    elif fp8:
        ps2 = ps.astype(np.float16)

    in_maps = []
    for si, (s_lo, s_hi, a_lo, a_hi, n_at, n_st) in enumerate(shards):
        pss = np.zeros((Ta, N_FEAT), dtype=DT)
        pss[:n_at] = psq[a_lo:a_hi]
        rb = rb_all[si]
        if segw is not None:
            rb = rb.reshape(C, CHUNK) - np.asarray(segw[1], dtype=np.float32)[:, None]
            rb = rb.reshape(Ta)
        nums = np.zeros(Ta, dtype=np.float32)
        nums[:n_at] = numbers[a_lo:a_hi].astype(np.float32)
        im = {
            "psT": _to_psT(pss, Ta),
            "relb": np.ascontiguousarray(rb.reshape(C, CHUNK).T),
            "nums": np.ascontiguousarray(nums.reshape(C, CHUNK).T),
            "w1": w1,
            "w2": w2,
            "wout": wout,
            "ones": ones,
            "iota": iota,
        }
        if fp8 and PSL_MODE == "hilo8":
            pss2 = np.zeros((Ta, N_FEAT), dtype=F8)
            pss2[:n_at] = ps2[a_lo:a_hi]
            im["psT2"] = _to_psT(pss2, Ta)
            im["wpsl_hi"] = wpsl_hi
            im["wpsl_lo"] = wpsl_lo
        elif fp8:
            pss2 = np.zeros((Ta, N_FEAT), dtype=ps2.dtype)
            pss2[:n_at] = ps2[a_lo:a_hi]
            im["psT2"] = _to_psT(pss2, Ta)
            im["wpsl"] = wpsl16
        else:
            im["wpsl"] = wpsl16
        in_maps.append(im)

    res = run_bass_kernel_spmd(nc, in_maps, list(range(N_CORES)), trace=TRACE)
    LAST_EXEC_NS = res.exec_time_ns
    LAST_RESULTS = res

    out = np.zeros((N_STRUCT, 1), dtype=np.float32)
    for i, (s_lo, s_hi, a_lo, a_hi, n_at, n_st) in enumerate(shards):
        vals = res.results[i]["out"][0, :n_st].astype(np.float32)
        empty = counts[s_lo:s_hi] == 0
        if empty.any():
            vals = np.where(empty, 0.0, vals)
        out[s_lo:s_hi, 0] = vals
    return out


# revision 22
# speedup vs baseline: 1.1559x; 1.0384x over previous
"""PowerSpectrumModel Trainium2 kernel (8 NeuronCores, SPMD).

Strategy (data-parallel over atoms, structures disjoint per shard):
 - Host: cut the atom axis at structure boundaries into 8 balanced shards;
   pre-TRANSPOSE ps to feature-major [128, nT, 8, TILE] (so every tile
   load is one contiguous stride per partition — no on-device transpose
   descriptors); fp32 PSUM accumulation on device; replicate the small
   weight matrices.
 - Precision (fp8mix mode): the big h1 GEMM runs fp8e4m3 DoubleRow (2x
   PE rate; w1-quant + ps-quant contribute only ~6e-3 rel err because
   errors decorrelate across atoms); h2/psnn stay fp16 (their weight
   quantization biases the per-atom energy coherently across a structure
   -> 3.6e-2 if fp8); psl gets hi+lo fp8 splits of both ps and W_psl
   (or an extra fp16 psT copy, PSL_MODE).
 - Device, per 512-atom tile:
     psT   <- contiguous loads [128, 8, 512]                      [DMA]
     h1    = W_h1 @ psT  (fp8 DoubleRow k-pairs / fp16 k-chunks)  [PE]
     psl   = W_psl row, M=1 matmuls -> e partial rows 0/32/64/96  [PE]
     sil1  = silu(h1)                                             [ACT]
     h2    = W_h2 @ sil1; sil2 = silu(h2)                         [PE/ACT]
     psnn  M=1 matmuls accumulated onto the psl PSUM rows         [PE]
     per 128-chunk: e column via K=97 ones matmul, + species
     energy, one-hot(struct) matmul accumulates into a [1,256]
     PSUM row holding this core's per-structure energies.         [PE/DVE]
 - Host: slice per-core structure ranges, concat -> [2000, 1].
"""

import numpy as np
import ml_dtypes

N_ATOMS = 200000
N_FEAT = 1024
N_SPECIES = 4
N_STRUCT = 2000
H1 = 256
H2 = 256
SCALE = 1.0
N_CORES = 8
TILE = 512
CHUNK = 128
SMAX = 256  # per-core structure capacity (PSUM row)

MODE = "fp8mix"  # "fp16" | "fp8mix"
PSL_MODE = "fp16"  # "e3" | "hilo8" | "fp16"
SEG_WINDOWED = True  # bake per-chunk structure windows into the seg matmul

_BUILD_CACHE = {}
TRACE = False
LAST_EXEC_NS = None
LAST_RESULTS = None

F8 = ml_dtypes.float8_e4m3


def _split_waits(nc, mybir, maxw=1):
    """walrus on this build rejects >1 sync wait per instruction; move
    overflow waits onto preceding same-engine NoOps."""
    cnt = 0
    for f in nc.m.functions:
        for blk in f.blocks:
            if not hasattr(blk, "instructions"):
                continue
            out = []
            changed = False
            for inst in blk.instructions:
                si = getattr(inst, "sync_info", None)
                if si is not None and si.on_wait and len(si.on_wait) > maxw:
                    waits = list(si.on_wait)
                    keep = waits[-maxw:]
                    extra = waits[:-maxw]
                    while extra:
                        chunk, extra = extra[:maxw], extra[maxw:]
                        cnt += 1
                        out.append(
                            mybir.InstNoOp(
                                name=f"waitfix-{cnt}",
                                engine=inst.engine,
                                text_hint="waitfix",
                                bass_nofuse=True,
                                ins=[],
                                outs=[],
                                sync_info=mybir.SyncInfo(on_wait=chunk, on_update=[]),
                            )
                        )
                    si.on_wait = keep
                    changed = True
                out.append(inst)
            if changed:
                blk.instructions[:] = out
    return cnt


def _build(Ta, C, poly, mode, psl_mode, segw=None):
    import concourse.bass as bass
    import concourse.tile as tile
    import concourse.mybir as mybir
    from contextlib import ExitStack

    fp8 = mode == "fp8mix"
    f8 = mybir.dt.float8e4
    f16 = mybir.dt.float16
    f32 = mybir.dt.float32
    fd = f8 if fp8 else f16  # h1 GEMM dtype
    AF = mybir.ActivationFunctionType
    ALU = mybir.AluOpType
    DR = mybir.MatmulPerfMode.DoubleRow if fp8 else None
    PSUM = bass.MemorySpace.PSUM
    nT = Ta // TILE
    c0, c1, c2, c3 = (float(x) for x in poly)
    KG = 4 if fp8 else 8  # h1 k-groups (DoubleRow contracts pairs)
    KS = 2 if fp8 else 1  # k-chunks per h1 matmul

    nc = bass.Bass("TRN2", target_bir_lowering=False, debug=False)

    # pre-transposed ps: [p, t, k, a] = ps[t*512+a, k*128+p]
    psT_d = nc.dram_tensor("psT", [128, nT, 8, TILE], fd, kind="ExternalInput").ap()
    f8e3 = mybir.dt.float8e3
    psl2_d = None
    if fp8:
        d2 = {"hilo8": f8, "e3": f8e3, "fp16": f16}[psl_mode]
        psl2_d = nc.dram_tensor("psT2", [128, nT, 8, TILE], d2, kind="ExternalInput").ap()
    relb_d = nc.dram_tensor("relb", [CHUNK, C], f32, kind="ExternalInput").ap()
    nums_d = nc.dram_tensor("nums", [CHUNK, C], f32, kind="ExternalInput").ap()
    # w1: [p, j, m, i, c] = W_h1[m*128+c, (KS*j+i)*128+p]
    w1_d = nc.dram_tensor("w1", [128, KG, 2, KS, 128], fd, kind="ExternalInput").ap()
    # w2: [p, m, i, c] = W_h2[m*128+c, i*128+p]  (fp16)
    w2_d = nc.dram_tensor("w2", [128, 2, 2, 128], f16, kind="ExternalInput").ap()
    # wout: [p, i] = W_out[0, i*128+p]  (fp16)
    wout_d = nc.dram_tensor("wout", [128, 2, 1], f16, kind="ExternalInput").ap()
    # psl weights
    if fp8 and psl_mode == "hilo8":
        # k-subtile dim padded to step 16 (ISA: dual-fp8 LW step%16==0)
        wpa_d = nc.dram_tensor("wpsl_hi", [128, 4, 2, 16], f8, kind="ExternalInput").ap()
        wpb_d = nc.dram_tensor("wpsl_lo", [128, 4, 2, 16], f8, kind="ExternalInput").ap()
    elif fp8 and psl_mode == "e3":
        wpa_d = nc.dram_tensor("wpsl", [128, 8, 1, 1], f8e3, kind="ExternalInput").ap()
        wpb_d = None
    else:
        wpa_d = nc.dram_tensor("wpsl", [128, 8, 1, 1], f16, kind="ExternalInput").ap()
        wpb_d = None
    ones_d = nc.dram_tensor("ones", [97, 1], f16, kind="ExternalInput").ap()
    iota_d = nc.dram_tensor("iota", [128, SMAX], f16, kind="ExternalInput").ap()
    out_d = nc.dram_tensor("out", [1, SMAX], f32, kind="ExternalOutput").ap()

    with tile.TileContext(nc) as tc, ExitStack() as ctx:
        const = ctx.enter_context(tc.tile_pool(name="const", bufs=1))
        psTp = ctx.enter_context(tc.tile_pool(name="psT", bufs=6))
        psTp2 = ctx.enter_context(tc.tile_pool(name="psT2", bufs=6)) if fp8 else None
        silp = ctx.enter_context(tc.tile_pool(name="sil", bufs=4))
        rowp = ctx.enter_context(tc.tile_pool(name="row", bufs=4))
        pp_h1 = ctx.enter_context(tc.tile_pool(name="pph1", bufs=1, space=PSUM))
        pp_h2 = ctx.enter_context(tc.tile_pool(name="pph2", bufs=1, space=PSUM))
        pp_e = ctx.enter_context(tc.tile_pool(name="ppe", bufs=1, space=PSUM))
        pp_ec = ctx.enter_context(tc.tile_pool(name="ppec", bufs=2, space=PSUM))
        pp_seg = ctx.enter_context(tc.tile_pool(name="ppseg", bufs=1, space=PSUM))

        # ---- constants ----
        w1_sb = const.tile([128, KG, 2, KS, 128], fd, tag="w1")
        nc.sync.dma_start(w1_sb[:], w1_d[:])
        w2_sb = const.tile([128, 2, 2, 128], f16, tag="w2")
        nc.sync.dma_start(w2_sb[:], w2_d[:])
        wout_sb = const.tile([128, 2, 1], f16, tag="wout")
        nc.sync.dma_start(wout_sb[:], wout_d[:])
        if fp8 and psl_mode == "hilo8":
            wpa_sb = const.tile([128, 4, 2, 16], f8, tag="wpa")
            nc.sync.dma_start(wpa_sb[:], wpa_d[:])
            wpb_sb = const.tile([128, 4, 2, 16], f8, tag="wpb")
            nc.sync.dma_start(wpb_sb[:], wpb_d[:])
        else:
            wpa_sb = const.tile([128, 8, 1, 1], f8e3 if (fp8 and psl_mode == "e3") else f16, tag="wpa")
            nc.sync.dma_start(wpa_sb[:], wpa_d[:])
            wpb_sb = None
        iota_sb = const.tile([128, SMAX], f16, tag="iota")
        nc.sync.dma_start(iota_sb[:], iota_d[:])
        relb_sb = const.tile([CHUNK, C], f32, tag="relb")
        nc.sync.dma_start(relb_sb[:], relb_d[:])
        nums_sb = const.tile([CHUNK, C], f32, tag="nums")
        nc.sync.dma_start(nums_sb[:], nums_d[:])
        ones_sb = const.tile([97, 1], f16, tag="ones")
        nc.sync.dma_start(ones_sb[:], ones_d[:])

        # species energy per atom: cubic through W_comp[0, 0..3]
        # comp = (c1*n + c0) + n*n*(c3*n + c2)
        t_n2 = const.tile([CHUNK, C], f32, tag="t_n2")
        nc.vector.tensor_mul(t_n2[:], nums_sb[:], nums_sb[:])
        t_a = const.tile([CHUNK, C], f32, tag="t_a")
        nc.vector.tensor_scalar(t_a[:], nums_sb[:], c3, c2, ALU.mult, ALU.add)
        t_b = const.tile([CHUNK, C], f32, tag="t_b")
        nc.vector.tensor_mul(t_b[:], t_n2[:], t_a[:])
        t_c = const.tile([CHUNK, C], f32, tag="t_c")
        nc.vector.tensor_scalar(t_c[:], nums_sb[:], c1, c0, ALU.mult, ALU.add)
        comp_sb = const.tile([CHUNK, C], f32, tag="comp")
        nc.vector.tensor_add(comp_sb[:], t_b[:], t_c[:])

        seg_ps = pp_seg.tile([1, SMAX], f32, tag="seg")
        nc.vector.memset(seg_ps[:], 0.0)
        # e-partials bank: psl rows 0/32, psnn rows 64/96, with a +16
        # partition offset on odd tiles (same PSUM quadrant, so the
        # matmul dst stays valid) — in-bank double buffering that breaks
        # the psl(t+1) <- e_row-copy(t) WAR. Rows in between stay 0 from
        # this one-time clear; the K=113 ones-matmul sums the partials.
        e_ps = pp_e.tile([128, TILE], f32, tag="e")
        nc.vector.memset(e_ps[:], 0.0)


        for t in range(nT):
            # ---- contiguous loads: [128, 8, 512]
            big = psTp.tile([128, 8, TILE], fd, tag="psT", name=f"psT{t}")
            nc.sync.dma_start(big[:], psT_d[:, t])
            if fp8:
                d2 = {"hilo8": f8, "e3": f8e3, "fp16": f16}[psl_mode]
                big2 = psTp2.tile([128, 8, TILE], d2, tag="psT2", name=f"psT2_{t}")
                nc.sync.dma_start(big2[:], psl2_d[:, t])

            # ---- h1: KG k-groups x 2 M-chunks
            h1ps = [pp_h1.tile([128, TILE], f32, tag=f"h1m{m}", name=f"h1ps{t}_{m}") for m in range(2)]
            for j in range(KG):
                for m in range(2):
                    nc.tensor.matmul(
                        h1ps[m][:],
                        w1_sb[:, j, m],
                        big[:, KS * j : KS * (j + 1)],
                        start=(j == 0),
                        stop=(j == KG - 1),
                        perf_mode=DR,
                    )
            # ---- psl: M=1 matmuls in column groups, partial sums landing
            # on e_ps rows 0/32/64/96.
            if fp8 and psl_mode == "hilo8":
                # hi*w_hi + lo*w_hi + hi*w_lo, all fp8 DoubleRow.
                # DoubleRow forbids col-group tiling (col_grp must be 0xf)
                # so every partial accumulates onto e_ps row 0.
                for cn, (wp, mv) in enumerate(
                    [(wpa_sb, big), (wpa_sb, big2), (wpb_sb, big)]
                ):
                    for j in range(4):
                        nc.tensor.matmul(
                            e_ps[0:1, :],
                            wp[:, j, :, 0:1],
                            mv[:, 2 * j : 2 * j + 2],
                            start=(cn == 0 and j == 0),
                            stop=False,
                            perf_mode=DR,
                        )
            elif fp8:
                # psl from the scaled-e3m4/fp16 psT copy: fast M=1 matmuls
                # in column groups 0/1, partials on e_ps rows 0/32 (the
                # 1/(s*t) unscale rides in the ones vector rows 0:33)
                for k in range(8):
                    g = 32 * (k % 2)
                    nc.tensor.matmul(
                        e_ps[g : g + 1, :],
                        wpa_sb[:, k],
                        big2[:, k],
                        start=(k < 2),
                        stop=False,
                        tile_position=(0, g),
                    )
            else:
                for k in range(8):
                    g = 32 * (k % 4)
                    nc.tensor.matmul(
                        e_ps[g : g + 1, :],
                        wpa_sb[:, k],
                        big[:, k],
                        start=(k < 4),
                        stop=False,
                        tile_position=(0, g),
                    )

            sil1 = silp.tile([128, 2, TILE], f16, tag="sil1")
            for m in range(2):
                nc.scalar.activation(sil1[:, m], h1ps[m][:], AF.Silu)

            # ---- h2 (fp16)
            h2ps = [pp_h2.tile([128, TILE], f32, tag=f"h2m{m}", name=f"h2ps{t}_{m}") for m in range(2)]
            for kj in range(2):
                for m in range(2):
                    nc.tensor.matmul(
                        h2ps[m][:],
                        w2_sb[:, m, kj],
                        sil1[:, kj],
                        start=(kj == 0),
                        stop=(kj == 1),
                    )
            sil2 = silp.tile([128, 2, TILE], f16, tag="sil2")
            for m in range(2):
                nc.scalar.activation(sil2[:, m], h2ps[m][:], AF.Silu)

            # ---- psnn (fp16): fresh rows 64/96 in fp8 modes (psl only
            # occupies rows 0/32); fp16 mode accumulates onto the psl rows
            for kj in range(2):
                if fp8:
                    g, st = 64 + 32 * kj, True
                else:
                    g, st = 32 * kj, False
                nc.tensor.matmul(
                    e_ps[g : g + 1, :],
                    wout_sb[:, kj],
                    sil2[:, kj],
                    start=st,
                    stop=(kj == 1),
                    tile_position=(0, g),
                )
            # partial rows 0/32/64/96 (zeros between) -> SBUF in one copy
            e_row = rowp.tile([97, TILE], f16, tag="erow")
            nc.vector.tensor_copy(e_row[:], e_ps[0:97, :])

            # ---- per-chunk: column-ize (K=97 sums the partials), add
            # species energy, segment matmul
            for cc in range(4):
                ch = t * 4 + cc
                ec_ps = pp_ec.tile([128, 1], f32, tag="ec")
                nc.tensor.matmul(
                    ec_ps[:],
                    e_row[0:97, cc * 128 : (cc + 1) * 128],
                    ones_sb[:],
                    start=True,
                    stop=True,
                )
                e_col = rowp.tile([128, 1], f16, tag="ecol")
                nc.vector.tensor_add(e_col[:], ec_ps[:], comp_sb[:, ch : ch + 1])
                if segw is not None:
                    # host guarantees this chunk's structures fall inside
                    # [w0, w0+W); relb is pre-shifted by -w0 per chunk
                    W, w0s = segw
                    w0 = w0s[ch]
                    oh = rowp.tile([128, W], f16, tag="oh")
                    nc.vector.tensor_scalar(
                        oh[:], iota_sb[:, 0:W], relb_sb[:, ch : ch + 1], None,
                        ALU.is_equal,
                    )
                    nc.tensor.matmul(
                        seg_ps[0:1, w0 : w0 + W], e_col[:], oh[:],
                        start=False, stop=(ch == C - 1), skip_group_check=True,
                    )
                else:
                    oh = rowp.tile([128, SMAX], f16, tag="oh")
                    nc.vector.tensor_scalar(
                        oh[:], iota_sb[:], relb_sb[:, ch : ch + 1], None, ALU.is_equal
                    )
                    nc.tensor.matmul(
                        seg_ps[:], e_col[:], oh[:], start=(ch == 0), stop=(ch == C - 1)
                    )

        out_sb = rowp.tile([1, SMAX], f32, tag="outsb")
        nc.scalar.activation(out_sb[:], seg_ps[:], AF.Copy)
        nc.sync.dma_start(out_d[:], out_sb[:])

    _split_waits(nc, mybir)
    return nc


def _install_ntff_hook():
    """Register the axon NTFF profile hook (missing antenv.axon_hooks in
    this image) so run_bass_kernel_spmd(trace=True) can report exec_time_ns."""
    import sys
    import types

    try:
        import antenv.axon_hooks  # noqa: F401

        return
    except ImportError:
        pass
    from trn_agent_boot.trn_boot import _ntff_profile_via_ctypes

    hook = _ntff_profile_via_ctypes("/opt/axon/libaxon_pjrt.so")
    mod = types.ModuleType("antenv.axon_hooks")
    mod.get_axon_ntff_profile_hook = lambda: hook
    mod.set_axon_ntff_profile_hook = lambda h: None
    sys.modules["antenv.axon_hooks"] = mod
    import antenv

    antenv.axon_hooks = mod
    import concourse.bass_utils as bu

    bu.upload_artifacts = lambda tmpdir: tmpdir


def _to_psT(pss, Ta):
    """[Ta, 1024] -> [128, nT, 8, TILE] with [p,t,k,a] = pss[t*TILE+a, k*128+p]"""
    nT = Ta // TILE
    return np.ascontiguousarray(pss.reshape(nT, TILE, 8, 128).transpose(3, 0, 2, 1))


def kernel(ps, numbers, batch, W_comp, W_psl, W_h1, W_h2, W_out):
    global LAST_EXEC_NS, LAST_RESULTS
    from concourse.bass_utils import run_bass_kernel_spmd

    if TRACE:
        _install_ntff_hook()

    fp8 = MODE == "fp8mix"
    DT = F8 if fp8 else np.float16
    KG = 4 if fp8 else 8
    KS = 2 if fp8 else 1

    ps = np.asarray(ps)
    numbers = np.asarray(numbers)
    batch = np.asarray(batch)
    W_comp = np.asarray(W_comp, dtype=np.float32)
    W_psl = np.asarray(W_psl, dtype=np.float32)
    W_h1 = np.asarray(W_h1, dtype=np.float32)
    W_h2 = np.asarray(W_h2, dtype=np.float32)
    W_out = np.asarray(W_out, dtype=np.float32)

    counts = np.bincount(batch, minlength=N_STRUCT)
    cum = np.zeros(N_STRUCT + 1, dtype=np.int64)
    np.cumsum(counts, out=cum[1:])

    # equal-structure shard cuts (atoms balance to ~1-2% by CLT; keeps
    # per-core structure count fixed at N_STRUCT/N_CORES <= SMAX)
    s_cut = [i * N_STRUCT // N_CORES for i in range(N_CORES + 1)]

    shards = []
    for i in range(N_CORES):
        s_lo, s_hi = s_cut[i], s_cut[i + 1]
        a_lo, a_hi = int(cum[s_lo]), int(cum[s_hi])
        n_at, n_st = a_hi - a_lo, s_hi - s_lo
        assert n_st <= SMAX, f"shard {i}: {n_st} structs > {SMAX}"
        shards.append((s_lo, s_hi, a_lo, a_hi, n_at, n_st))

    Ta = max(s[4] for s in shards)
    Ta = (Ta + TILE - 1) // TILE * TILE
    nT = Ta // TILE
    C = Ta // CHUNK

    # replicated weights: w1[p, j, m, i, c] = W_h1[m*128+c, (KS*j+i)*128+p]
    w1 = np.ascontiguousarray(
        W_h1.T.reshape(KG, KS, 128, 2, 128).transpose(2, 0, 3, 1, 4)
    ).astype(DT)
    # w2[p, m, i, c] = W_h2[m*128+c, i*128+p]  (fp16)
    w2 = np.ascontiguousarray(
        W_h2.T.reshape(2, 128, 2, 128).transpose(1, 2, 0, 3)
    ).astype(np.float16)
    # wout[p, i] = W_out[0, i*128+p]  (fp16)
    wout = np.ascontiguousarray(
        W_out[0].reshape(2, 128).T.reshape(128, 2, 1)
    ).astype(np.float16)
    # psl weights
    if fp8 and PSL_MODE == "hilo8":
        wp32 = W_psl[0].astype(np.float32)
        wp_hi8 = wp32.astype(F8)
        wp_lo8 = (wp32 - wp_hi8.astype(np.float32)).astype(F8)
        def pack(w):
            out = np.zeros((128, 4, 2, 16), dtype=F8)
            out[..., 0] = np.asarray(w).reshape(4, 2, 128).transpose(2, 0, 1)
            return out

        wpsl_hi, wpsl_lo = pack(wp_hi8), pack(wp_lo8)
    elif fp8 and PSL_MODE == "e3":
        t_w = 15.5 / max(float(np.abs(W_psl).max()), 1e-30)
        _WPSL_T = [t_w]
        wpsl16 = np.asarray(
            np.clip(W_psl[0].astype(np.float32) * t_w, -15.5, 15.5)
            .reshape(8, 128).T.reshape(128, 8, 1, 1),
            dtype=ml_dtypes.float8_e3m4,
        ).copy()
    else:
        wpsl16 = np.ascontiguousarray(
            W_psl[0].reshape(8, 128).T.reshape(128, 8, 1, 1)
        ).astype(np.float16)
    iota = np.ascontiguousarray(
        np.tile(np.arange(SMAX, dtype=np.float16), (128, 1))
    )
    # exact cubic through the 4 species energies
    V = np.vander(np.arange(N_SPECIES, dtype=np.float64), 4, increasing=True)
    poly = np.linalg.solve(V, W_comp[0, :N_SPECIES].astype(np.float64))

    # per-shard relative batch ids (padding = -1)
    rb_all = []
    for s_lo, s_hi, a_lo, a_hi, n_at, n_st in shards:
        rb = np.full(Ta, -1.0, dtype=np.float32)
        rb[:n_at] = (batch[a_lo:a_hi] - s_lo).astype(np.float32)
        rb_all.append(rb)

    segw = None
    if SEG_WINDOWED:
        # shared per-chunk structure windows: batch is sorted so each
        # 128-atom chunk only spans a few structures; bake [w0, w0+W)
        m0 = np.full(C, np.inf)
        m1 = np.full(C, -np.inf)
        for rb in rb_all:
            r2 = rb.reshape(C, CHUNK)
            mask = r2 >= 0
            has = mask.any(axis=1)
            lo = np.where(has, np.where(mask, r2, np.inf).min(axis=1), np.inf)
            hi = np.where(has, np.where(mask, r2, -np.inf).max(axis=1), -np.inf)
            m0 = np.minimum(m0, lo)
            m1 = np.maximum(m1, hi)
        w0 = np.where(np.isfinite(m0), m0, 0.0)
        span = np.where(np.isfinite(m1), m1 - w0 + 1, 1.0)
        W = int(max(1, span.max()))
        W = min((W + 3) // 4 * 4, SMAX)
        w0 = np.clip(w0, 0, SMAX - W).astype(np.int64)
        segw = (W, tuple(int(x) for x in w0))

    key = (Ta, C, tuple(np.round(poly, 12)), MODE, PSL_MODE, segw)
    if key not in _BUILD_CACHE:
        _BUILD_CACHE.clear()
        _BUILD_CACHE[key] = _build(Ta, C, poly, MODE, PSL_MODE, segw)
    nc = _BUILD_CACHE[key]

    psq = np.asarray(ps, dtype=DT)  # quantize once, slice per shard
    ones = np.ones((97, 1), dtype=np.float16)
    if fp8 and PSL_MODE == "hilo8":
        ps2 = (ps.astype(np.float32) - psq.astype(np.float32)).astype(F8)
    elif fp8 and PSL_MODE == "e3":
        # scale ps and wpsl into e3m4's normal range (subnormals would
        # cost ~8% error on small values); the 1/(s*t) unscale rides in
        # the colize ones vector rows 0:33 (psl partial rows 0/32)
        s_ps = 15.5 / max(float(np.abs(ps).max()), 1e-30)
        ps2 = np.asarray(
            np.clip(ps.astype(np.float32) * s_ps, -15.5, 15.5),
            dtype=ml_dtypes.float8_e3m4,
        )
        raise NotImplementedError("e3 psl scaling incompatible with shared psl/psnn rows")
    elif fp8:
        ps2 = ps.astype(np.float16)

    in_maps = []
    for si, (s_lo, s_hi, a_lo, a_hi, n_at, n_st) in enumerate(shards):
        pss = np.zeros((Ta, N_FEAT), dtype=DT)
        pss[:n_at] = psq[a_lo:a_hi]
        rb = rb_all[si]
        if segw is not None:
            rb = rb.reshape(C, CHUNK) - np.asarray(segw[1], dtype=np.float32)[:, None]
            rb = rb.reshape(Ta)
        nums = np.zeros(Ta, dtype=np.float32)
        nums[:n_at] = numbers[a_lo:a_hi].astype(np.float32)
        im = {
            "psT": _to_psT(pss, Ta),
            "relb": np.ascontiguousarray(rb.reshape(C, CHUNK).T),
            "nums": np.ascontiguousarray(nums.reshape(C, CHUNK).T),
            "w1": w1,
            "w2": w2,
            "wout": wout,
            "ones": ones,
            "iota": iota,
        }
        if fp8 and PSL_MODE == "hilo8":
            pss2 = np.zeros((Ta, N_FEAT), dtype=F8)
            pss2[:n_at] = ps2[a_lo:a_hi]
            im["psT2"] = _to_psT(pss2, Ta)
            im["wpsl_hi"] = wpsl_hi
            im["wpsl_lo"] = wpsl_lo
        elif fp8:
            pss2 = np.zeros((Ta, N_FEAT), dtype=ps2.dtype)
            pss2[:n_at] = ps2[a_lo:a_hi]
            im["psT2"] = _to_psT(pss2, Ta)
            im["wpsl"] = wpsl16
        else:
            im["wpsl"] = wpsl16
        in_maps.append(im)

    res = run_bass_kernel_spmd(nc, in_maps, list(range(N_CORES)), trace=TRACE)
    LAST_EXEC_NS = res.exec_time_ns
    LAST_RESULTS = res

    out = np.zeros((N_STRUCT, 1), dtype=np.float32)
    for i, (s_lo, s_hi, a_lo, a_hi, n_at, n_st) in enumerate(shards):
        vals = res.results[i]["out"][0, :n_st].astype(np.float32)
        empty = counts[s_lo:s_hi] == 0
        if empty.any():
            vals = np.where(empty, 0.0, vals)
        out[s_lo:s_hi, 0] = vals
    return out


# revision 29
# speedup vs baseline: 1.1784x; 1.0195x over previous
"""PowerSpectrumModel Trainium2 kernel (8 NeuronCores, SPMD).

Strategy (data-parallel over atoms, structures disjoint per shard):
 - Host: cut the atom axis at structure boundaries into 8 balanced shards;
   pre-TRANSPOSE ps to feature-major [128, nT, 8, TILE] (so every tile
   load is one contiguous stride per partition — no on-device transpose
   descriptors); fp32 PSUM accumulation on device; replicate the small
   weight matrices.
 - Precision (fp8mix mode): the big h1 GEMM runs fp8e4m3 DoubleRow (2x
   PE rate; w1-quant + ps-quant contribute only ~6e-3 rel err because
   errors decorrelate across atoms); h2/psnn stay fp16 (their weight
   quantization biases the per-atom energy coherently across a structure
   -> 3.6e-2 if fp8); psl gets hi+lo fp8 splits of both ps and W_psl
   (or an extra fp16 psT copy, PSL_MODE).
 - Device, per 512-atom tile:
     psT   <- contiguous loads [128, 8, 512]                      [DMA]
     h1    = W_h1 @ psT  (fp8 DoubleRow k-pairs / fp16 k-chunks)  [PE]
     psl   = W_psl row, M=1 matmuls -> e partial rows 0/32/64/96  [PE]
     sil1  = silu(h1)                                             [ACT]
     h2    = W_h2 @ sil1; sil2 = silu(h2)                         [PE/ACT]
     psnn  M=1 matmuls accumulated onto the psl PSUM rows         [PE]
     per 128-chunk: e column via K=97 ones matmul, + species
     energy, one-hot(struct) matmul accumulates into a [1,256]
     PSUM row holding this core's per-structure energies.         [PE/DVE]
 - Host: slice per-core structure ranges, concat -> [2000, 1].
"""

import numpy as np
import ml_dtypes

N_ATOMS = 200000
N_FEAT = 1024
N_SPECIES = 4
N_STRUCT = 2000
H1 = 256
H2 = 256
SCALE = 1.0
N_CORES = 8
TILE = 512
CHUNK = 128
SMAX = 256  # per-core structure capacity (PSUM row)

MODE = "fp8mix"  # "fp16" | "fp8mix"
PSL_MODE = "fp16"  # "e3" | "hilo8" | "fp16"
SEG_WINDOWED = True  # bake per-chunk structure windows into the seg matmul

_BUILD_CACHE = {}
TRACE = False
LAST_EXEC_NS = None
LAST_RESULTS = None

F8 = ml_dtypes.float8_e4m3


def _split_waits(nc, mybir, maxw=1):
    """walrus on this build rejects >1 sync wait per instruction; move
    overflow waits onto preceding same-engine NoOps."""
    cnt = 0
    for f in nc.m.functions:
        for blk in f.blocks:
            if not hasattr(blk, "instructions"):
                continue
            out = []
            changed = False
            for inst in blk.instructions:
                si = getattr(inst, "sync_info", None)
                if si is not None and si.on_wait and len(si.on_wait) > maxw:
                    waits = list(si.on_wait)
                    keep = waits[-maxw:]
                    extra = waits[:-maxw]
                    while extra:
                        chunk, extra = extra[:maxw], extra[maxw:]
                        cnt += 1
                        out.append(
                            mybir.InstNoOp(
                                name=f"waitfix-{cnt}",
                                engine=inst.engine,
                                text_hint="waitfix",
                                bass_nofuse=True,
                                ins=[],
                                outs=[],
                                sync_info=mybir.SyncInfo(on_wait=chunk, on_update=[]),
                            )
                        )
                    si.on_wait = keep
                    changed = True
                out.append(inst)
            if changed:
                blk.instructions[:] = out
    return cnt


def _build(Ta, C, poly, mode, psl_mode, segw=None):
    import concourse.bass as bass
    import concourse.tile as tile
    import concourse.mybir as mybir
    from contextlib import ExitStack

    fp8 = mode == "fp8mix"
    f8 = mybir.dt.float8e4
    f16 = mybir.dt.float16
    f32 = mybir.dt.float32
    fd = f8 if fp8 else f16  # h1 GEMM dtype
    AF = mybir.ActivationFunctionType
    ALU = mybir.AluOpType
    DR = mybir.MatmulPerfMode.DoubleRow if fp8 else None
    PSUM = bass.MemorySpace.PSUM
    nT = Ta // TILE
    c0, c1, c2, c3 = (float(x) for x in poly)
    KG = 4 if fp8 else 8  # h1 k-groups (DoubleRow contracts pairs)
    KS = 2 if fp8 else 1  # k-chunks per h1 matmul

    nc = bass.Bass("TRN2", target_bir_lowering=False, debug=False)

    # pre-transposed ps: [p, t, k, a] = ps[t*512+a, k*128+p]
    psT_d = nc.dram_tensor("psT", [128, nT, 8, TILE], fd, kind="ExternalInput").ap()
    f8e3 = mybir.dt.float8e3
    psl2_d = None
    if fp8:
        d2 = {"hilo8": f8, "e3": f8e3, "fp16": f16}[psl_mode]
        psl2_d = nc.dram_tensor("psT2", [128, nT, 8, TILE], d2, kind="ExternalInput").ap()
    relb_d = nc.dram_tensor("relb", [CHUNK, C], f32, kind="ExternalInput").ap()
    nums_d = nc.dram_tensor("nums", [CHUNK, C], f32, kind="ExternalInput").ap()
    # w1: [p, j, m, i, c] = W_h1[m*128+c, (KS*j+i)*128+p]
    w1_d = nc.dram_tensor("w1", [128, KG, 2, KS, 128], fd, kind="ExternalInput").ap()
    # w2: [p, m, i, c] = W_h2[m*128+c, i*128+p]  (fp16)
    w2_d = nc.dram_tensor("w2", [128, 2, 2, 128], f16, kind="ExternalInput").ap()
    # wout: [p, i] = W_out[0, i*128+p]  (fp16)
    wout_d = nc.dram_tensor("wout", [128, 2, 1], f16, kind="ExternalInput").ap()
    # psl weights
    if fp8 and psl_mode == "hilo8":
        # k-subtile dim padded to step 16 (ISA: dual-fp8 LW step%16==0)
        wpa_d = nc.dram_tensor("wpsl_hi", [128, 4, 2, 16], f8, kind="ExternalInput").ap()
        wpb_d = nc.dram_tensor("wpsl_lo", [128, 4, 2, 16], f8, kind="ExternalInput").ap()
    elif fp8 and psl_mode == "e3":
        wpa_d = nc.dram_tensor("wpsl", [128, 8, 1, 1], f8e3, kind="ExternalInput").ap()
        wpb_d = None
    else:
        wpa_d = nc.dram_tensor("wpsl", [128, 8, 1, 1], f16, kind="ExternalInput").ap()
        wpb_d = None
    ones_d = nc.dram_tensor("ones", [97, 1], f16, kind="ExternalInput").ap()
    iota_d = nc.dram_tensor("iota", [128, SMAX], f16, kind="ExternalInput").ap()
    out_d = nc.dram_tensor("out", [1, SMAX], f32, kind="ExternalOutput").ap()

    with tile.TileContext(nc) as tc, ExitStack() as ctx:
        const = ctx.enter_context(tc.tile_pool(name="const", bufs=1))
        psTp = ctx.enter_context(tc.tile_pool(name="psT", bufs=6))
        psTp2 = ctx.enter_context(tc.tile_pool(name="psT2", bufs=6)) if fp8 else None
        silp = ctx.enter_context(tc.tile_pool(name="sil", bufs=4))
        rowp = ctx.enter_context(tc.tile_pool(name="row", bufs=6))
        pp_h1 = ctx.enter_context(tc.tile_pool(name="pph1", bufs=1, space=PSUM))
        pp_h2 = ctx.enter_context(tc.tile_pool(name="pph2", bufs=1, space=PSUM))
        pp_e = ctx.enter_context(tc.tile_pool(name="ppe", bufs=1, space=PSUM))
        pp_ec = ctx.enter_context(tc.tile_pool(name="ppec", bufs=2, space=PSUM))
        pp_seg = ctx.enter_context(tc.tile_pool(name="ppseg", bufs=1, space=PSUM))

        # ---- constants ----
        w1_sb = const.tile([128, KG, 2, KS, 128], fd, tag="w1")
        nc.sync.dma_start(w1_sb[:], w1_d[:])
        w2_sb = const.tile([128, 2, 2, 128], f16, tag="w2")
        nc.sync.dma_start(w2_sb[:], w2_d[:])
        wout_sb = const.tile([128, 2, 1], f16, tag="wout")
        nc.sync.dma_start(wout_sb[:], wout_d[:])
        if fp8 and psl_mode == "hilo8":
            wpa_sb = const.tile([128, 4, 2, 16], f8, tag="wpa")
            nc.sync.dma_start(wpa_sb[:], wpa_d[:])
            wpb_sb = const.tile([128, 4, 2, 16], f8, tag="wpb")
            nc.sync.dma_start(wpb_sb[:], wpb_d[:])
        else:
            wpa_sb = const.tile([128, 8, 1, 1], f8e3 if (fp8 and psl_mode == "e3") else f16, tag="wpa")
            nc.sync.dma_start(wpa_sb[:], wpa_d[:])
            wpb_sb = None
        iota_sb = const.tile([128, SMAX], f16, tag="iota")
        nc.sync.dma_start(iota_sb[:], iota_d[:])
        relb_sb = const.tile([CHUNK, C], f32, tag="relb")
        nc.sync.dma_start(relb_sb[:], relb_d[:])
        nums_sb = const.tile([CHUNK, C], f32, tag="nums")
        nc.sync.dma_start(nums_sb[:], nums_d[:])
        ones_sb = const.tile([97, 1], f16, tag="ones")
        nc.sync.dma_start(ones_sb[:], ones_d[:])

        # species energy per atom: cubic through W_comp[0, 0..3]
        # comp = (c1*n + c0) + n*n*(c3*n + c2)
        t_n2 = const.tile([CHUNK, C], f32, tag="t_n2")
        nc.vector.tensor_mul(t_n2[:], nums_sb[:], nums_sb[:])
        t_a = const.tile([CHUNK, C], f32, tag="t_a")
        nc.vector.tensor_scalar(t_a[:], nums_sb[:], c3, c2, ALU.mult, ALU.add)
        t_b = const.tile([CHUNK, C], f32, tag="t_b")
        nc.vector.tensor_mul(t_b[:], t_n2[:], t_a[:])
        t_c = const.tile([CHUNK, C], f32, tag="t_c")
        nc.vector.tensor_scalar(t_c[:], nums_sb[:], c1, c0, ALU.mult, ALU.add)
        comp_sb = const.tile([CHUNK, C], f32, tag="comp")
        nc.vector.tensor_add(comp_sb[:], t_b[:], t_c[:])

        seg_ps = pp_seg.tile([1, SMAX], f32, tag="seg")
        nc.vector.memset(seg_ps[:], 0.0)
        # e-partials bank: psl rows 0/32, psnn rows 64/96, with a +16
        # partition offset on odd tiles (same PSUM quadrant, so the
        # matmul dst stays valid) — in-bank double buffering that breaks
        # the psl(t+1) <- e_row-copy(t) WAR. Rows in between stay 0 from
        # this one-time clear; the K=113 ones-matmul sums the partials.
        e_ps = pp_e.tile([128, TILE], f32, tag="e")
        nc.vector.memset(e_ps[:], 0.0)


        for t in range(nT):
            # ---- contiguous loads: [128, 8, 512]
            big = psTp.tile([128, 8, TILE], fd, tag="psT", name=f"psT{t}")
            nc.sync.dma_start(big[:], psT_d[:, t])
            if fp8:
                d2 = {"hilo8": f8, "e3": f8e3, "fp16": f16}[psl_mode]
                big2 = psTp2.tile([128, 8, TILE], d2, tag="psT2", name=f"psT2_{t}")
                nc.sync.dma_start(big2[:], psl2_d[:, t])

            # ---- h1: KG k-groups x 2 M-chunks
            h1ps = [pp_h1.tile([128, TILE], f32, tag=f"h1m{m}", name=f"h1ps{t}_{m}") for m in range(2)]
            for j in range(KG):
                for m in range(2):
                    nc.tensor.matmul(
                        h1ps[m][:],
                        w1_sb[:, j, m],
                        big[:, KS * j : KS * (j + 1)],
                        start=(j == 0),
                        stop=(j == KG - 1),
                        perf_mode=DR,
                    )
            # ---- psl: M=1 matmuls in column groups, partial sums landing
            # on e_ps rows 0/32/64/96.
            if fp8 and psl_mode == "hilo8":
                # hi*w_hi + lo*w_hi + hi*w_lo, all fp8 DoubleRow.
                # DoubleRow forbids col-group tiling (col_grp must be 0xf)
                # so every partial accumulates onto e_ps row 0.
                for cn, (wp, mv) in enumerate(
                    [(wpa_sb, big), (wpa_sb, big2), (wpb_sb, big)]
                ):
                    for j in range(4):
                        nc.tensor.matmul(
                            e_ps[0:1, :],
                            wp[:, j, :, 0:1],
                            mv[:, 2 * j : 2 * j + 2],
                            start=(cn == 0 and j == 0),
                            stop=False,
                            perf_mode=DR,
                        )
            elif fp8:
                # psl from the scaled-e3m4/fp16 psT copy: fast M=1 matmuls
                # in column groups 0/1, partials on e_ps rows 0/32 (the
                # 1/(s*t) unscale rides in the ones vector rows 0:33)
                for k in range(8):
                    g = 32 * (k % 2)
                    nc.tensor.matmul(
                        e_ps[g : g + 1, :],
                        wpa_sb[:, k],
                        big2[:, k],
                        start=(k < 2),
                        stop=False,
                        tile_position=(0, g),
                    )
            else:
                for k in range(8):
                    g = 32 * (k % 4)
                    nc.tensor.matmul(
                        e_ps[g : g + 1, :],
                        wpa_sb[:, k],
                        big[:, k],
                        start=(k < 4),
                        stop=False,
                        tile_position=(0, g),
                    )

            sil1 = silp.tile([128, 2, TILE], f16, tag="sil1")
            for m in range(2):
                nc.scalar.activation(sil1[:, m], h1ps[m][:], AF.Silu)

            # ---- h2 (fp16)
            h2ps = [pp_h2.tile([128, TILE], f32, tag=f"h2m{m}", name=f"h2ps{t}_{m}") for m in range(2)]
            for kj in range(2):
                for m in range(2):
                    nc.tensor.matmul(
                        h2ps[m][:],
                        w2_sb[:, m, kj],
                        sil1[:, kj],
                        start=(kj == 0),
                        stop=(kj == 1),
                    )
            sil2 = silp.tile([128, 2, TILE], f16, tag="sil2")
            for m in range(2):
                nc.scalar.activation(sil2[:, m], h2ps[m][:], AF.Silu)

            # ---- psnn (fp16): fresh rows 64/96 in fp8 modes (psl only
            # occupies rows 0/32); fp16 mode accumulates onto the psl rows
            for kj in range(2):
                if fp8:
                    g, st = 64 + 32 * kj, True
                else:
                    g, st = 32 * kj, False
                nc.tensor.matmul(
                    e_ps[g : g + 1, :],
                    wout_sb[:, kj],
                    sil2[:, kj],
                    start=st,
                    stop=(kj == 1),
                    tile_position=(0, g),
                )
            # partial rows 0/32/64/96 (zeros between) -> SBUF in one copy
            e_row = rowp.tile([97, TILE], f16, tag="erow")
            nc.vector.tensor_copy(e_row[:], e_ps[0:97, :])

            # ---- per-chunk: column-ize (K=97 sums the partials), add
            # species energy, segment matmul
            for cc in range(4):
                ch = t * 4 + cc
                ec_ps = pp_ec.tile([128, 1], f32, tag="ec")
                nc.tensor.matmul(
                    ec_ps[:],
                    e_row[0:97, cc * 128 : (cc + 1) * 128],
                    ones_sb[:],
                    start=True,
                    stop=True,
                )
                e_col = rowp.tile([128, 1], f16, tag="ecol")
                nc.vector.tensor_add(e_col[:], ec_ps[:], comp_sb[:, ch : ch + 1])
                if segw is not None:
                    # host guarantees this chunk's structures fall inside
                    # [w0, w0+W); relb is pre-shifted by -w0 per chunk
                    W, w0s = segw
                    w0 = w0s[ch]
                    oh = rowp.tile([128, W], f16, tag="oh")
                    nc.vector.tensor_scalar(
                        oh[:], iota_sb[:, 0:W], relb_sb[:, ch : ch + 1], None,
                        ALU.is_equal,
                    )
                    nc.tensor.matmul(
                        seg_ps[0:1, w0 : w0 + W], e_col[:], oh[:],
                        start=False, stop=(ch == C - 1), skip_group_check=True,
                    )
                else:
                    oh = rowp.tile([128, SMAX], f16, tag="oh")
                    nc.vector.tensor_scalar(
                        oh[:], iota_sb[:], relb_sb[:, ch : ch + 1], None, ALU.is_equal
                    )
                    nc.tensor.matmul(
                        seg_ps[:], e_col[:], oh[:], start=(ch == 0), stop=(ch == C - 1)
                    )

        out_sb = rowp.tile([1, SMAX], f32, tag="outsb")
        nc.scalar.activation(out_sb[:], seg_ps[:], AF.Copy)
        nc.sync.dma_start(out_d[:], out_sb[:])

    _split_waits(nc, mybir)
    return nc


def _install_ntff_hook():
    """Register the axon NTFF profile hook (missing antenv.axon_hooks in
    this image) so run_bass_kernel_spmd(trace=True) can report exec_time_ns."""
    import sys
    import types

    try:
        import antenv.axon_hooks  # noqa: F401

        return
    except ImportError:
        pass
    from trn_agent_boot.trn_boot import _ntff_profile_via_ctypes

    hook = _ntff_profile_via_ctypes("/opt/axon/libaxon_pjrt.so")
    mod = types.ModuleType("antenv.axon_hooks")
    mod.get_axon_ntff_profile_hook = lambda: hook
    mod.set_axon_ntff_profile_hook = lambda h: None
    sys.modules["antenv.axon_hooks"] = mod
    import antenv

    antenv.axon_hooks = mod
    import concourse.bass_utils as bu

    bu.upload_artifacts = lambda tmpdir: tmpdir


def _to_psT(pss, Ta):
    """[Ta, 1024] -> [128, nT, 8, TILE] with [p,t,k,a] = pss[t*TILE+a, k*128+p]"""
    nT = Ta // TILE
    return np.ascontiguousarray(pss.reshape(nT, TILE, 8, 128).transpose(3, 0, 2, 1))


def _to_psT_i(pss, Ta):
    """[Ta, 1024] -> [128, nT, 4, TILE, 2] byte-interleaved k-pairs:
    [p, t, j, a, i] = pss[t*TILE+a, (2j+i)*128+p]"""
    nT = Ta // TILE
    return np.ascontiguousarray(
        pss.reshape(nT, TILE, 4, 2, 128).transpose(4, 0, 2, 1, 3)
    )


def kernel(ps, numbers, batch, W_comp, W_psl, W_h1, W_h2, W_out):
    global LAST_EXEC_NS, LAST_RESULTS
    from concourse.bass_utils import run_bass_kernel_spmd

    if TRACE:
        _install_ntff_hook()

    fp8 = MODE == "fp8mix"
    DT = F8 if fp8 else np.float16
    KG = 4 if fp8 else 8
    KS = 2 if fp8 else 1

    ps = np.asarray(ps)
    numbers = np.asarray(numbers)
    batch = np.asarray(batch)
    W_comp = np.asarray(W_comp, dtype=np.float32)
    W_psl = np.asarray(W_psl, dtype=np.float32)
    W_h1 = np.asarray(W_h1, dtype=np.float32)
    W_h2 = np.asarray(W_h2, dtype=np.float32)
    W_out = np.asarray(W_out, dtype=np.float32)

    counts = np.bincount(batch, minlength=N_STRUCT)
    cum = np.zeros(N_STRUCT + 1, dtype=np.int64)
    np.cumsum(counts, out=cum[1:])

    # equal-structure shard cuts (atoms balance to ~1-2% by CLT; keeps
    # per-core structure count fixed at N_STRUCT/N_CORES <= SMAX)
    s_cut = [i * N_STRUCT // N_CORES for i in range(N_CORES + 1)]

    shards = []
    for i in range(N_CORES):
        s_lo, s_hi = s_cut[i], s_cut[i + 1]
        a_lo, a_hi = int(cum[s_lo]), int(cum[s_hi])
        n_at, n_st = a_hi - a_lo, s_hi - s_lo
        assert n_st <= SMAX, f"shard {i}: {n_st} structs > {SMAX}"
        shards.append((s_lo, s_hi, a_lo, a_hi, n_at, n_st))

    Ta = max(s[4] for s in shards)
    Ta = (Ta + TILE - 1) // TILE * TILE
    nT = Ta // TILE
    C = Ta // CHUNK

    # replicated weights: w1[p, j, m, i, c] = W_h1[m*128+c, (KS*j+i)*128+p]
    w1 = np.ascontiguousarray(
        W_h1.T.reshape(KG, KS, 128, 2, 128).transpose(2, 0, 3, 1, 4)
    ).astype(DT)
    # w2[p, m, i, c] = W_h2[m*128+c, i*128+p]  (fp16)
    w2 = np.ascontiguousarray(
        W_h2.T.reshape(2, 128, 2, 128).transpose(1, 2, 0, 3)
    ).astype(np.float16)
    # wout[p, i] = W_out[0, i*128+p]  (fp16)
    wout = np.ascontiguousarray(
        W_out[0].reshape(2, 128).T.reshape(128, 2, 1)
    ).astype(np.float16)
    # psl weights
    if fp8 and PSL_MODE == "hilo8":
        wp32 = W_psl[0].astype(np.float32)
        wp_hi8 = wp32.astype(F8)
        wp_lo8 = (wp32 - wp_hi8.astype(np.float32)).astype(F8)
        def pack(w):
            out = np.zeros((128, 4, 2, 16), dtype=F8)
            out[..., 0] = np.asarray(w).reshape(4, 2, 128).transpose(2, 0, 1)
            return out

        wpsl_hi, wpsl_lo = pack(wp_hi8), pack(wp_lo8)
    elif fp8 and PSL_MODE == "e3":
        t_w = 15.5 / max(float(np.abs(W_psl).max()), 1e-30)
        _WPSL_T = [t_w]
        wpsl16 = np.asarray(
            np.clip(W_psl[0].astype(np.float32) * t_w, -15.5, 15.5)
            .reshape(8, 128).T.reshape(128, 8, 1, 1),
            dtype=ml_dtypes.float8_e3m4,
        ).copy()
    else:
        wpsl16 = np.ascontiguousarray(
            W_psl[0].reshape(8, 128).T.reshape(128, 8, 1, 1)
        ).astype(np.float16)
    iota = np.ascontiguousarray(
        np.tile(np.arange(SMAX, dtype=np.float16), (128, 1))
    )
    # exact cubic through the 4 species energies
    V = np.vander(np.arange(N_SPECIES, dtype=np.float64), 4, increasing=True)
    poly = np.linalg.solve(V, W_comp[0, :N_SPECIES].astype(np.float64))

    # per-shard relative batch ids (padding = -1)
    rb_all = []
    for s_lo, s_hi, a_lo, a_hi, n_at, n_st in shards:
        rb = np.full(Ta, -1.0, dtype=np.float32)
        rb[:n_at] = (batch[a_lo:a_hi] - s_lo).astype(np.float32)
        rb_all.append(rb)

    segw = None
    if SEG_WINDOWED:
        # shared per-chunk structure windows: batch is sorted so each
        # 128-atom chunk only spans a few structures; bake [w0, w0+W)
        m0 = np.full(C, np.inf)
        m1 = np.full(C, -np.inf)
        for rb in rb_all:
            r2 = rb.reshape(C, CHUNK)
            mask = r2 >= 0
            has = mask.any(axis=1)
            lo = np.where(has, np.where(mask, r2, np.inf).min(axis=1), np.inf)
            hi = np.where(has, np.where(mask, r2, -np.inf).max(axis=1), -np.inf)
            m0 = np.minimum(m0, lo)
            m1 = np.maximum(m1, hi)
        w0 = np.where(np.isfinite(m0), m0, 0.0)
        span = np.where(np.isfinite(m1), m1 - w0 + 1, 1.0)
        W = int(max(1, span.max()))
        W = min((W + 3) // 4 * 4, SMAX)
        w0 = np.clip(w0, 0, SMAX - W).astype(np.int64)
        segw = (W, tuple(int(x) for x in w0))

    key = (Ta, C, tuple(np.round(poly, 12)), MODE, PSL_MODE, segw)
    if key not in _BUILD_CACHE:
        _BUILD_CACHE.clear()
        _BUILD_CACHE[key] = _build(Ta, C, poly, MODE, PSL_MODE, segw)
    nc = _BUILD_CACHE[key]

    psq = np.asarray(ps, dtype=DT)  # quantize once, slice per shard
    ones = np.ones((97, 1), dtype=np.float16)
    if fp8 and PSL_MODE == "hilo8":
        ps2 = (ps.astype(np.float32) - psq.astype(np.float32)).astype(F8)
    elif fp8 and PSL_MODE == "e3":
        # scale ps and wpsl into e3m4's normal range (subnormals would
        # cost ~8% error on small values); the 1/(s*t) unscale rides in
        # the colize ones vector rows 0:33 (psl partial rows 0/32)
        s_ps = 15.5 / max(float(np.abs(ps).max()), 1e-30)
        ps2 = np.asarray(
            np.clip(ps.astype(np.float32) * s_ps, -15.5, 15.5),
            dtype=ml_dtypes.float8_e3m4,
        )
        ones[0:33] = np.float16(1.0 / (s_ps * _WPSL_T[0]))
    elif fp8:
        ps2 = ps.astype(np.float16)

    in_maps = []
    for si, (s_lo, s_hi, a_lo, a_hi, n_at, n_st) in enumerate(shards):
        pss = np.zeros((Ta, N_FEAT), dtype=DT)
        pss[:n_at] = psq[a_lo:a_hi]
        rb = rb_all[si]
        if segw is not None:
            rb = rb.reshape(C, CHUNK) - np.asarray(segw[1], dtype=np.float32)[:, None]
            rb = rb.reshape(Ta)
        nums = np.zeros(Ta, dtype=np.float32)
        nums[:n_at] = numbers[a_lo:a_hi].astype(np.float32)
        im = {
            "psT": _to_psT(pss, Ta),
            "relb": np.ascontiguousarray(rb.reshape(C, CHUNK).T),
            "nums": np.ascontiguousarray(nums.reshape(C, CHUNK).T),
            "w1": w1,
            "w2": w2,
            "wout": wout,
            "ones": ones,
            "iota": iota,
        }
        if fp8 and PSL_MODE == "hilo8":
            pss2 = np.zeros((Ta, N_FEAT), dtype=F8)
            pss2[:n_at] = ps2[a_lo:a_hi]
            im["psT2"] = _to_psT(pss2, Ta)
            im["wpsl_hi"] = wpsl_hi
            im["wpsl_lo"] = wpsl_lo
        elif fp8:
            pss2 = np.zeros((Ta, N_FEAT), dtype=ps2.dtype)
            pss2[:n_at] = ps2[a_lo:a_hi]
            im["psT2"] = _to_psT(pss2, Ta)
            im["wpsl"] = wpsl16
        else:
            im["wpsl"] = wpsl16
        in_maps.append(im)

    res = run_bass_kernel_spmd(nc, in_maps, list(range(N_CORES)), trace=TRACE)
    LAST_EXEC_NS = res.exec_time_ns
    LAST_RESULTS = res

    out = np.zeros((N_STRUCT, 1), dtype=np.float32)
    for i, (s_lo, s_hi, a_lo, a_hi, n_at, n_st) in enumerate(shards):
        vals = res.results[i]["out"][0, :n_st].astype(np.float32)
        empty = counts[s_lo:s_hi] == 0
        if empty.any():
            vals = np.where(empty, 0.0, vals)
        out[s_lo:s_hi, 0] = vals
    return out


# revision 30
# speedup vs baseline: 1.1947x; 1.0139x over previous
"""PowerSpectrumModel Trainium2 kernel (8 NeuronCores, SPMD).

Strategy (data-parallel over atoms, structures disjoint per shard):
 - Host: cut the atom axis at structure boundaries into 8 balanced shards;
   pre-TRANSPOSE ps to feature-major [128, nT, 8, TILE] (so every tile
   load is one contiguous stride per partition — no on-device transpose
   descriptors); fp32 PSUM accumulation on device; replicate the small
   weight matrices.
 - Precision (fp8mix mode): the big h1 GEMM runs fp8e4m3 DoubleRow (2x
   PE rate; w1-quant + ps-quant contribute only ~6e-3 rel err because
   errors decorrelate across atoms); h2/psnn stay fp16 (their weight
   quantization biases the per-atom energy coherently across a structure
   -> 3.6e-2 if fp8); psl gets hi+lo fp8 splits of both ps and W_psl
   (or an extra fp16 psT copy, PSL_MODE).
 - Device, per 512-atom tile:
     psT   <- contiguous loads [128, 8, 512]                      [DMA]
     h1    = W_h1 @ psT  (fp8 DoubleRow k-pairs / fp16 k-chunks)  [PE]
     psl   = W_psl row, M=1 matmuls -> e partial rows 0/32/64/96  [PE]
     sil1  = silu(h1)                                             [ACT]
     h2    = W_h2 @ sil1; sil2 = silu(h2)                         [PE/ACT]
     psnn  M=1 matmuls accumulated onto the psl PSUM rows         [PE]
     per 128-chunk: e column via K=97 ones matmul, + species
     energy, one-hot(struct) matmul accumulates into a [1,256]
     PSUM row holding this core's per-structure energies.         [PE/DVE]
 - Host: slice per-core structure ranges, concat -> [2000, 1].
"""

import numpy as np
import ml_dtypes

N_ATOMS = 200000
N_FEAT = 1024
N_SPECIES = 4
N_STRUCT = 2000
H1 = 256
H2 = 256
SCALE = 1.0
N_CORES = 8
TILE = 512
CHUNK = 128
SMAX = 256  # per-core structure capacity (PSUM row)

MODE = "fp8mix"  # "fp16" | "fp8mix"
PSL_MODE = "e3"  # "e3" | "hilo8" | "fp16"
SEG_WINDOWED = True  # bake per-chunk structure windows into the seg matmul

_BUILD_CACHE = {}
TRACE = False
LAST_EXEC_NS = None
LAST_RESULTS = None

F8 = ml_dtypes.float8_e4m3


def _split_waits(nc, mybir, maxw=1):
    """walrus on this build rejects >1 sync wait per instruction; move
    overflow waits onto preceding same-engine NoOps."""
    cnt = 0
    for f in nc.m.functions:
        for blk in f.blocks:
            if not hasattr(blk, "instructions"):
                continue
            out = []
            changed = False
            for inst in blk.instructions:
                si = getattr(inst, "sync_info", None)
                if si is not None and si.on_wait and len(si.on_wait) > maxw:
                    waits = list(si.on_wait)
                    keep = waits[-maxw:]
                    extra = waits[:-maxw]
                    while extra:
                        chunk, extra = extra[:maxw], extra[maxw:]
                        cnt += 1
                        out.append(
                            mybir.InstNoOp(
                                name=f"waitfix-{cnt}",
                                engine=inst.engine,
                                text_hint="waitfix",
                                bass_nofuse=True,
                                ins=[],
                                outs=[],
                                sync_info=mybir.SyncInfo(on_wait=chunk, on_update=[]),
                            )
                        )
                    si.on_wait = keep
                    changed = True
                out.append(inst)
            if changed:
                blk.instructions[:] = out
    return cnt


def _build(Ta, C, poly, mode, psl_mode, segw=None):
    import concourse.bass as bass
    import concourse.tile as tile
    import concourse.mybir as mybir
    from contextlib import ExitStack

    fp8 = mode == "fp8mix"
    f8 = mybir.dt.float8e4
    f16 = mybir.dt.float16
    f32 = mybir.dt.float32
    fd = f8 if fp8 else f16  # h1 GEMM dtype
    AF = mybir.ActivationFunctionType
    ALU = mybir.AluOpType
    DR = mybir.MatmulPerfMode.DoubleRow if fp8 else None
    PSUM = bass.MemorySpace.PSUM
    nT = Ta // TILE
    c0, c1, c2, c3 = (float(x) for x in poly)
    KG = 4 if fp8 else 8  # h1 k-groups (DoubleRow contracts pairs)
    KS = 2 if fp8 else 1  # k-chunks per h1 matmul

    nc = bass.Bass("TRN2", target_bir_lowering=False, debug=False)

    # pre-transposed ps: [p, t, k, a] = ps[t*512+a, k*128+p]
    psT_d = nc.dram_tensor("psT", [128, nT, 8, TILE], fd, kind="ExternalInput").ap()
    f8e3 = mybir.dt.float8e3
    psl2_d = None
    if fp8:
        d2 = {"hilo8": f8, "e3": f8e3, "fp16": f16}[psl_mode]
        psl2_d = nc.dram_tensor("psT2", [128, nT, 8, TILE], d2, kind="ExternalInput").ap()
    relb_d = nc.dram_tensor("relb", [CHUNK, C], f32, kind="ExternalInput").ap()
    nums_d = nc.dram_tensor("nums", [CHUNK, C], f32, kind="ExternalInput").ap()
    # w1: [p, j, m, i, c] = W_h1[m*128+c, (KS*j+i)*128+p]
    w1_d = nc.dram_tensor("w1", [128, KG, 2, KS, 128], fd, kind="ExternalInput").ap()
    # w2: [p, m, i, c] = W_h2[m*128+c, i*128+p]  (fp16)
    w2_d = nc.dram_tensor("w2", [128, 2, 2, 128], f16, kind="ExternalInput").ap()
    # wout: [p, i] = W_out[0, i*128+p]  (fp16)
    wout_d = nc.dram_tensor("wout", [128, 2, 1], f16, kind="ExternalInput").ap()
    # psl weights
    if fp8 and psl_mode == "hilo8":
        # k-subtile dim padded to step 16 (ISA: dual-fp8 LW step%16==0)
        wpa_d = nc.dram_tensor("wpsl_hi", [128, 4, 2, 16], f8, kind="ExternalInput").ap()
        wpb_d = nc.dram_tensor("wpsl_lo", [128, 4, 2, 16], f8, kind="ExternalInput").ap()
    elif fp8 and psl_mode == "e3":
        wpa_d = nc.dram_tensor("wpsl", [128, 8, 1, 1], f8e3, kind="ExternalInput").ap()
        wpb_d = None
    else:
        wpa_d = nc.dram_tensor("wpsl", [128, 8, 1, 1], f16, kind="ExternalInput").ap()
        wpb_d = None
    ones_d = nc.dram_tensor("ones", [97, 1], f16, kind="ExternalInput").ap()
    iota_d = nc.dram_tensor("iota", [128, SMAX], f16, kind="ExternalInput").ap()
    out_d = nc.dram_tensor("out", [1, SMAX], f32, kind="ExternalOutput").ap()

    with tile.TileContext(nc) as tc, ExitStack() as ctx:
        const = ctx.enter_context(tc.tile_pool(name="const", bufs=1))
        psTp = ctx.enter_context(tc.tile_pool(name="psT", bufs=6))
        psTp2 = ctx.enter_context(tc.tile_pool(name="psT2", bufs=6)) if fp8 else None
        silp = ctx.enter_context(tc.tile_pool(name="sil", bufs=4))
        rowp = ctx.enter_context(tc.tile_pool(name="row", bufs=6))
        pp_h1 = ctx.enter_context(tc.tile_pool(name="pph1", bufs=1, space=PSUM))
        pp_h2 = ctx.enter_context(tc.tile_pool(name="pph2", bufs=1, space=PSUM))
        pp_e = ctx.enter_context(tc.tile_pool(name="ppe", bufs=1, space=PSUM))
        pp_ec = ctx.enter_context(tc.tile_pool(name="ppec", bufs=2, space=PSUM))
        pp_seg = ctx.enter_context(tc.tile_pool(name="ppseg", bufs=1, space=PSUM))

        # ---- constants ----
        w1_sb = const.tile([128, KG, 2, KS, 128], fd, tag="w1")
        nc.sync.dma_start(w1_sb[:], w1_d[:])
        w2_sb = const.tile([128, 2, 2, 128], f16, tag="w2")
        nc.sync.dma_start(w2_sb[:], w2_d[:])
        wout_sb = const.tile([128, 2, 1], f16, tag="wout")
        nc.sync.dma_start(wout_sb[:], wout_d[:])
        if fp8 and psl_mode == "hilo8":
            wpa_sb = const.tile([128, 4, 2, 16], f8, tag="wpa")
            nc.sync.dma_start(wpa_sb[:], wpa_d[:])
            wpb_sb = const.tile([128, 4, 2, 16], f8, tag="wpb")
            nc.sync.dma_start(wpb_sb[:], wpb_d[:])
        else:
            wpa_sb = const.tile([128, 8, 1, 1], f8e3 if (fp8 and psl_mode == "e3") else f16, tag="wpa")
            nc.sync.dma_start(wpa_sb[:], wpa_d[:])
            wpb_sb = None
        iota_sb = const.tile([128, SMAX], f16, tag="iota")
        nc.sync.dma_start(iota_sb[:], iota_d[:])
        relb_sb = const.tile([CHUNK, C], f32, tag="relb")
        nc.sync.dma_start(relb_sb[:], relb_d[:])
        nums_sb = const.tile([CHUNK, C], f32, tag="nums")
        nc.sync.dma_start(nums_sb[:], nums_d[:])
        ones_sb = const.tile([97, 1], f16, tag="ones")
        nc.sync.dma_start(ones_sb[:], ones_d[:])

        # species energy per atom: cubic through W_comp[0, 0..3]
        # comp = (c1*n + c0) + n*n*(c3*n + c2)
        t_n2 = const.tile([CHUNK, C], f32, tag="t_n2")
        nc.vector.tensor_mul(t_n2[:], nums_sb[:], nums_sb[:])
        t_a = const.tile([CHUNK, C], f32, tag="t_a")
        nc.vector.tensor_scalar(t_a[:], nums_sb[:], c3, c2, ALU.mult, ALU.add)
        t_b = const.tile([CHUNK, C], f32, tag="t_b")
        nc.vector.tensor_mul(t_b[:], t_n2[:], t_a[:])
        t_c = const.tile([CHUNK, C], f32, tag="t_c")
        nc.vector.tensor_scalar(t_c[:], nums_sb[:], c1, c0, ALU.mult, ALU.add)
        comp_sb = const.tile([CHUNK, C], f32, tag="comp")
        nc.vector.tensor_add(comp_sb[:], t_b[:], t_c[:])

        seg_ps = pp_seg.tile([1, SMAX], f32, tag="seg")
        nc.vector.memset(seg_ps[:], 0.0)
        # e-partials bank: psl rows 0/32, psnn rows 64/96, with a +16
        # partition offset on odd tiles (same PSUM quadrant, so the
        # matmul dst stays valid) — in-bank double buffering that breaks
        # the psl(t+1) <- e_row-copy(t) WAR. Rows in between stay 0 from
        # this one-time clear; the K=113 ones-matmul sums the partials.
        e_ps = pp_e.tile([128, TILE], f32, tag="e")
        nc.vector.memset(e_ps[:], 0.0)


        for t in range(nT):
            # ---- contiguous loads: [128, 8, 512]
            big = psTp.tile([128, 8, TILE], fd, tag="psT", name=f"psT{t}")
            nc.sync.dma_start(big[:], psT_d[:, t])
            if fp8:
                d2 = {"hilo8": f8, "e3": f8e3, "fp16": f16}[psl_mode]
                big2 = psTp2.tile([128, 8, TILE], d2, tag="psT2", name=f"psT2_{t}")
                nc.sync.dma_start(big2[:], psl2_d[:, t])

            # ---- h1: KG k-groups x 2 M-chunks
            h1ps = [pp_h1.tile([128, TILE], f32, tag=f"h1m{m}", name=f"h1ps{t}_{m}") for m in range(2)]
            for j in range(KG):
                for m in range(2):
                    nc.tensor.matmul(
                        h1ps[m][:],
                        w1_sb[:, j, m],
                        big[:, KS * j : KS * (j + 1)],
                        start=(j == 0),
                        stop=(j == KG - 1),
                        perf_mode=DR,
                    )
            # ---- psl: M=1 matmuls in column groups, partial sums landing
            # on e_ps rows 0/32/64/96.
            if fp8 and psl_mode == "hilo8":
                # hi*w_hi + lo*w_hi + hi*w_lo, all fp8 DoubleRow.
                # DoubleRow forbids col-group tiling (col_grp must be 0xf)
                # so every partial accumulates onto e_ps row 0.
                for cn, (wp, mv) in enumerate(
                    [(wpa_sb, big), (wpa_sb, big2), (wpb_sb, big)]
                ):
                    for j in range(4):
                        nc.tensor.matmul(
                            e_ps[0:1, :],
                            wp[:, j, :, 0:1],
                            mv[:, 2 * j : 2 * j + 2],
                            start=(cn == 0 and j == 0),
                            stop=False,
                            perf_mode=DR,
                        )
            elif fp8:
                # psl from the scaled-e3m4/fp16 psT copy: fast M=1 matmuls
                # in column groups 0/1, partials on e_ps rows 0/32 (the
                # 1/(s*t) unscale rides in the ones vector rows 0:33)
                for k in range(8):
                    g = 32 * (k % 2)
                    nc.tensor.matmul(
                        e_ps[g : g + 1, :],
                        wpa_sb[:, k],
                        big2[:, k],
                        start=(k < 2),
                        stop=False,
                        tile_position=(0, g),
                    )
            else:
                for k in range(8):
                    g = 32 * (k % 4)
                    nc.tensor.matmul(
                        e_ps[g : g + 1, :],
                        wpa_sb[:, k],
                        big[:, k],
                        start=(k < 4),
                        stop=False,
                        tile_position=(0, g),
                    )

            sil1 = silp.tile([128, 2, TILE], f16, tag="sil1")
            for m in range(2):
                nc.scalar.activation(sil1[:, m], h1ps[m][:], AF.Silu)

            # ---- h2 (fp16)
            h2ps = [pp_h2.tile([128, TILE], f32, tag=f"h2m{m}", name=f"h2ps{t}_{m}") for m in range(2)]
            for kj in range(2):
                for m in range(2):
                    nc.tensor.matmul(
                        h2ps[m][:],
                        w2_sb[:, m, kj],
                        sil1[:, kj],
                        start=(kj == 0),
                        stop=(kj == 1),
                    )
            sil2 = silp.tile([128, 2, TILE], f16, tag="sil2")
            for m in range(2):
                nc.scalar.activation(sil2[:, m], h2ps[m][:], AF.Silu)

            # ---- psnn (fp16): fresh rows 64/96 in fp8 modes (psl only
            # occupies rows 0/32); fp16 mode accumulates onto the psl rows
            for kj in range(2):
                if fp8:
                    g, st = 64 + 32 * kj, True
                else:
                    g, st = 32 * kj, False
                nc.tensor.matmul(
                    e_ps[g : g + 1, :],
                    wout_sb[:, kj],
                    sil2[:, kj],
                    start=st,
                    stop=(kj == 1),
                    tile_position=(0, g),
                )
            # partial rows 0/32/64/96 (zeros between) -> SBUF in one copy
            e_row = rowp.tile([97, TILE], f16, tag="erow")
            nc.vector.tensor_copy(e_row[:], e_ps[0:97, :])

            # ---- per-chunk: column-ize (K=97 sums the partials), add
            # species energy, segment matmul
            for cc in range(4):
                ch = t * 4 + cc
                ec_ps = pp_ec.tile([128, 1], f32, tag="ec")
                nc.tensor.matmul(
                    ec_ps[:],
                    e_row[0:97, cc * 128 : (cc + 1) * 128],
                    ones_sb[:],
                    start=True,
                    stop=True,
                )
                e_col = rowp.tile([128, 1], f16, tag="ecol")
                nc.vector.tensor_add(e_col[:], ec_ps[:], comp_sb[:, ch : ch + 1])
                if segw is not None:
                    # host guarantees this chunk's structures fall inside
                    # [w0, w0+W); relb is pre-shifted by -w0 per chunk
                    W, w0s = segw
                    w0 = w0s[ch]
                    oh = rowp.tile([128, W], f16, tag="oh")
                    nc.vector.tensor_scalar(
                        oh[:], iota_sb[:, 0:W], relb_sb[:, ch : ch + 1], None,
                        ALU.is_equal,
                    )
                    nc.tensor.matmul(
                        seg_ps[0:1, w0 : w0 + W], e_col[:], oh[:],
                        start=False, stop=(ch == C - 1), skip_group_check=True,
                    )
                else:
                    oh = rowp.tile([128, SMAX], f16, tag="oh")
                    nc.vector.tensor_scalar(
                        oh[:], iota_sb[:], relb_sb[:, ch : ch + 1], None, ALU.is_equal
                    )
                    nc.tensor.matmul(
                        seg_ps[:], e_col[:], oh[:], start=(ch == 0), stop=(ch == C - 1)
                    )

        out_sb = rowp.tile([1, SMAX], f32, tag="outsb")
        nc.scalar.activation(out_sb[:], seg_ps[:], AF.Copy)
        nc.sync.dma_start(out_d[:], out_sb[:])

    _split_waits(nc, mybir)
    return nc


def _install_ntff_hook():
    """Register the axon NTFF profile hook (missing antenv.axon_hooks in
    this image) so run_bass_kernel_spmd(trace=True) can report exec_time_ns."""
    import sys
    import types

    try:
        import antenv.axon_hooks  # noqa: F401

        return
    except ImportError:
        pass
    from trn_agent_boot.trn_boot import _ntff_profile_via_ctypes

    hook = _ntff_profile_via_ctypes("/opt/axon/libaxon_pjrt.so")
    mod = types.ModuleType("antenv.axon_hooks")
    mod.get_axon_ntff_profile_hook = lambda: hook
    mod.set_axon_ntff_profile_hook = lambda h: None
    sys.modules["antenv.axon_hooks"] = mod
    import antenv

    antenv.axon_hooks = mod
    import concourse.bass_utils as bu

    bu.upload_artifacts = lambda tmpdir: tmpdir


def _to_psT(pss, Ta):
    """[Ta, 1024] -> [128, nT, 8, TILE] with [p,t,k,a] = pss[t*TILE+a, k*128+p]"""
    nT = Ta // TILE
    return np.ascontiguousarray(pss.reshape(nT, TILE, 8, 128).transpose(3, 0, 2, 1))


def _to_psT_i(pss, Ta):
    """[Ta, 1024] -> [128, nT, 4, TILE, 2] byte-interleaved k-pairs:
    [p, t, j, a, i] = pss[t*TILE+a, (2j+i)*128+p]"""
    nT = Ta // TILE
    return np.ascontiguousarray(
        pss.reshape(nT, TILE, 4, 2, 128).transpose(4, 0, 2, 1, 3)
    )


def kernel(ps, numbers, batch, W_comp, W_psl, W_h1, W_h2, W_out):
    global LAST_EXEC_NS, LAST_RESULTS
    from concourse.bass_utils import run_bass_kernel_spmd

    if TRACE:
        _install_ntff_hook()

    fp8 = MODE == "fp8mix"
    DT = F8 if fp8 else np.float16
    KG = 4 if fp8 else 8
    KS = 2 if fp8 else 1

    ps = np.asarray(ps)
    numbers = np.asarray(numbers)
    batch = np.asarray(batch)
    W_comp = np.asarray(W_comp, dtype=np.float32)
    W_psl = np.asarray(W_psl, dtype=np.float32)
    W_h1 = np.asarray(W_h1, dtype=np.float32)
    W_h2 = np.asarray(W_h2, dtype=np.float32)
    W_out = np.asarray(W_out, dtype=np.float32)

    counts = np.bincount(batch, minlength=N_STRUCT)
    cum = np.zeros(N_STRUCT + 1, dtype=np.int64)
    np.cumsum(counts, out=cum[1:])

    # equal-structure shard cuts (atoms balance to ~1-2% by CLT; keeps
    # per-core structure count fixed at N_STRUCT/N_CORES <= SMAX)
    s_cut = [i * N_STRUCT // N_CORES for i in range(N_CORES + 1)]

    shards = []
    for i in range(N_CORES):
        s_lo, s_hi = s_cut[i], s_cut[i + 1]
        a_lo, a_hi = int(cum[s_lo]), int(cum[s_hi])
        n_at, n_st = a_hi - a_lo, s_hi - s_lo
        assert n_st <= SMAX, f"shard {i}: {n_st} structs > {SMAX}"
        shards.append((s_lo, s_hi, a_lo, a_hi, n_at, n_st))

    Ta = max(s[4] for s in shards)
    Ta = (Ta + TILE - 1) // TILE * TILE
    nT = Ta // TILE
    C = Ta // CHUNK

    # replicated weights: w1[p, j, m, i, c] = W_h1[m*128+c, (KS*j+i)*128+p]
    w1 = np.ascontiguousarray(
        W_h1.T.reshape(KG, KS, 128, 2, 128).transpose(2, 0, 3, 1, 4)
    ).astype(DT)
    # w2[p, m, i, c] = W_h2[m*128+c, i*128+p]  (fp16)
    w2 = np.ascontiguousarray(
        W_h2.T.reshape(2, 128, 2, 128).transpose(1, 2, 0, 3)
    ).astype(np.float16)
    # wout[p, i] = W_out[0, i*128+p]  (fp16)
    wout = np.ascontiguousarray(
        W_out[0].reshape(2, 128).T.reshape(128, 2, 1)
    ).astype(np.float16)
    # psl weights
    if fp8 and PSL_MODE == "hilo8":
        wp32 = W_psl[0].astype(np.float32)
        wp_hi8 = wp32.astype(F8)
        wp_lo8 = (wp32 - wp_hi8.astype(np.float32)).astype(F8)
        def pack(w):
            out = np.zeros((128, 4, 2, 16), dtype=F8)
            out[..., 0] = np.asarray(w).reshape(4, 2, 128).transpose(2, 0, 1)
            return out

        wpsl_hi, wpsl_lo = pack(wp_hi8), pack(wp_lo8)
    elif fp8 and PSL_MODE == "e3":
        t_w = 15.5 / max(float(np.abs(W_psl).max()), 1e-30)
        _WPSL_T = [t_w]
        wpsl16 = np.asarray(
            np.clip(W_psl[0].astype(np.float32) * t_w, -15.5, 15.5)
            .reshape(8, 128).T.reshape(128, 8, 1, 1),
            dtype=ml_dtypes.float8_e3m4,
        ).copy()
    else:
        wpsl16 = np.ascontiguousarray(
            W_psl[0].reshape(8, 128).T.reshape(128, 8, 1, 1)
        ).astype(np.float16)
    iota = np.ascontiguousarray(
        np.tile(np.arange(SMAX, dtype=np.float16), (128, 1))
    )
    # exact cubic through the 4 species energies
    V = np.vander(np.arange(N_SPECIES, dtype=np.float64), 4, increasing=True)
    poly = np.linalg.solve(V, W_comp[0, :N_SPECIES].astype(np.float64))

    # per-shard relative batch ids (padding = -1)
    rb_all = []
    for s_lo, s_hi, a_lo, a_hi, n_at, n_st in shards:
        rb = np.full(Ta, -1.0, dtype=np.float32)
        rb[:n_at] = (batch[a_lo:a_hi] - s_lo).astype(np.float32)
        rb_all.append(rb)

    segw = None
    if SEG_WINDOWED:
        # shared per-chunk structure windows: batch is sorted so each
        # 128-atom chunk only spans a few structures; bake [w0, w0+W)
        m0 = np.full(C, np.inf)
        m1 = np.full(C, -np.inf)
        for rb in rb_all:
            r2 = rb.reshape(C, CHUNK)
            mask = r2 >= 0
            has = mask.any(axis=1)
            lo = np.where(has, np.where(mask, r2, np.inf).min(axis=1), np.inf)
            hi = np.where(has, np.where(mask, r2, -np.inf).max(axis=1), -np.inf)
            m0 = np.minimum(m0, lo)
            m1 = np.maximum(m1, hi)
        w0 = np.where(np.isfinite(m0), m0, 0.0)
        span = np.where(np.isfinite(m1), m1 - w0 + 1, 1.0)
        W = int(max(1, span.max()))
        W = min((W + 3) // 4 * 4, SMAX)
        w0 = np.clip(w0, 0, SMAX - W).astype(np.int64)
        segw = (W, tuple(int(x) for x in w0))

    key = (Ta, C, tuple(np.round(poly, 12)), MODE, PSL_MODE, segw)
    if key not in _BUILD_CACHE:
        _BUILD_CACHE.clear()
        _BUILD_CACHE[key] = _build(Ta, C, poly, MODE, PSL_MODE, segw)
    nc = _BUILD_CACHE[key]

    psq = np.asarray(ps, dtype=DT)  # quantize once, slice per shard
    ones = np.ones((97, 1), dtype=np.float16)
    if fp8 and PSL_MODE == "hilo8":
        ps2 = (ps.astype(np.float32) - psq.astype(np.float32)).astype(F8)
    elif fp8 and PSL_MODE == "e3":
        # scale ps and wpsl into e3m4's normal range (subnormals would
        # cost ~8% error on small values); the 1/(s*t) unscale rides in
        # the colize ones vector rows 0:33 (psl partial rows 0/32)
        s_ps = 15.5 / max(float(np.abs(ps).max()), 1e-30)
        ps2 = np.asarray(
            np.clip(ps.astype(np.float32) * s_ps, -15.5, 15.5),
            dtype=ml_dtypes.float8_e3m4,
        )
        ones[0:33] = np.float16(1.0 / (s_ps * _WPSL_T[0]))
    elif fp8:
        ps2 = ps.astype(np.float16)

    in_maps = []
    for si, (s_lo, s_hi, a_lo, a_hi, n_at, n_st) in enumerate(shards):
        pss = np.zeros((Ta, N_FEAT), dtype=DT)
        pss[:n_at] = psq[a_lo:a_hi]
        rb = rb_all[si]
        if segw is not None:
            rb = rb.reshape(C, CHUNK) - np.asarray(segw[1], dtype=np.float32)[:, None]
            rb = rb.reshape(Ta)
        nums = np.zeros(Ta, dtype=np.float32)
        nums[:n_at] = numbers[a_lo:a_hi].astype(np.float32)
        im = {
            "psT": _to_psT(pss, Ta),
            "relb": np.ascontiguousarray(rb.reshape(C, CHUNK).T),
            "nums": np.ascontiguousarray(nums.reshape(C, CHUNK).T),
            "w1": w1,
            "w2": w2,
            "wout": wout,
            "ones": ones,
            "iota": iota,
        }
        if fp8 and PSL_MODE == "hilo8":
            pss2 = np.zeros((Ta, N_FEAT), dtype=F8)
            pss2[:n_at] = ps2[a_lo:a_hi]
            im["psT2"] = _to_psT(pss2, Ta)
            im["wpsl_hi"] = wpsl_hi
            im["wpsl_lo"] = wpsl_lo
        elif fp8:
            pss2 = np.zeros((Ta, N_FEAT), dtype=ps2.dtype)
            pss2[:n_at] = ps2[a_lo:a_hi]
            im["psT2"] = _to_psT(pss2, Ta)
            im["wpsl"] = wpsl16
        else:
            im["wpsl"] = wpsl16
        in_maps.append(im)

    res = run_bass_kernel_spmd(nc, in_maps, list(range(N_CORES)), trace=TRACE)
    LAST_EXEC_NS = res.exec_time_ns
    LAST_RESULTS = res

    out = np.zeros((N_STRUCT, 1), dtype=np.float32)
    for i, (s_lo, s_hi, a_lo, a_hi, n_at, n_st) in enumerate(shards):
        vals = res.results[i]["out"][0, :n_st].astype(np.float32)
        empty = counts[s_lo:s_hi] == 0
        if empty.any():
            vals = np.where(empty, 0.0, vals)
        out[s_lo:s_hi, 0] = vals
    return out


# revision 31
# speedup vs baseline: 1.2431x; 1.0405x over previous
"""PowerSpectrumModel Trainium2 kernel (8 NeuronCores, SPMD).

Strategy (data-parallel over atoms, structures disjoint per shard):
 - Host: cut the atom axis at structure boundaries into 8 balanced shards;
   pre-TRANSPOSE ps to feature-major [128, nT, 8, TILE] (so every tile
   load is one contiguous stride per partition — no on-device transpose
   descriptors); fp32 PSUM accumulation on device; replicate the small
   weight matrices.
 - Precision (fp8mix mode): the big h1 GEMM runs fp8e4m3 DoubleRow (2x
   PE rate; w1-quant + ps-quant contribute only ~6e-3 rel err because
   errors decorrelate across atoms); h2/psnn stay fp16 (their weight
   quantization biases the per-atom energy coherently across a structure
   -> 3.6e-2 if fp8); psl gets hi+lo fp8 splits of both ps and W_psl
   (or an extra fp16 psT copy, PSL_MODE).
 - Device, per 512-atom tile:
     psT   <- contiguous loads [128, 8, 512]                      [DMA]
     h1    = W_h1 @ psT  (fp8 DoubleRow k-pairs / fp16 k-chunks)  [PE]
     psl   = W_psl row, M=1 matmuls -> e partial rows 0/32/64/96  [PE]
     sil1  = silu(h1)                                             [ACT]
     h2    = W_h2 @ sil1; sil2 = silu(h2)                         [PE/ACT]
     psnn  M=1 matmuls accumulated onto the psl PSUM rows         [PE]
     per 128-chunk: e column via K=97 ones matmul, + species
     energy, one-hot(struct) matmul accumulates into a [1,256]
     PSUM row holding this core's per-structure energies.         [PE/DVE]
 - Host: slice per-core structure ranges, concat -> [2000, 1].
"""

import numpy as np
import ml_dtypes

N_ATOMS = 200000
N_FEAT = 1024
N_SPECIES = 4
N_STRUCT = 2000
H1 = 256
H2 = 256
SCALE = 1.0
N_CORES = 8
TILE = 512
CHUNK = 128
SMAX = 256  # per-core structure capacity (PSUM row)

MODE = "fp8mix"  # "fp16" | "fp8mix"
PSL_MODE = "e3"  # "e3" | "hilo8" | "fp16"
SEG_WINDOWED = True  # bake per-chunk structure windows into the seg matmul

_BUILD_CACHE = {}
TRACE = False
LAST_EXEC_NS = None
LAST_RESULTS = None

F8 = ml_dtypes.float8_e4m3


def _split_waits(nc, mybir, maxw=1):
    """walrus on this build rejects >1 sync wait per instruction; move
    overflow waits onto preceding same-engine NoOps."""
    cnt = 0
    for f in nc.m.functions:
        for blk in f.blocks:
            if not hasattr(blk, "instructions"):
                continue
            out = []
            changed = False
            for inst in blk.instructions:
                si = getattr(inst, "sync_info", None)
                if si is not None and si.on_wait and len(si.on_wait) > maxw:
                    waits = list(si.on_wait)
                    keep = waits[-maxw:]
                    extra = waits[:-maxw]
                    while extra:
                        chunk, extra = extra[:maxw], extra[maxw:]
                        cnt += 1
                        out.append(
                            mybir.InstNoOp(
                                name=f"waitfix-{cnt}",
                                engine=inst.engine,
                                text_hint="waitfix",
                                bass_nofuse=True,
                                ins=[],
                                outs=[],
                                sync_info=mybir.SyncInfo(on_wait=chunk, on_update=[]),
                            )
                        )
                    si.on_wait = keep
                    changed = True
                out.append(inst)
            if changed:
                blk.instructions[:] = out
    return cnt


def _build(Ta, C, poly, mode, psl_mode, segw=None):
    import concourse.bass as bass
    import concourse.tile as tile
    import concourse.mybir as mybir
    from contextlib import ExitStack

    fp8 = mode == "fp8mix"
    f8 = mybir.dt.float8e4
    f16 = mybir.dt.float16
    f32 = mybir.dt.float32
    fd = f8 if fp8 else f16  # h1 GEMM dtype
    AF = mybir.ActivationFunctionType
    ALU = mybir.AluOpType
    DR = mybir.MatmulPerfMode.DoubleRow if fp8 else None
    PSUM = bass.MemorySpace.PSUM
    nT = Ta // TILE
    c0, c1, c2, c3 = (float(x) for x in poly)
    KG = 4 if fp8 else 8  # h1 k-groups (DoubleRow contracts pairs)
    KS = 2 if fp8 else 1  # k-chunks per h1 matmul

    nc = bass.Bass("TRN2", target_bir_lowering=False, debug=False)

    # pre-transposed ps: [p, t, k, a] = ps[t*512+a, k*128+p]
    psT_d = nc.dram_tensor("psT", [128, nT, 8, TILE], fd, kind="ExternalInput").ap()
    f8e3 = mybir.dt.float8e3
    psl2_d = None
    if fp8:
        d2 = {"hilo8": f8, "e3": f8e3, "fp16": f16}[psl_mode]
        psl2_d = nc.dram_tensor("psT2", [128, nT, 8, TILE], d2, kind="ExternalInput").ap()
    relb_d = nc.dram_tensor("relb", [CHUNK, C], f32, kind="ExternalInput").ap()
    nums_d = nc.dram_tensor("nums", [CHUNK, C], f32, kind="ExternalInput").ap()
    # w1: [p, j, m, i, c] = W_h1[m*128+c, (KS*j+i)*128+p]
    w1_d = nc.dram_tensor("w1", [128, KG, 2, KS, 128], fd, kind="ExternalInput").ap()
    # w2: [p, m, i, c] = W_h2[m*128+c, i*128+p]  (fp16)
    w2_d = nc.dram_tensor("w2", [128, 2, 2, 128], f16, kind="ExternalInput").ap()
    # wout: [p, i] = W_out[0, i*128+p]  (fp16)
    wout_d = nc.dram_tensor("wout", [128, 2, 1], f16, kind="ExternalInput").ap()
    # psl weights
    if fp8 and psl_mode == "hilo8":
        # k-subtile dim padded to step 16 (ISA: dual-fp8 LW step%16==0)
        wpa_d = nc.dram_tensor("wpsl_hi", [128, 4, 2, 16], f8, kind="ExternalInput").ap()
        wpb_d = nc.dram_tensor("wpsl_lo", [128, 4, 2, 16], f8, kind="ExternalInput").ap()
    elif fp8 and psl_mode == "e3":
        wpa_d = nc.dram_tensor("wpsl", [128, 8, 1, 1], f8e3, kind="ExternalInput").ap()
        wpb_d = None
    else:
        wpa_d = nc.dram_tensor("wpsl", [128, 8, 1, 1], f16, kind="ExternalInput").ap()
        wpb_d = None
    ones_d = nc.dram_tensor("ones", [97, 1], f16, kind="ExternalInput").ap()
    iota_d = nc.dram_tensor("iota", [128, SMAX], f16, kind="ExternalInput").ap()
    out_d = nc.dram_tensor("out", [1, SMAX], f32, kind="ExternalOutput").ap()

    with tile.TileContext(nc) as tc, ExitStack() as ctx:
        const = ctx.enter_context(tc.tile_pool(name="const", bufs=1))
        psTp = ctx.enter_context(tc.tile_pool(name="psT", bufs=6))
        psTp2 = ctx.enter_context(tc.tile_pool(name="psT2", bufs=6)) if fp8 else None
        silp = ctx.enter_context(tc.tile_pool(name="sil", bufs=4))
        rowp = ctx.enter_context(tc.tile_pool(name="row", bufs=6))
        pp_h1 = ctx.enter_context(tc.tile_pool(name="pph1", bufs=1, space=PSUM))
        pp_h2 = ctx.enter_context(tc.tile_pool(name="pph2", bufs=1, space=PSUM))
        pp_e = ctx.enter_context(tc.tile_pool(name="ppe", bufs=1, space=PSUM))
        pp_ec = ctx.enter_context(tc.tile_pool(name="ppec", bufs=2, space=PSUM))
        pp_seg = ctx.enter_context(tc.tile_pool(name="ppseg", bufs=1, space=PSUM))

        # ---- constants ----
        w1_sb = const.tile([128, KG, 2, KS, 128], fd, tag="w1")
        nc.sync.dma_start(w1_sb[:], w1_d[:])
        w2_sb = const.tile([128, 2, 2, 128], f16, tag="w2")
        nc.sync.dma_start(w2_sb[:], w2_d[:])
        wout_sb = const.tile([128, 2, 1], f16, tag="wout")
        nc.sync.dma_start(wout_sb[:], wout_d[:])
        if fp8 and psl_mode == "hilo8":
            wpa_sb = const.tile([128, 4, 2, 16], f8, tag="wpa")
            nc.sync.dma_start(wpa_sb[:], wpa_d[:])
            wpb_sb = const.tile([128, 4, 2, 16], f8, tag="wpb")
            nc.sync.dma_start(wpb_sb[:], wpb_d[:])
        else:
            wpa_sb = const.tile([128, 8, 1, 1], f8e3 if (fp8 and psl_mode == "e3") else f16, tag="wpa")
            nc.sync.dma_start(wpa_sb[:], wpa_d[:])
            wpb_sb = None
        iota_sb = const.tile([128, SMAX], f16, tag="iota")
        nc.sync.dma_start(iota_sb[:], iota_d[:])
        relb_sb = const.tile([CHUNK, C], f32, tag="relb")
        nc.sync.dma_start(relb_sb[:], relb_d[:])
        nums_sb = const.tile([CHUNK, C], f32, tag="nums")
        nc.sync.dma_start(nums_sb[:], nums_d[:])
        ones_sb = const.tile([97, 1], f16, tag="ones")
        nc.sync.dma_start(ones_sb[:], ones_d[:])

        # species energy per atom: cubic through W_comp[0, 0..3]
        # comp = (c1*n + c0) + n*n*(c3*n + c2)
        t_n2 = const.tile([CHUNK, C], f32, tag="t_n2")
        nc.vector.tensor_mul(t_n2[:], nums_sb[:], nums_sb[:])
        t_a = const.tile([CHUNK, C], f32, tag="t_a")
        nc.vector.tensor_scalar(t_a[:], nums_sb[:], c3, c2, ALU.mult, ALU.add)
        t_b = const.tile([CHUNK, C], f32, tag="t_b")
        nc.vector.tensor_mul(t_b[:], t_n2[:], t_a[:])
        t_c = const.tile([CHUNK, C], f32, tag="t_c")
        nc.vector.tensor_scalar(t_c[:], nums_sb[:], c1, c0, ALU.mult, ALU.add)
        comp_sb = const.tile([CHUNK, C], f32, tag="comp")
        nc.vector.tensor_add(comp_sb[:], t_b[:], t_c[:])

        seg_ps = pp_seg.tile([1, SMAX], f32, tag="seg")
        nc.vector.memset(seg_ps[:], 0.0)
        # e-partials bank: psl rows 0/32, psnn rows 64/96, with a +16
        # partition offset on odd tiles (same PSUM quadrant, so the
        # matmul dst stays valid) — in-bank double buffering that breaks
        # the psl(t+1) <- e_row-copy(t) WAR. Rows in between stay 0 from
        # this one-time clear; the K=113 ones-matmul sums the partials.
        e_ps = pp_e.tile([128, TILE], f32, tag="e")
        nc.vector.memset(e_ps[:], 0.0)


        ohp = ctx.enter_context(tc.tile_pool(name="ohp", bufs=8))

        def gen_ohs(tn):
            ohs = []
            for cc in range(4):
                ch = tn * 4 + cc
                if segw is not None:
                    W = segw[0]
                    oh = ohp.tile([128, W], f16, tag="oh", name=f"oh{tn}_{cc}")
                    nc.vector.tensor_scalar(
                        oh[:], iota_sb[:, 0:W], relb_sb[:, ch : ch + 1], None,
                        ALU.is_equal,
                    )
                else:
                    oh = ohp.tile([128, SMAX], f16, tag="oh", name=f"oh{tn}_{cc}")
                    nc.vector.tensor_scalar(
                        oh[:], iota_sb[:], relb_sb[:, ch : ch + 1], None,
                        ALU.is_equal,
                    )
                ohs.append(oh)
            return ohs

        def tail(tp, e_row_p, oh_p):
            # column-ize all 4 chunks into one PSUM tile, add species
            # energy on the DVE, then the 4 windowed seg matmuls
            ec4 = pp_ec.tile([128, 4], f32, tag="ec", name=f"ec4_{tp}")
            ecols = []
            for cc in range(4):
                ch = tp * 4 + cc
                nc.tensor.matmul(
                    ec4[:, cc : cc + 1],
                    e_row_p[0:97, cc * 128 : (cc + 1) * 128],
                    ones_sb[:],
                    start=True,
                    stop=True,
                )
                e_col = rowp.tile([128, 1], f16, tag="ecol", name=f"ecol{tp}_{cc}")
                nc.vector.tensor_add(e_col[:], ec4[:, cc : cc + 1], comp_sb[:, ch : ch + 1])
                ecols.append(e_col)
            for cc in range(4):
                ch = tp * 4 + cc
                if segw is not None:
                    W, w0s = segw
                    w0 = w0s[ch]
                    nc.tensor.matmul(
                        seg_ps[0:1, w0 : w0 + W], ecols[cc][:], oh_p[cc][:],
                        start=False, stop=(ch == C - 1), skip_group_check=True,
                    )
                else:
                    nc.tensor.matmul(
                        seg_ps[:], ecols[cc][:], oh_p[cc][:],
                        start=(ch == 0), stop=(ch == C - 1),
                    )

        # Software-pipelined main loop: tile t-1's colize/seg run between
        # tile t's h2 and psnn (filling the sil2 wait on the PE), with the
        # one-hot windows generated a tile ahead on the DVE.
        prev = None
        ohs_next = gen_ohs(0)
        for t in range(nT):
            # ---- contiguous loads: [128, 8, 512]
            big = psTp.tile([128, 8, TILE], fd, tag="psT", name=f"psT{t}")
            nc.sync.dma_start(big[:], psT_d[:, t])
            if fp8:
                d2 = {"hilo8": f8, "e3": f8e3, "fp16": f16}[psl_mode]
                big2 = psTp2.tile([128, 8, TILE], d2, tag="psT2", name=f"psT2_{t}")
                nc.sync.dma_start(big2[:], psl2_d[:, t])

            # ---- h1: KG k-groups x 2 M-chunks
            h1ps = [pp_h1.tile([128, TILE], f32, tag=f"h1m{m}", name=f"h1ps{t}_{m}") for m in range(2)]
            for j in range(KG):
                for m in range(2):
                    nc.tensor.matmul(
                        h1ps[m][:],
                        w1_sb[:, j, m],
                        big[:, KS * j : KS * (j + 1)],
                        start=(j == 0),
                        stop=(j == KG - 1),
                        perf_mode=DR,
                    )
            sil1 = silp.tile([128, 2, TILE], f16, tag="sil1")
            for m in range(2):
                nc.scalar.activation(sil1[:, m], h1ps[m][:], AF.Silu)

            # ---- psl: M=1 matmuls in column groups, partial sums landing
            # on e_ps rows (fills the PE while sil1 runs on ACT)
            if fp8 and psl_mode == "hilo8":
                for cn, (wp, mv) in enumerate(
                    [(wpa_sb, big), (wpa_sb, big2), (wpb_sb, big)]
                ):
                    for j in range(4):
                        nc.tensor.matmul(
                            e_ps[0:1, :],
                            wp[:, j, :, 0:1],
                            mv[:, 2 * j : 2 * j + 2],
                            start=(cn == 0 and j == 0),
                            stop=False,
                            perf_mode=DR,
                        )
            elif fp8:
                # psl from the scaled-e3m4/fp16 psT copy: fast M=1 matmuls
                # in column groups 0/1, partials on e_ps rows 0/32 (the
                # 1/(s*t) unscale rides in the ones vector rows 0:33)
                for k in range(8):
                    g = 32 * (k % 2)
                    nc.tensor.matmul(
                        e_ps[g : g + 1, :],
                        wpa_sb[:, k],
                        big2[:, k],
                        start=(k < 2),
                        stop=False,
                        tile_position=(0, g),
                    )
            else:
                for k in range(8):
                    g = 32 * (k % 4)
                    nc.tensor.matmul(
                        e_ps[g : g + 1, :],
                        wpa_sb[:, k],
                        big[:, k],
                        start=(k < 4),
                        stop=False,
                        tile_position=(0, g),
                    )

            # ---- h2 (fp16)
            h2ps = [pp_h2.tile([128, TILE], f32, tag=f"h2m{m}", name=f"h2ps{t}_{m}") for m in range(2)]
            for kj in range(2):
                for m in range(2):
                    nc.tensor.matmul(
                        h2ps[m][:],
                        w2_sb[:, m, kj],
                        sil1[:, kj],
                        start=(kj == 0),
                        stop=(kj == 1),
                    )
            sil2 = silp.tile([128, 2, TILE], f16, tag="sil2")
            for m in range(2):
                nc.scalar.activation(sil2[:, m], h2ps[m][:], AF.Silu)

            # ---- previous tile's tail fills the sil2 wait
            if prev is not None:
                tail(*prev)

            # ---- psnn (fp16): fresh rows 64/96 in fp8 modes (psl only
            # occupies rows 0/32); fp16 mode accumulates onto the psl rows
            for kj in range(2):
                if fp8:
                    g, st = 64 + 32 * kj, True
                else:
                    g, st = 32 * kj, False
                nc.tensor.matmul(
                    e_ps[g : g + 1, :],
                    wout_sb[:, kj],
                    sil2[:, kj],
                    start=st,
                    stop=(kj == 1),
                    tile_position=(0, g),
                )
            # partial rows 0/32/64/96 (zeros between) -> SBUF in one copy
            e_row = rowp.tile([97, TILE], f16, tag="erow")
            nc.vector.tensor_copy(e_row[:], e_ps[0:97, :])

            ohs_cur = ohs_next
            if t + 1 < nT:
                ohs_next = gen_ohs(t + 1)
            prev = (t, e_row, ohs_cur)

        tail(*prev)

        out_sb = rowp.tile([1, SMAX], f32, tag="outsb")
        nc.scalar.activation(out_sb[:], seg_ps[:], AF.Copy)
        nc.sync.dma_start(out_d[:], out_sb[:])

    _split_waits(nc, mybir)
    return nc


def _install_ntff_hook():
    """Register the axon NTFF profile hook (missing antenv.axon_hooks in
    this image) so run_bass_kernel_spmd(trace=True) can report exec_time_ns."""
    import sys
    import types

    try:
        import antenv.axon_hooks  # noqa: F401

        return
    except ImportError:
        pass
    from trn_agent_boot.trn_boot import _ntff_profile_via_ctypes

    hook = _ntff_profile_via_ctypes("/opt/axon/libaxon_pjrt.so")
    mod = types.ModuleType("antenv.axon_hooks")
    mod.get_axon_ntff_profile_hook = lambda: hook
    mod.set_axon_ntff_profile_hook = lambda h: None
    sys.modules["antenv.axon_hooks"] = mod
    import antenv

    antenv.axon_hooks = mod
    import concourse.bass_utils as bu

    bu.upload_artifacts = lambda tmpdir: tmpdir


def _to_psT(pss, Ta):
    """[Ta, 1024] -> [128, nT, 8, TILE] with [p,t,k,a] = pss[t*TILE+a, k*128+p]"""
    nT = Ta // TILE
    return np.ascontiguousarray(pss.reshape(nT, TILE, 8, 128).transpose(3, 0, 2, 1))


def _to_psT_i(pss, Ta):
    """[Ta, 1024] -> [128, nT, 4, TILE, 2] byte-interleaved k-pairs:
    [p, t, j, a, i] = pss[t*TILE+a, (2j+i)*128+p]"""
    nT = Ta // TILE
    return np.ascontiguousarray(
        pss.reshape(nT, TILE, 4, 2, 128).transpose(4, 0, 2, 1, 3)
    )


def kernel(ps, numbers, batch, W_comp, W_psl, W_h1, W_h2, W_out):
    global LAST_EXEC_NS, LAST_RESULTS
    from concourse.bass_utils import run_bass_kernel_spmd

    if TRACE:
        _install_ntff_hook()

    fp8 = MODE == "fp8mix"
    DT = F8 if fp8 else np.float16
    KG = 4 if fp8 else 8
    KS = 2 if fp8 else 1

    ps = np.asarray(ps)
    numbers = np.asarray(numbers)
    batch = np.asarray(batch)
    W_comp = np.asarray(W_comp, dtype=np.float32)
    W_psl = np.asarray(W_psl, dtype=np.float32)
    W_h1 = np.asarray(W_h1, dtype=np.float32)
    W_h2 = np.asarray(W_h2, dtype=np.float32)
    W_out = np.asarray(W_out, dtype=np.float32)

    counts = np.bincount(batch, minlength=N_STRUCT)
    cum = np.zeros(N_STRUCT + 1, dtype=np.int64)
    np.cumsum(counts, out=cum[1:])

    # equal-structure shard cuts (atoms balance to ~1-2% by CLT; keeps
    # per-core structure count fixed at N_STRUCT/N_CORES <= SMAX)
    s_cut = [i * N_STRUCT // N_CORES for i in range(N_CORES + 1)]

    shards = []
    for i in range(N_CORES):
        s_lo, s_hi = s_cut[i], s_cut[i + 1]
        a_lo, a_hi = int(cum[s_lo]), int(cum[s_hi])
        n_at, n_st = a_hi - a_lo, s_hi - s_lo
        assert n_st <= SMAX, f"shard {i}: {n_st} structs > {SMAX}"
        shards.append((s_lo, s_hi, a_lo, a_hi, n_at, n_st))

    Ta = max(s[4] for s in shards)
    Ta = (Ta + TILE - 1) // TILE * TILE
    nT = Ta // TILE
    C = Ta // CHUNK

    # replicated weights: w1[p, j, m, i, c] = W_h1[m*128+c, (KS*j+i)*128+p]
    w1 = np.ascontiguousarray(
        W_h1.T.reshape(KG, KS, 128, 2, 128).transpose(2, 0, 3, 1, 4)
    ).astype(DT)
    # w2[p, m, i, c] = W_h2[m*128+c, i*128+p]  (fp16)
    w2 = np.ascontiguousarray(
        W_h2.T.reshape(2, 128, 2, 128).transpose(1, 2, 0, 3)
    ).astype(np.float16)
    # wout[p, i] = W_out[0, i*128+p]  (fp16)
    wout = np.ascontiguousarray(
        W_out[0].reshape(2, 128).T.reshape(128, 2, 1)
    ).astype(np.float16)
    # psl weights
    if fp8 and PSL_MODE == "hilo8":
        wp32 = W_psl[0].astype(np.float32)
        wp_hi8 = wp32.astype(F8)
        wp_lo8 = (wp32 - wp_hi8.astype(np.float32)).astype(F8)
        def pack(w):
            out = np.zeros((128, 4, 2, 16), dtype=F8)
            out[..., 0] = np.asarray(w).reshape(4, 2, 128).transpose(2, 0, 1)
            return out

        wpsl_hi, wpsl_lo = pack(wp_hi8), pack(wp_lo8)
    elif fp8 and PSL_MODE == "e3":
        t_w = 15.5 / max(float(np.abs(W_psl).max()), 1e-30)
        _WPSL_T = [t_w]
        wpsl16 = np.asarray(
            np.clip(W_psl[0].astype(np.float32) * t_w, -15.5, 15.5)
            .reshape(8, 128).T.reshape(128, 8, 1, 1),
            dtype=ml_dtypes.float8_e3m4,
        ).copy()
    else:
        wpsl16 = np.ascontiguousarray(
            W_psl[0].reshape(8, 128).T.reshape(128, 8, 1, 1)
        ).astype(np.float16)
    iota = np.ascontiguousarray(
        np.tile(np.arange(SMAX, dtype=np.float16), (128, 1))
    )
    # exact cubic through the 4 species energies
    V = np.vander(np.arange(N_SPECIES, dtype=np.float64), 4, increasing=True)
    poly = np.linalg.solve(V, W_comp[0, :N_SPECIES].astype(np.float64))

    # per-shard relative batch ids (padding = -1)
    rb_all = []
    for s_lo, s_hi, a_lo, a_hi, n_at, n_st in shards:
        rb = np.full(Ta, -1.0, dtype=np.float32)
        rb[:n_at] = (batch[a_lo:a_hi] - s_lo).astype(np.float32)
        rb_all.append(rb)

    segw = None
    if SEG_WINDOWED:
        # shared per-chunk structure windows: batch is sorted so each
        # 128-atom chunk only spans a few structures; bake [w0, w0+W)
        m0 = np.full(C, np.inf)
        m1 = np.full(C, -np.inf)
        for rb in rb_all:
            r2 = rb.reshape(C, CHUNK)
            mask = r2 >= 0
            has = mask.any(axis=1)
            lo = np.where(has, np.where(mask, r2, np.inf).min(axis=1), np.inf)
            hi = np.where(has, np.where(mask, r2, -np.inf).max(axis=1), -np.inf)
            m0 = np.minimum(m0, lo)
            m1 = np.maximum(m1, hi)
        w0 = np.where(np.isfinite(m0), m0, 0.0)
        span = np.where(np.isfinite(m1), m1 - w0 + 1, 1.0)
        W = int(max(1, span.max()))
        W = min((W + 3) // 4 * 4, SMAX)
        w0 = np.clip(w0, 0, SMAX - W).astype(np.int64)
        segw = (W, tuple(int(x) for x in w0))

    key = (Ta, C, tuple(np.round(poly, 12)), MODE, PSL_MODE, segw)
    if key not in _BUILD_CACHE:
        _BUILD_CACHE.clear()
        _BUILD_CACHE[key] = _build(Ta, C, poly, MODE, PSL_MODE, segw)
    nc = _BUILD_CACHE[key]

    psq = np.asarray(ps, dtype=DT)  # quantize once, slice per shard
    ones = np.ones((97, 1), dtype=np.float16)
    if fp8 and PSL_MODE == "hilo8":
        ps2 = (ps.astype(np.float32) - psq.astype(np.float32)).astype(F8)
    elif fp8 and PSL_MODE == "e3":
        # scale ps and wpsl into e3m4's normal range (subnormals would
        # cost ~8% error on small values); the 1/(s*t) unscale rides in
        # the colize ones vector rows 0:33 (psl partial rows 0/32)
        s_ps = 15.5 / max(float(np.abs(ps).max()), 1e-30)
        ps2 = np.asarray(
            np.clip(ps.astype(np.float32) * s_ps, -15.5, 15.5),
            dtype=ml_dtypes.float8_e3m4,
        )
        ones[0:33] = np.float16(1.0 / (s_ps * _WPSL_T[0]))
    elif fp8:
        ps2 = ps.astype(np.float16)

    in_maps = []
    for si, (s_lo, s_hi, a_lo, a_hi, n_at, n_st) in enumerate(shards):
        pss = np.zeros((Ta, N_FEAT), dtype=DT)
        pss[:n_at] = psq[a_lo:a_hi]
        rb = rb_all[si]
        if segw is not None:
            rb = rb.reshape(C, CHUNK) - np.asarray(segw[1], dtype=np.float32)[:, None]
            rb = rb.reshape(Ta)
        nums = np.zeros(Ta, dtype=np.float32)
        nums[:n_at] = numbers[a_lo:a_hi].astype(np.float32)
        im = {
            "psT": _to_psT(pss, Ta),
            "relb": np.ascontiguousarray(rb.reshape(C, CHUNK).T),
            "nums": np.ascontiguousarray(nums.reshape(C, CHUNK).T),
            "w1": w1,
            "w2": w2,
            "wout": wout,
            "ones": ones,
            "iota": iota,
        }
        if fp8 and PSL_MODE == "hilo8":
            pss2 = np.zeros((Ta, N_FEAT), dtype=F8)
            pss2[:n_at] = ps2[a_lo:a_hi]
            im["psT2"] = _to_psT(pss2, Ta)
            im["wpsl_hi"] = wpsl_hi
            im["wpsl_lo"] = wpsl_lo
        elif fp8:
            pss2 = np.zeros((Ta, N_FEAT), dtype=ps2.dtype)
            pss2[:n_at] = ps2[a_lo:a_hi]
            im["psT2"] = _to_psT(pss2, Ta)
            im["wpsl"] = wpsl16
        else:
            im["wpsl"] = wpsl16
        in_maps.append(im)

    res = run_bass_kernel_spmd(nc, in_maps, list(range(N_CORES)), trace=TRACE)
    LAST_EXEC_NS = res.exec_time_ns
    LAST_RESULTS = res

    out = np.zeros((N_STRUCT, 1), dtype=np.float32)
    for i, (s_lo, s_hi, a_lo, a_hi, n_at, n_st) in enumerate(shards):
        vals = res.results[i]["out"][0, :n_st].astype(np.float32)
        empty = counts[s_lo:s_hi] == 0
        if empty.any():
            vals = np.where(empty, 0.0, vals)
        out[s_lo:s_hi, 0] = vals
    return out


# revision 32
# speedup vs baseline: 1.2453x; 1.0018x over previous
"""PowerSpectrumModel Trainium2 kernel (8 NeuronCores, SPMD).

Strategy (data-parallel over atoms, structures disjoint per shard):
 - Host: cut the atom axis at structure boundaries into 8 balanced shards;
   pre-TRANSPOSE ps to feature-major [128, nT, 8, TILE] (so every tile
   load is one contiguous stride per partition — no on-device transpose
   descriptors); fp32 PSUM accumulation on device; replicate the small
   weight matrices.
 - Precision (fp8mix mode): the big h1 GEMM runs fp8e4m3 DoubleRow (2x
   PE rate; w1-quant + ps-quant contribute only ~6e-3 rel err because
   errors decorrelate across atoms); h2/psnn stay fp16 (their weight
   quantization biases the per-atom energy coherently across a structure
   -> 3.6e-2 if fp8); psl gets hi+lo fp8 splits of both ps and W_psl
   (or an extra fp16 psT copy, PSL_MODE).
 - Device, per 512-atom tile:
     psT   <- contiguous loads [128, 8, 512]                      [DMA]
     h1    = W_h1 @ psT  (fp8 DoubleRow k-pairs / fp16 k-chunks)  [PE]
     psl   = W_psl row, M=1 matmuls -> e partial rows 0/32/64/96  [PE]
     sil1  = silu(h1)                                             [ACT]
     h2    = W_h2 @ sil1; sil2 = silu(h2)                         [PE/ACT]
     psnn  M=1 matmuls accumulated onto the psl PSUM rows         [PE]
     per 128-chunk: e column via K=97 ones matmul, + species
     energy, one-hot(struct) matmul accumulates into a [1,256]
     PSUM row holding this core's per-structure energies.         [PE/DVE]
 - Host: slice per-core structure ranges, concat -> [2000, 1].
"""

import numpy as np
import ml_dtypes

N_ATOMS = 200000
N_FEAT = 1024
N_SPECIES = 4
N_STRUCT = 2000
H1 = 256
H2 = 256
SCALE = 1.0
N_CORES = 8
TILE = 512
CHUNK = 128
SMAX = 256  # per-core structure capacity (PSUM row)

MODE = "fp8mix"  # "fp16" | "fp8mix"
PSL_MODE = "e3"  # "e3" | "hilo8" | "fp16"
SEG_WINDOWED = True  # bake per-chunk structure windows into the seg matmul

_BUILD_CACHE = {}
TRACE = False
LAST_EXEC_NS = None
LAST_RESULTS = None

F8 = ml_dtypes.float8_e4m3


def _split_waits(nc, mybir, maxw=1):
    """walrus on this build rejects >1 sync wait per instruction; move
    overflow waits onto preceding same-engine NoOps."""
    cnt = 0
    for f in nc.m.functions:
        for blk in f.blocks:
            if not hasattr(blk, "instructions"):
                continue
            out = []
            changed = False
            for inst in blk.instructions:
                si = getattr(inst, "sync_info", None)
                if si is not None and si.on_wait and len(si.on_wait) > maxw:
                    waits = list(si.on_wait)
                    keep = waits[-maxw:]
                    extra = waits[:-maxw]
                    while extra:
                        chunk, extra = extra[:maxw], extra[maxw:]
                        cnt += 1
                        out.append(
                            mybir.InstNoOp(
                                name=f"waitfix-{cnt}",
                                engine=inst.engine,
                                text_hint="waitfix",
                                bass_nofuse=True,
                                ins=[],
                                outs=[],
                                sync_info=mybir.SyncInfo(on_wait=chunk, on_update=[]),
                            )
                        )
                    si.on_wait = keep
                    changed = True
                out.append(inst)
            if changed:
                blk.instructions[:] = out
    return cnt


def _build(Ta, C, poly, mode, psl_mode, segw=None):
    import concourse.bass as bass
    import concourse.tile as tile
    import concourse.mybir as mybir
    from contextlib import ExitStack

    fp8 = mode == "fp8mix"
    f8 = mybir.dt.float8e4
    f16 = mybir.dt.float16
    f32 = mybir.dt.float32
    fd = f8 if fp8 else f16  # h1 GEMM dtype
    AF = mybir.ActivationFunctionType
    ALU = mybir.AluOpType
    DR = mybir.MatmulPerfMode.DoubleRow if fp8 else None
    PSUM = bass.MemorySpace.PSUM
    nT = Ta // TILE
    c0, c1, c2, c3 = (float(x) for x in poly)
    KG = 4 if fp8 else 8  # h1 k-groups (DoubleRow contracts pairs)
    KS = 2 if fp8 else 1  # k-chunks per h1 matmul

    nc = bass.Bass("TRN2", target_bir_lowering=False, debug=False)

    # pre-transposed ps: [p, t, k, a] = ps[t*512+a, k*128+p]
    psT_d = nc.dram_tensor("psT", [128, nT, 8, TILE], fd, kind="ExternalInput").ap()
    f8e3 = mybir.dt.float8e3
    psl2_d = None
    if fp8:
        d2 = {"hilo8": f8, "e3": f8e3, "fp16": f16}[psl_mode]
        psl2_d = nc.dram_tensor("psT2", [128, nT, 8, TILE], d2, kind="ExternalInput").ap()
    relb_d = nc.dram_tensor("relb", [CHUNK, C], f32, kind="ExternalInput").ap()
    nums_d = nc.dram_tensor("nums", [CHUNK, C], f32, kind="ExternalInput").ap()
    # w1: [p, j, m, i, c] = W_h1[m*128+c, (KS*j+i)*128+p]
    w1_d = nc.dram_tensor("w1", [128, KG, 2, KS, 128], fd, kind="ExternalInput").ap()
    # w2: [p, m, i, c] = W_h2[m*128+c, i*128+p]  (fp16)
    w2_d = nc.dram_tensor("w2", [128, 2, 2, 128], f16, kind="ExternalInput").ap()
    # wout: [p, i] = W_out[0, i*128+p]  (fp16)
    wout_d = nc.dram_tensor("wout", [128, 2, 1], f16, kind="ExternalInput").ap()
    # psl weights
    if fp8 and psl_mode == "hilo8":
        # k-subtile dim padded to step 16 (ISA: dual-fp8 LW step%16==0)
        wpa_d = nc.dram_tensor("wpsl_hi", [128, 4, 2, 16], f8, kind="ExternalInput").ap()
        wpb_d = nc.dram_tensor("wpsl_lo", [128, 4, 2, 16], f8, kind="ExternalInput").ap()
    elif fp8 and psl_mode == "e3":
        wpa_d = nc.dram_tensor("wpsl", [128, 8, 1, 1], f8e3, kind="ExternalInput").ap()
        wpb_d = None
    else:
        wpa_d = nc.dram_tensor("wpsl", [128, 8, 1, 1], f16, kind="ExternalInput").ap()
        wpb_d = None
    ones_d = nc.dram_tensor("ones", [97, 1], f16, kind="ExternalInput").ap()
    iota_d = nc.dram_tensor("iota", [128, SMAX], f16, kind="ExternalInput").ap()
    out_d = nc.dram_tensor("out", [1, SMAX], f32, kind="ExternalOutput").ap()

    with tile.TileContext(nc) as tc, ExitStack() as ctx:
        const = ctx.enter_context(tc.tile_pool(name="const", bufs=1))
        psTp = ctx.enter_context(tc.tile_pool(name="psT", bufs=8))
        psTp2 = ctx.enter_context(tc.tile_pool(name="psT2", bufs=6)) if fp8 else None
        silp = ctx.enter_context(tc.tile_pool(name="sil", bufs=6))
        rowp = ctx.enter_context(tc.tile_pool(name="row", bufs=6))
        pp_h1 = ctx.enter_context(tc.tile_pool(name="pph1", bufs=1, space=PSUM))
        pp_h2 = ctx.enter_context(tc.tile_pool(name="pph2", bufs=1, space=PSUM))
        pp_e = ctx.enter_context(tc.tile_pool(name="ppe", bufs=1, space=PSUM))
        pp_ec = ctx.enter_context(tc.tile_pool(name="ppec", bufs=2, space=PSUM))
        pp_seg = ctx.enter_context(tc.tile_pool(name="ppseg", bufs=1, space=PSUM))

        # ---- constants ----
        w1_sb = const.tile([128, KG, 2, KS, 128], fd, tag="w1")
        nc.sync.dma_start(w1_sb[:], w1_d[:])
        w2_sb = const.tile([128, 2, 2, 128], f16, tag="w2")
        nc.sync.dma_start(w2_sb[:], w2_d[:])
        wout_sb = const.tile([128, 2, 1], f16, tag="wout")
        nc.sync.dma_start(wout_sb[:], wout_d[:])
        if fp8 and psl_mode == "hilo8":
            wpa_sb = const.tile([128, 4, 2, 16], f8, tag="wpa")
            nc.sync.dma_start(wpa_sb[:], wpa_d[:])
            wpb_sb = const.tile([128, 4, 2, 16], f8, tag="wpb")
            nc.sync.dma_start(wpb_sb[:], wpb_d[:])
        else:
            wpa_sb = const.tile([128, 8, 1, 1], f8e3 if (fp8 and psl_mode == "e3") else f16, tag="wpa")
            nc.sync.dma_start(wpa_sb[:], wpa_d[:])
            wpb_sb = None
        iota_sb = const.tile([128, SMAX], f16, tag="iota")
        nc.sync.dma_start(iota_sb[:], iota_d[:])
        relb_sb = const.tile([CHUNK, C], f32, tag="relb")
        nc.sync.dma_start(relb_sb[:], relb_d[:])
        nums_sb = const.tile([CHUNK, C], f32, tag="nums")
        nc.sync.dma_start(nums_sb[:], nums_d[:])
        ones_sb = const.tile([97, 1], f16, tag="ones")
        nc.sync.dma_start(ones_sb[:], ones_d[:])

        # species energy per atom: cubic through W_comp[0, 0..3]
        # comp = (c1*n + c0) + n*n*(c3*n + c2)
        t_n2 = const.tile([CHUNK, C], f32, tag="t_n2")
        nc.vector.tensor_mul(t_n2[:], nums_sb[:], nums_sb[:])
        t_a = const.tile([CHUNK, C], f32, tag="t_a")
        nc.vector.tensor_scalar(t_a[:], nums_sb[:], c3, c2, ALU.mult, ALU.add)
        t_b = const.tile([CHUNK, C], f32, tag="t_b")
        nc.vector.tensor_mul(t_b[:], t_n2[:], t_a[:])
        t_c = const.tile([CHUNK, C], f32, tag="t_c")
        nc.vector.tensor_scalar(t_c[:], nums_sb[:], c1, c0, ALU.mult, ALU.add)
        comp_sb = const.tile([CHUNK, C], f32, tag="comp")
        nc.vector.tensor_add(comp_sb[:], t_b[:], t_c[:])

        seg_ps = pp_seg.tile([1, SMAX], f32, tag="seg")
        nc.vector.memset(seg_ps[:], 0.0)
        # e-partials bank: psl rows 0/32, psnn rows 64/96, with a +16
        # partition offset on odd tiles (same PSUM quadrant, so the
        # matmul dst stays valid) — in-bank double buffering that breaks
        # the psl(t+1) <- e_row-copy(t) WAR. Rows in between stay 0 from
        # this one-time clear; the K=113 ones-matmul sums the partials.
        e_ps = pp_e.tile([128, TILE], f32, tag="e")
        nc.vector.memset(e_ps[:], 0.0)


        ohp = ctx.enter_context(tc.tile_pool(name="ohp", bufs=8))

        def gen_ohs(tn):
            ohs = []
            for cc in range(4):
                ch = tn * 4 + cc
                if segw is not None:
                    W = segw[0]
                    oh = ohp.tile([128, W], f16, tag="oh", name=f"oh{tn}_{cc}")
                    nc.vector.tensor_scalar(
                        oh[:], iota_sb[:, 0:W], relb_sb[:, ch : ch + 1], None,
                        ALU.is_equal,
                    )
                else:
                    oh = ohp.tile([128, SMAX], f16, tag="oh", name=f"oh{tn}_{cc}")
                    nc.vector.tensor_scalar(
                        oh[:], iota_sb[:], relb_sb[:, ch : ch + 1], None,
                        ALU.is_equal,
                    )
                ohs.append(oh)
            return ohs

        def tail(tp, e_row_p, oh_p):
            # column-ize all 4 chunks into one PSUM tile, add species
            # energy on the DVE, then the 4 windowed seg matmuls
            ec4 = pp_ec.tile([128, 4], f32, tag="ec", name=f"ec4_{tp}")
            ecols = []
            for cc in range(4):
                ch = tp * 4 + cc
                nc.tensor.matmul(
                    ec4[:, cc : cc + 1],
                    e_row_p[0:97, cc * 128 : (cc + 1) * 128],
                    ones_sb[:],
                    start=True,
                    stop=True,
                )
                e_col = rowp.tile([128, 1], f16, tag="ecol", name=f"ecol{tp}_{cc}")
                nc.vector.tensor_add(e_col[:], ec4[:, cc : cc + 1], comp_sb[:, ch : ch + 1])
                ecols.append(e_col)
            for cc in range(4):
                ch = tp * 4 + cc
                if segw is not None:
                    W, w0s = segw
                    w0 = w0s[ch]
                    nc.tensor.matmul(
                        seg_ps[0:1, w0 : w0 + W], ecols[cc][:], oh_p[cc][:],
                        start=False, stop=(ch == C - 1), skip_group_check=True,
                    )
                else:
                    nc.tensor.matmul(
                        seg_ps[:], ecols[cc][:], oh_p[cc][:],
                        start=(ch == 0), stop=(ch == C - 1),
                    )

        # Software-pipelined main loop: tile t-1's colize/seg run between
        # tile t's h2 and psnn (filling the sil2 wait on the PE), with the
        # one-hot windows generated a tile ahead on the DVE.
        prev = None
        ohs_next = gen_ohs(0)
        for t in range(nT):
            # ---- contiguous loads: [128, 8, 512]
            big = psTp.tile([128, 8, TILE], fd, tag="psT", name=f"psT{t}")
            nc.sync.dma_start(big[:], psT_d[:, t])
            if fp8:
                d2 = {"hilo8": f8, "e3": f8e3, "fp16": f16}[psl_mode]
                big2 = psTp2.tile([128, 8, TILE], d2, tag="psT2", name=f"psT2_{t}")
                nc.sync.dma_start(big2[:], psl2_d[:, t])

            # ---- h1: KG k-groups x 2 M-chunks
            # m-outer so h1ps[0] completes (and sil1[0] starts) ~1us
            # earlier, shrinking the h2 wait on sil1
            h1ps = [pp_h1.tile([128, TILE], f32, tag=f"h1m{m}", name=f"h1ps{t}_{m}") for m in range(2)]
            sil1 = silp.tile([128, 2, TILE], f16, tag="sil1")
            for m in range(2):
                for j in range(KG):
                    nc.tensor.matmul(
                        h1ps[m][:],
                        w1_sb[:, j, m],
                        big[:, KS * j : KS * (j + 1)],
                        start=(j == 0),
                        stop=(j == KG - 1),
                        perf_mode=DR,
                    )
                nc.scalar.activation(sil1[:, m], h1ps[m][:], AF.Silu)

            # ---- psl: M=1 matmuls in column groups, partial sums landing
            # on e_ps rows (fills the PE while sil1 runs on ACT)
            if fp8 and psl_mode == "hilo8":
                for cn, (wp, mv) in enumerate(
                    [(wpa_sb, big), (wpa_sb, big2), (wpb_sb, big)]
                ):
                    for j in range(4):
                        nc.tensor.matmul(
                            e_ps[0:1, :],
                            wp[:, j, :, 0:1],
                            mv[:, 2 * j : 2 * j + 2],
                            start=(cn == 0 and j == 0),
                            stop=False,
                            perf_mode=DR,
                        )
            elif fp8:
                # psl from the scaled-e3m4/fp16 psT copy: fast M=1 matmuls
                # in column groups 0/1, partials on e_ps rows 0/32 (the
                # 1/(s*t) unscale rides in the ones vector rows 0:33)
                for k in range(8):
                    g = 32 * (k % 2)
                    nc.tensor.matmul(
                        e_ps[g : g + 1, :],
                        wpa_sb[:, k],
                        big2[:, k],
                        start=(k < 2),
                        stop=False,
                        tile_position=(0, g),
                    )
            else:
                for k in range(8):
                    g = 32 * (k % 4)
                    nc.tensor.matmul(
                        e_ps[g : g + 1, :],
                        wpa_sb[:, k],
                        big[:, k],
                        start=(k < 4),
                        stop=False,
                        tile_position=(0, g),
                    )

            # ---- h2 (fp16)
            h2ps = [pp_h2.tile([128, TILE], f32, tag=f"h2m{m}", name=f"h2ps{t}_{m}") for m in range(2)]
            for kj in range(2):
                for m in range(2):
                    nc.tensor.matmul(
                        h2ps[m][:],
                        w2_sb[:, m, kj],
                        sil1[:, kj],
                        start=(kj == 0),
                        stop=(kj == 1),
                    )
            sil2 = silp.tile([128, 2, TILE], f16, tag="sil2")
            for m in range(2):
                nc.scalar.activation(sil2[:, m], h2ps[m][:], AF.Silu)

            # ---- previous tile's tail fills the sil2 wait
            if prev is not None:
                tail(*prev)

            # ---- psnn (fp16): fresh rows 64/96 in fp8 modes (psl only
            # occupies rows 0/32); fp16 mode accumulates onto the psl rows
            for kj in range(2):
                if fp8:
                    g, st = 64 + 32 * kj, True
                else:
                    g, st = 32 * kj, False
                nc.tensor.matmul(
                    e_ps[g : g + 1, :],
                    wout_sb[:, kj],
                    sil2[:, kj],
                    start=st,
                    stop=(kj == 1),
                    tile_position=(0, g),
                )
            # partial rows 0/32/64/96 (zeros between) -> SBUF in one copy
            e_row = rowp.tile([97, TILE], f16, tag="erow")
            nc.vector.tensor_copy(e_row[:], e_ps[0:97, :])

            ohs_cur = ohs_next
            if t + 1 < nT:
                ohs_next = gen_ohs(t + 1)
            prev = (t, e_row, ohs_cur)

        tail(*prev)

        out_sb = rowp.tile([1, SMAX], f32, tag="outsb")
        nc.scalar.activation(out_sb[:], seg_ps[:], AF.Copy)
        nc.sync.dma_start(out_d[:], out_sb[:])

    _split_waits(nc, mybir)
    return nc


def _install_ntff_hook():
    """Register the axon NTFF profile hook (missing antenv.axon_hooks in
    this image) so run_bass_kernel_spmd(trace=True) can report exec_time_ns."""
    import sys
    import types

    try:
        import antenv.axon_hooks  # noqa: F401

        return
    except ImportError:
        pass
    from trn_agent_boot.trn_boot import _ntff_profile_via_ctypes

    hook = _ntff_profile_via_ctypes("/opt/axon/libaxon_pjrt.so")
    mod = types.ModuleType("antenv.axon_hooks")
    mod.get_axon_ntff_profile_hook = lambda: hook
    mod.set_axon_ntff_profile_hook = lambda h: None
    sys.modules["antenv.axon_hooks"] = mod
    import antenv

    antenv.axon_hooks = mod
    import concourse.bass_utils as bu

    bu.upload_artifacts = lambda tmpdir: tmpdir


def _to_psT(pss, Ta):
    """[Ta, 1024] -> [128, nT, 8, TILE] with [p,t,k,a] = pss[t*TILE+a, k*128+p]"""
    nT = Ta // TILE
    return np.ascontiguousarray(pss.reshape(nT, TILE, 8, 128).transpose(3, 0, 2, 1))


def _to_psT_i(pss, Ta):
    """[Ta, 1024] -> [128, nT, 4, TILE, 2] byte-interleaved k-pairs:
    [p, t, j, a, i] = pss[t*TILE+a, (2j+i)*128+p]"""
    nT = Ta // TILE
    return np.ascontiguousarray(
        pss.reshape(nT, TILE, 4, 2, 128).transpose(4, 0, 2, 1, 3)
    )


def kernel(ps, numbers, batch, W_comp, W_psl, W_h1, W_h2, W_out):
    global LAST_EXEC_NS, LAST_RESULTS
    from concourse.bass_utils import run_bass_kernel_spmd

    if TRACE:
        _install_ntff_hook()

    fp8 = MODE == "fp8mix"
    DT = F8 if fp8 else np.float16
    KG = 4 if fp8 else 8
    KS = 2 if fp8 else 1

    ps = np.asarray(ps)
    numbers = np.asarray(numbers)
    batch = np.asarray(batch)
    W_comp = np.asarray(W_comp, dtype=np.float32)
    W_psl = np.asarray(W_psl, dtype=np.float32)
    W_h1 = np.asarray(W_h1, dtype=np.float32)
    W_h2 = np.asarray(W_h2, dtype=np.float32)
    W_out = np.asarray(W_out, dtype=np.float32)

    counts = np.bincount(batch, minlength=N_STRUCT)
    cum = np.zeros(N_STRUCT + 1, dtype=np.int64)
    np.cumsum(counts, out=cum[1:])

    # equal-structure shard cuts (atoms balance to ~1-2% by CLT; keeps
    # per-core structure count fixed at N_STRUCT/N_CORES <= SMAX)
    s_cut = [i * N_STRUCT // N_CORES for i in range(N_CORES + 1)]

    shards = []
    for i in range(N_CORES):
        s_lo, s_hi = s_cut[i], s_cut[i + 1]
        a_lo, a_hi = int(cum[s_lo]), int(cum[s_hi])
        n_at, n_st = a_hi - a_lo, s_hi - s_lo
        assert n_st <= SMAX, f"shard {i}: {n_st} structs > {SMAX}"
        shards.append((s_lo, s_hi, a_lo, a_hi, n_at, n_st))

    Ta = max(s[4] for s in shards)
    Ta = (Ta + TILE - 1) // TILE * TILE
    nT = Ta // TILE
    C = Ta // CHUNK

    # replicated weights: w1[p, j, m, i, c] = W_h1[m*128+c, (KS*j+i)*128+p]
    w1 = np.ascontiguousarray(
        W_h1.T.reshape(KG, KS, 128, 2, 128).transpose(2, 0, 3, 1, 4)
    ).astype(DT)
    # w2[p, m, i, c] = W_h2[m*128+c, i*128+p]  (fp16)
    w2 = np.ascontiguousarray(
        W_h2.T.reshape(2, 128, 2, 128).transpose(1, 2, 0, 3)
    ).astype(np.float16)
    # wout[p, i] = W_out[0, i*128+p]  (fp16)
    wout = np.ascontiguousarray(
        W_out[0].reshape(2, 128).T.reshape(128, 2, 1)
    ).astype(np.float16)
    # psl weights
    if fp8 and PSL_MODE == "hilo8":
        wp32 = W_psl[0].astype(np.float32)
        wp_hi8 = wp32.astype(F8)
        wp_lo8 = (wp32 - wp_hi8.astype(np.float32)).astype(F8)
        def pack(w):
            out = np.zeros((128, 4, 2, 16), dtype=F8)
            out[..., 0] = np.asarray(w).reshape(4, 2, 128).transpose(2, 0, 1)
            return out

        wpsl_hi, wpsl_lo = pack(wp_hi8), pack(wp_lo8)
    elif fp8 and PSL_MODE == "e3":
        t_w = 15.5 / max(float(np.abs(W_psl).max()), 1e-30)
        _WPSL_T = [t_w]
        wpsl16 = np.asarray(
            np.clip(W_psl[0].astype(np.float32) * t_w, -15.5, 15.5)
            .reshape(8, 128).T.reshape(128, 8, 1, 1),
            dtype=ml_dtypes.float8_e3m4,
        ).copy()
    else:
        wpsl16 = np.ascontiguousarray(
            W_psl[0].reshape(8, 128).T.reshape(128, 8, 1, 1)
        ).astype(np.float16)
    iota = np.ascontiguousarray(
        np.tile(np.arange(SMAX, dtype=np.float16), (128, 1))
    )
    # exact cubic through the 4 species energies
    V = np.vander(np.arange(N_SPECIES, dtype=np.float64), 4, increasing=True)
    poly = np.linalg.solve(V, W_comp[0, :N_SPECIES].astype(np.float64))

    # per-shard relative batch ids (padding = -1)
    rb_all = []
    for s_lo, s_hi, a_lo, a_hi, n_at, n_st in shards:
        rb = np.full(Ta, -1.0, dtype=np.float32)
        rb[:n_at] = (batch[a_lo:a_hi] - s_lo).astype(np.float32)
        rb_all.append(rb)

    segw = None
    if SEG_WINDOWED:
        # shared per-chunk structure windows: batch is sorted so each
        # 128-atom chunk only spans a few structures; bake [w0, w0+W)
        m0 = np.full(C, np.inf)
        m1 = np.full(C, -np.inf)
        for rb in rb_all:
            r2 = rb.reshape(C, CHUNK)
            mask = r2 >= 0
            has = mask.any(axis=1)
            lo = np.where(has, np.where(mask, r2, np.inf).min(axis=1), np.inf)
            hi = np.where(has, np.where(mask, r2, -np.inf).max(axis=1), -np.inf)
            m0 = np.minimum(m0, lo)
            m1 = np.maximum(m1, hi)
        w0 = np.where(np.isfinite(m0), m0, 0.0)
        span = np.where(np.isfinite(m1), m1 - w0 + 1, 1.0)
        W = int(max(1, span.max()))
        W = min((W + 3) // 4 * 4, SMAX)
        w0 = np.clip(w0, 0, SMAX - W).astype(np.int64)
        segw = (W, tuple(int(x) for x in w0))

    key = (Ta, C, tuple(np.round(poly, 12)), MODE, PSL_MODE, segw)
    if key not in _BUILD_CACHE:
        _BUILD_CACHE.clear()
        _BUILD_CACHE[key] = _build(Ta, C, poly, MODE, PSL_MODE, segw)
    nc = _BUILD_CACHE[key]

    psq = np.asarray(ps, dtype=DT)  # quantize once, slice per shard
    ones = np.ones((97, 1), dtype=np.float16)
    if fp8 and PSL_MODE == "hilo8":
        ps2 = (ps.astype(np.float32) - psq.astype(np.float32)).astype(F8)
    elif fp8 and PSL_MODE == "e3":
        # scale ps and wpsl into e3m4's normal range (subnormals would
        # cost ~8% error on small values); the 1/(s*t) unscale rides in
        # the colize ones vector rows 0:33 (psl partial rows 0/32)
        s_ps = 15.5 / max(float(np.abs(ps).max()), 1e-30)
        ps2 = np.asarray(
            np.clip(ps.astype(np.float32) * s_ps, -15.5, 15.5),
            dtype=ml_dtypes.float8_e3m4,
        )
        ones[0:33] = np.float16(1.0 / (s_ps * _WPSL_T[0]))
    elif fp8:
        ps2 = ps.astype(np.float16)

    in_maps = []
    for si, (s_lo, s_hi, a_lo, a_hi, n_at, n_st) in enumerate(shards):
        pss = np.zeros((Ta, N_FEAT), dtype=DT)
        pss[:n_at] = psq[a_lo:a_hi]
        rb = rb_all[si]
        if segw is not None:
            rb = rb.reshape(C, CHUNK) - np.asarray(segw[1], dtype=np.float32)[:, None]
            rb = rb.reshape(Ta)
        nums = np.zeros(Ta, dtype=np.float32)
        nums[:n_at] = numbers[a_lo:a_hi].astype(np.float32)
        im = {
            "psT": _to_psT(pss, Ta),
            "relb": np.ascontiguousarray(rb.reshape(C, CHUNK).T),
            "nums": np.ascontiguousarray(nums.reshape(C, CHUNK).T),
            "w1": w1,
            "w2": w2,
            "wout": wout,
            "ones": ones,
            "iota": iota,
        }
        if fp8 and PSL_MODE == "hilo8":
            pss2 = np.zeros((Ta, N_FEAT), dtype=F8)
            pss2[:n_at] = ps2[a_lo:a_hi]
            im["psT2"] = _to_psT(pss2, Ta)
            im["wpsl_hi"] = wpsl_hi
            im["wpsl_lo"] = wpsl_lo
        elif fp8:
            pss2 = np.zeros((Ta, N_FEAT), dtype=ps2.dtype)
            pss2[:n_at] = ps2[a_lo:a_hi]
            im["psT2"] = _to_psT(pss2, Ta)
            im["wpsl"] = wpsl16
        else:
            im["wpsl"] = wpsl16
        in_maps.append(im)

    res = run_bass_kernel_spmd(nc, in_maps, list(range(N_CORES)), trace=TRACE)
    LAST_EXEC_NS = res.exec_time_ns
    LAST_RESULTS = res

    out = np.zeros((N_STRUCT, 1), dtype=np.float32)
    for i, (s_lo, s_hi, a_lo, a_hi, n_at, n_st) in enumerate(shards):
        vals = res.results[i]["out"][0, :n_st].astype(np.float32)
        empty = counts[s_lo:s_hi] == 0
        if empty.any():
            vals = np.where(empty, 0.0, vals)
        out[s_lo:s_hi, 0] = vals
    return out


# revision 34
# speedup vs baseline: 1.3362x; 1.0730x over previous
"""PowerSpectrumModel Trainium2 kernel (8 NeuronCores, SPMD).

Strategy (data-parallel over atoms, structures disjoint per shard):
 - Host: cut the atom axis at structure boundaries into 8 balanced shards;
   pre-TRANSPOSE ps to feature-major [128, nT, 8, TILE] (so every tile
   load is one contiguous stride per partition — no on-device transpose
   descriptors); fp32 PSUM accumulation on device; replicate the small
   weight matrices.
 - Precision (fp8mix mode): the big h1 GEMM runs fp8e4m3 DoubleRow (2x
   PE rate; w1-quant + ps-quant contribute only ~6e-3 rel err because
   errors decorrelate across atoms); h2/psnn stay fp16 (their weight
   quantization biases the per-atom energy coherently across a structure
   -> 3.6e-2 if fp8); psl gets hi+lo fp8 splits of both ps and W_psl
   (or an extra fp16 psT copy, PSL_MODE).
 - Device, per 512-atom tile:
     psT   <- contiguous loads [128, 8, 512]                      [DMA]
     h1    = W_h1 @ psT  (fp8 DoubleRow k-pairs / fp16 k-chunks)  [PE]
     psl   = W_psl row, M=1 matmuls -> e partial rows 0/32/64/96  [PE]
     sil1  = silu(h1)                                             [ACT]
     h2    = W_h2 @ sil1; sil2 = silu(h2)                         [PE/ACT]
     psnn  M=1 matmuls accumulated onto the psl PSUM rows         [PE]
     per 128-chunk: e column via K=97 ones matmul, + species
     energy, one-hot(struct) matmul accumulates into a [1,256]
     PSUM row holding this core's per-structure energies.         [PE/DVE]
 - Host: slice per-core structure ranges, concat -> [2000, 1].
"""

import numpy as np
import ml_dtypes

N_ATOMS = 200000
N_FEAT = 1024
N_SPECIES = 4
N_STRUCT = 2000
H1 = 256
H2 = 256
SCALE = 1.0
N_CORES = 8
TILE = 512
CHUNK = 128
SMAX = 256  # per-core structure capacity (PSUM row)

MODE = "fp8mix"  # "fp16" | "fp8mix"
PSL_MODE = "e3"  # "e3" | "hilo8" | "fp16"
SEG_WINDOWED = True  # bake per-chunk structure windows into the seg matmul

_BUILD_CACHE = {}
TRACE = False
LAST_EXEC_NS = None
LAST_RESULTS = None

F8 = ml_dtypes.float8_e4m3


def _split_waits(nc, mybir, maxw=1):
    """walrus on this build rejects >1 sync wait per instruction; move
    overflow waits onto preceding same-engine NoOps."""
    cnt = 0
    for f in nc.m.functions:
        for blk in f.blocks:
            if not hasattr(blk, "instructions"):
                continue
            out = []
            changed = False
            for inst in blk.instructions:
                si = getattr(inst, "sync_info", None)
                if si is not None and si.on_wait and len(si.on_wait) > maxw:
                    waits = list(si.on_wait)
                    keep = waits[-maxw:]
                    extra = waits[:-maxw]
                    while extra:
                        chunk, extra = extra[:maxw], extra[maxw:]
                        cnt += 1
                        out.append(
                            mybir.InstNoOp(
                                name=f"waitfix-{cnt}",
                                engine=inst.engine,
                                text_hint="waitfix",
                                bass_nofuse=True,
                                ins=[],
                                outs=[],
                                sync_info=mybir.SyncInfo(on_wait=chunk, on_update=[]),
                            )
                        )
                    si.on_wait = keep
                    changed = True
                out.append(inst)
            if changed:
                blk.instructions[:] = out
    return cnt


def _build(Ta, C, poly, mode, psl_mode, segw=None):
    import concourse.bass as bass
    import concourse.tile as tile
    import concourse.mybir as mybir
    from contextlib import ExitStack

    fp8 = mode == "fp8mix"
    f8 = mybir.dt.float8e4
    f16 = mybir.dt.float16
    f32 = mybir.dt.float32
    fd = f8 if fp8 else f16  # h1 GEMM dtype
    AF = mybir.ActivationFunctionType
    ALU = mybir.AluOpType
    DR = mybir.MatmulPerfMode.DoubleRow if fp8 else None
    PSUM = bass.MemorySpace.PSUM
    nT = Ta // TILE
    c0, c1, c2, c3 = (float(x) for x in poly)
    KG = 4 if fp8 else 8  # h1 k-groups (DoubleRow contracts pairs)
    KS = 2 if fp8 else 1  # k-chunks per h1 matmul

    nc = bass.Bass("TRN2", target_bir_lowering=False, debug=False)

    # pre-transposed ps: [p, t, k, a] = ps[t*512+a, k*128+p]
    psT_d = nc.dram_tensor("psT", [128, nT, 8, TILE], fd, kind="ExternalInput").ap()
    f8e3 = mybir.dt.float8e3
    psl2_d = None
    if fp8:
        d2 = {"hilo8": f8, "e3": f8e3, "fp16": f16}[psl_mode]
        psl2_d = nc.dram_tensor("psT2", [128, nT, 8, TILE], d2, kind="ExternalInput").ap()
    relb_d = nc.dram_tensor("relb", [CHUNK, C], f32, kind="ExternalInput").ap()
    nums_d = nc.dram_tensor("nums", [CHUNK, C], f32, kind="ExternalInput").ap()
    # w1: [p, j, m, i, c] = W_h1[m*128+c, (KS*j+i)*128+p]
    w1_d = nc.dram_tensor("w1", [128, KG, 2, KS, 128], fd, kind="ExternalInput").ap()
    # w2: [p, m, i, c] = W_h2[m*128+c, i*128+p]  (fp16)
    w2_d = nc.dram_tensor("w2", [128, 2, 2, 128], f16, kind="ExternalInput").ap()
    # wout: [p, i] = W_out[0, i*128+p]  (fp16)
    wout_d = nc.dram_tensor("wout", [128, 2, 1], f16, kind="ExternalInput").ap()
    # psl weights
    if fp8 and psl_mode == "hilo8":
        # k-subtile dim padded to step 16 (ISA: dual-fp8 LW step%16==0)
        wpa_d = nc.dram_tensor("wpsl_hi", [128, 4, 2, 16], f8, kind="ExternalInput").ap()
        wpb_d = nc.dram_tensor("wpsl_lo", [128, 4, 2, 16], f8, kind="ExternalInput").ap()
    elif fp8 and psl_mode == "e3":
        wpa_d = nc.dram_tensor("wpsl", [128, 8, 1, 1], f8e3, kind="ExternalInput").ap()
        wpb_d = None
    else:
        wpa_d = nc.dram_tensor("wpsl", [128, 8, 1, 1], f16, kind="ExternalInput").ap()
        wpb_d = None
    ones_d = nc.dram_tensor("ones", [97, 1], f16, kind="ExternalInput").ap()
    iota_d = nc.dram_tensor("iota", [128, SMAX], f16, kind="ExternalInput").ap()
    out_d = nc.dram_tensor("out", [1, SMAX], f32, kind="ExternalOutput").ap()

    with tile.TileContext(nc) as tc, ExitStack() as ctx:
        const = ctx.enter_context(tc.tile_pool(name="const", bufs=1))
        psTp = ctx.enter_context(tc.tile_pool(name="psT", bufs=8))
        psTp2 = ctx.enter_context(tc.tile_pool(name="psT2", bufs=6)) if fp8 else None
        silp = ctx.enter_context(tc.tile_pool(name="sil", bufs=6))
        rowp = ctx.enter_context(tc.tile_pool(name="row", bufs=6))
        pp_h1 = ctx.enter_context(tc.tile_pool(name="pph1", bufs=1, space=PSUM))
        pp_h2 = ctx.enter_context(tc.tile_pool(name="pph2", bufs=1, space=PSUM))
        pp_e = ctx.enter_context(tc.tile_pool(name="ppe", bufs=1, space=PSUM))
        pp_ec = ctx.enter_context(tc.tile_pool(name="ppec", bufs=1, space=PSUM))
        pp_seg = ctx.enter_context(tc.tile_pool(name="ppseg", bufs=1, space=PSUM))

        # ---- constants ----
        w1_sb = const.tile([128, KG, 2, KS, 128], fd, tag="w1")
        nc.sync.dma_start(w1_sb[:], w1_d[:])
        w2_sb = const.tile([128, 2, 2, 128], f16, tag="w2")
        nc.sync.dma_start(w2_sb[:], w2_d[:])
        wout_sb = const.tile([128, 2, 1], f16, tag="wout")
        nc.sync.dma_start(wout_sb[:], wout_d[:])
        if fp8 and psl_mode == "hilo8":
            wpa_sb = const.tile([128, 4, 2, 16], f8, tag="wpa")
            nc.sync.dma_start(wpa_sb[:], wpa_d[:])
            wpb_sb = const.tile([128, 4, 2, 16], f8, tag="wpb")
            nc.sync.dma_start(wpb_sb[:], wpb_d[:])
        else:
            wpa_sb = const.tile([128, 8, 1, 1], f8e3 if (fp8 and psl_mode == "e3") else f16, tag="wpa")
            nc.sync.dma_start(wpa_sb[:], wpa_d[:])
            wpb_sb = None
        iota_sb = const.tile([128, SMAX], f16, tag="iota")
        nc.sync.dma_start(iota_sb[:], iota_d[:])
        relb_sb = const.tile([CHUNK, C], f32, tag="relb")
        nc.sync.dma_start(relb_sb[:], relb_d[:])
        nums_sb = const.tile([CHUNK, C], f32, tag="nums")
        nc.sync.dma_start(nums_sb[:], nums_d[:])
        ones_sb = const.tile([97, 1], f16, tag="ones")
        nc.sync.dma_start(ones_sb[:], ones_d[:])

        # species energy per atom: cubic through W_comp[0, 0..3]
        # comp = (c1*n + c0) + n*n*(c3*n + c2)
        t_n2 = const.tile([CHUNK, C], f32, tag="t_n2")
        nc.vector.tensor_mul(t_n2[:], nums_sb[:], nums_sb[:])
        t_a = const.tile([CHUNK, C], f32, tag="t_a")
        nc.vector.tensor_scalar(t_a[:], nums_sb[:], c3, c2, ALU.mult, ALU.add)
        t_b = const.tile([CHUNK, C], f32, tag="t_b")
        nc.vector.tensor_mul(t_b[:], t_n2[:], t_a[:])
        t_c = const.tile([CHUNK, C], f32, tag="t_c")
        nc.vector.tensor_scalar(t_c[:], nums_sb[:], c1, c0, ALU.mult, ALU.add)
        comp_sb = const.tile([CHUNK, C], f32, tag="comp")
        nc.vector.tensor_add(comp_sb[:], t_b[:], t_c[:])

        seg_ps = pp_seg.tile([1, SMAX], f32, tag="seg")
        nc.vector.memset(seg_ps[:], 0.0)
        seg_ps2 = None
        if segw is not None:
            # second accumulator so consecutive seg matmuls alternate
            # PSUM banks instead of read-modify-writing the same one
            seg_ps2 = pp_seg.tile([1, SMAX], f32, tag="seg2")
            nc.vector.memset(seg_ps2[:], 0.0)
        # e-partials bank: psl rows 0/32, psnn rows 64/96, with a +16
        # partition offset on odd tiles (same PSUM quadrant, so the
        # matmul dst stays valid) — in-bank double buffering that breaks
        # the psl(t+1) <- e_row-copy(t) WAR. Rows in between stay 0 from
        # this one-time clear; the K=113 ones-matmul sums the partials.
        e_ps = pp_e.tile([128, TILE], f32, tag="e")
        nc.vector.memset(e_ps[:], 0.0)


        ohp = ctx.enter_context(tc.tile_pool(name="ohp", bufs=8))

        def gen_ohs(tn):
            ohs = []
            for cc in range(4):
                ch = tn * 4 + cc
                if segw is not None:
                    W = segw[0]
                    oh = ohp.tile([128, W], f16, tag="oh", name=f"oh{tn}_{cc}")
                    nc.vector.tensor_scalar(
                        oh[:], iota_sb[:, 0:W], relb_sb[:, ch : ch + 1], None,
                        ALU.is_equal,
                    )
                else:
                    oh = ohp.tile([128, SMAX], f16, tag="oh", name=f"oh{tn}_{cc}")
                    nc.vector.tensor_scalar(
                        oh[:], iota_sb[:], relb_sb[:, ch : ch + 1], None,
                        ALU.is_equal,
                    )
                ohs.append(oh)
            return ohs

        def tail(tp, e_row_p, oh_p):
            # column-ize all 4 chunks into one PSUM tile, add species
            # energy on the DVE, then the 4 windowed seg matmuls
            ec4 = pp_ec.tile([128, 4], f32, tag="ec", name=f"ec4_{tp}")
            ecols = []
            for cc in range(4):
                ch = tp * 4 + cc
                nc.tensor.matmul(
                    ec4[:, cc : cc + 1],
                    e_row_p[0:97, cc * 128 : (cc + 1) * 128],
                    ones_sb[:],
                    start=True,
                    stop=True,
                )
                e_col = rowp.tile([128, 1], f16, tag="ecol", name=f"ecol{tp}_{cc}")
                nc.vector.tensor_add(e_col[:], ec4[:, cc : cc + 1], comp_sb[:, ch : ch + 1])
                ecols.append(e_col)
            for cc in range(4):
                ch = tp * 4 + cc
                if segw is not None:
                    W, w0s = segw
                    w0 = w0s[ch]
                    acc = seg_ps if cc % 2 == 0 else seg_ps2
                    nc.tensor.matmul(
                        acc[0:1, w0 : w0 + W], ecols[cc][:], oh_p[cc][:],
                        start=False, stop=(ch >= C - 2), skip_group_check=True,
                    )
                else:
                    nc.tensor.matmul(
                        seg_ps[:], ecols[cc][:], oh_p[cc][:],
                        start=(ch == 0), stop=(ch == C - 1),
                    )

        # Software-pipelined main loop: tile t-1's colize/seg run between
        # tile t's h2 and psnn (filling the sil2 wait on the PE), with the
        # one-hot windows generated a tile ahead on the DVE.
        prev = None
        ohs_next = gen_ohs(0)
        for t in range(nT):
            # ---- contiguous loads: [128, 8, 512]
            big = psTp.tile([128, 8, TILE], fd, tag="psT", name=f"psT{t}")
            nc.sync.dma_start(big[:], psT_d[:, t])
            if fp8:
                d2 = {"hilo8": f8, "e3": f8e3, "fp16": f16}[psl_mode]
                big2 = psTp2.tile([128, 8, TILE], d2, tag="psT2", name=f"psT2_{t}")
                nc.sync.dma_start(big2[:], psl2_d[:, t])

            # ---- h1: KG k-groups x 2 M-chunks
            # m-outer so h1ps[0] completes (and sil1[0] starts) ~1us
            # earlier, shrinking the h2 wait on sil1
            h1ps = [pp_h1.tile([128, TILE], f32, tag=f"h1m{m}", name=f"h1ps{t}_{m}") for m in range(2)]
            sil1 = silp.tile([128, 2, TILE], f16, tag="sil1")
            for m in range(2):
                for j in range(KG):
                    nc.tensor.matmul(
                        h1ps[m][:],
                        w1_sb[:, j, m],
                        big[:, KS * j : KS * (j + 1)],
                        start=(j == 0),
                        stop=(j == KG - 1),
                        perf_mode=DR,
                    )
                nc.scalar.activation(sil1[:, m], h1ps[m][:], AF.Silu)

            # ---- psl: M=1 matmuls in column groups, partial sums landing
            # on e_ps rows (fills the PE while sil1 runs on ACT)
            if fp8 and psl_mode == "hilo8":
                for cn, (wp, mv) in enumerate(
                    [(wpa_sb, big), (wpa_sb, big2), (wpb_sb, big)]
                ):
                    for j in range(4):
                        nc.tensor.matmul(
                            e_ps[0:1, :],
                            wp[:, j, :, 0:1],
                            mv[:, 2 * j : 2 * j + 2],
                            start=(cn == 0 and j == 0),
                            stop=False,
                            perf_mode=DR,
                        )
            elif fp8:
                # psl from the scaled-e3m4/fp16 psT copy: fast M=1 matmuls
                # spread over all 4 column groups (consecutive same-row
                # PSUM accumulation stalls ~40ns each); the 1/(s*t)
                # unscale rides in the whole ones vector, with wout
                # pre-scaled by s*t so psnn can share rows 0/32
                for k in range(8):
                    g = 32 * (k % 4)
                    nc.tensor.matmul(
                        e_ps[g : g + 1, :],
                        wpa_sb[:, k],
                        big2[:, k],
                        start=(k < 4),
                        stop=False,
                        tile_position=(0, g),
                    )
            else:
                for k in range(8):
                    g = 32 * (k % 4)
                    nc.tensor.matmul(
                        e_ps[g : g + 1, :],
                        wpa_sb[:, k],
                        big[:, k],
                        start=(k < 4),
                        stop=False,
                        tile_position=(0, g),
                    )

            # ---- h2 (fp16)
            h2ps = [pp_h2.tile([128, TILE], f32, tag=f"h2m{m}", name=f"h2ps{t}_{m}") for m in range(2)]
            for kj in range(2):
                for m in range(2):
                    nc.tensor.matmul(
                        h2ps[m][:],
                        w2_sb[:, m, kj],
                        sil1[:, kj],
                        start=(kj == 0),
                        stop=(kj == 1),
                    )
            sil2 = silp.tile([128, 2, TILE], f16, tag="sil2")
            for m in range(2):
                nc.scalar.activation(sil2[:, m], h2ps[m][:], AF.Silu)

            # ---- previous tile's tail fills the sil2 wait
            if prev is not None:
                tail(*prev)

            # ---- psnn (fp16) accumulates onto the psl rows 0/32
            # (hilo8: fresh rows 64/96 since psl only writes row 0)
            for kj in range(2):
                if fp8 and psl_mode == "hilo8":
                    g, st = 64 + 32 * kj, True
                else:
                    g, st = 32 * kj, False
                nc.tensor.matmul(
                    e_ps[g : g + 1, :],
                    wout_sb[:, kj],
                    sil2[:, kj],
                    start=st,
                    stop=(kj == 1),
                    tile_position=(0, g),
                )
            # partial rows 0/32/64/96 (zeros between) -> SBUF in one copy
            e_row = rowp.tile([97, TILE], f16, tag="erow")
            nc.vector.tensor_copy(e_row[:], e_ps[0:97, :])

            ohs_cur = ohs_next
            if t + 1 < nT:
                ohs_next = gen_ohs(t + 1)
            prev = (t, e_row, ohs_cur)

        tail(*prev)

        out_sb = rowp.tile([1, SMAX], f32, tag="outsb")
        if seg_ps2 is not None:
            # DVE may read only one PSUM operand: stage one bank via SBUF
            tmp_sb = rowp.tile([1, SMAX], f32, tag="outtmp")
            nc.scalar.activation(tmp_sb[:], seg_ps[:], AF.Copy)
            nc.vector.tensor_add(out_sb[:], tmp_sb[:], seg_ps2[:])
        else:
            nc.scalar.activation(out_sb[:], seg_ps[:], AF.Copy)
        nc.sync.dma_start(out_d[:], out_sb[:])

    _split_waits(nc, mybir)
    return nc


def _install_ntff_hook():
    """Register the axon NTFF profile hook (missing antenv.axon_hooks in
    this image) so run_bass_kernel_spmd(trace=True) can report exec_time_ns."""
    import sys
    import types

    try:
        import antenv.axon_hooks  # noqa: F401

        return
    except ImportError:
        pass
    from trn_agent_boot.trn_boot import _ntff_profile_via_ctypes

    hook = _ntff_profile_via_ctypes("/opt/axon/libaxon_pjrt.so")
    mod = types.ModuleType("antenv.axon_hooks")
    mod.get_axon_ntff_profile_hook = lambda: hook
    mod.set_axon_ntff_profile_hook = lambda h: None
    sys.modules["antenv.axon_hooks"] = mod
    import antenv

    antenv.axon_hooks = mod
    import concourse.bass_utils as bu

    bu.upload_artifacts = lambda tmpdir: tmpdir


def _to_psT(pss, Ta):
    """[Ta, 1024] -> [128, nT, 8, TILE] with [p,t,k,a] = pss[t*TILE+a, k*128+p]"""
    nT = Ta // TILE
    return np.ascontiguousarray(pss.reshape(nT, TILE, 8, 128).transpose(3, 0, 2, 1))


def _to_psT_i(pss, Ta):
    """[Ta, 1024] -> [128, nT, 4, TILE, 2] byte-interleaved k-pairs:
    [p, t, j, a, i] = pss[t*TILE+a, (2j+i)*128+p]"""
    nT = Ta // TILE
    return np.ascontiguousarray(
        pss.reshape(nT, TILE, 4, 2, 128).transpose(4, 0, 2, 1, 3)
    )


def kernel(ps, numbers, batch, W_comp, W_psl, W_h1, W_h2, W_out):
    global LAST_EXEC_NS, LAST_RESULTS
    from concourse.bass_utils import run_bass_kernel_spmd

    if TRACE:
        _install_ntff_hook()

    fp8 = MODE == "fp8mix"
    DT = F8 if fp8 else np.float16
    KG = 4 if fp8 else 8
    KS = 2 if fp8 else 1

    ps = np.asarray(ps)
    numbers = np.asarray(numbers)
    batch = np.asarray(batch)
    W_comp = np.asarray(W_comp, dtype=np.float32)
    W_psl = np.asarray(W_psl, dtype=np.float32)
    W_h1 = np.asarray(W_h1, dtype=np.float32)
    W_h2 = np.asarray(W_h2, dtype=np.float32)
    W_out = np.asarray(W_out, dtype=np.float32)

    counts = np.bincount(batch, minlength=N_STRUCT)
    cum = np.zeros(N_STRUCT + 1, dtype=np.int64)
    np.cumsum(counts, out=cum[1:])

    # equal-structure shard cuts (atoms balance to ~1-2% by CLT; keeps
    # per-core structure count fixed at N_STRUCT/N_CORES <= SMAX)
    s_cut = [i * N_STRUCT // N_CORES for i in range(N_CORES + 1)]

    shards = []
    for i in range(N_CORES):
        s_lo, s_hi = s_cut[i], s_cut[i + 1]
        a_lo, a_hi = int(cum[s_lo]), int(cum[s_hi])
        n_at, n_st = a_hi - a_lo, s_hi - s_lo
        assert n_st <= SMAX, f"shard {i}: {n_st} structs > {SMAX}"
        shards.append((s_lo, s_hi, a_lo, a_hi, n_at, n_st))

    Ta = max(s[4] for s in shards)
    Ta = (Ta + TILE - 1) // TILE * TILE
    nT = Ta // TILE
    C = Ta // CHUNK

    # replicated weights: w1[p, j, m, i, c] = W_h1[m*128+c, (KS*j+i)*128+p]
    w1 = np.ascontiguousarray(
        W_h1.T.reshape(KG, KS, 128, 2, 128).transpose(2, 0, 3, 1, 4)
    ).astype(DT)
    # w2[p, m, i, c] = W_h2[m*128+c, i*128+p]  (fp16)
    w2 = np.ascontiguousarray(
        W_h2.T.reshape(2, 128, 2, 128).transpose(1, 2, 0, 3)
    ).astype(np.float16)
    # wout[p, i] = W_out[0, i*128+p]  (fp16)
    wout = np.ascontiguousarray(
        W_out[0].reshape(2, 128).T.reshape(128, 2, 1)
    ).astype(np.float16)
    # psl weights
    if fp8 and PSL_MODE == "hilo8":
        wp32 = W_psl[0].astype(np.float32)
        wp_hi8 = wp32.astype(F8)
        wp_lo8 = (wp32 - wp_hi8.astype(np.float32)).astype(F8)
        def pack(w):
            out = np.zeros((128, 4, 2, 16), dtype=F8)
            out[..., 0] = np.asarray(w).reshape(4, 2, 128).transpose(2, 0, 1)
            return out

        wpsl_hi, wpsl_lo = pack(wp_hi8), pack(wp_lo8)
    elif fp8 and PSL_MODE == "e3":
        t_w = 15.5 / max(float(np.abs(W_psl).max()), 1e-30)
        _WPSL_T = [t_w]
        wpsl16 = np.asarray(
            np.clip(W_psl[0].astype(np.float32) * t_w, -15.5, 15.5)
            .reshape(8, 128).T.reshape(128, 8, 1, 1),
            dtype=ml_dtypes.float8_e3m4,
        ).copy()
    else:
        wpsl16 = np.ascontiguousarray(
            W_psl[0].reshape(8, 128).T.reshape(128, 8, 1, 1)
        ).astype(np.float16)
    iota = np.ascontiguousarray(
        np.tile(np.arange(SMAX, dtype=np.float16), (128, 1))
    )
    # exact cubic through the 4 species energies
    V = np.vander(np.arange(N_SPECIES, dtype=np.float64), 4, increasing=True)
    poly = np.linalg.solve(V, W_comp[0, :N_SPECIES].astype(np.float64))

    # per-shard relative batch ids (padding = -1)
    rb_all = []
    for s_lo, s_hi, a_lo, a_hi, n_at, n_st in shards:
        rb = np.full(Ta, -1.0, dtype=np.float32)
        rb[:n_at] = (batch[a_lo:a_hi] - s_lo).astype(np.float32)
        rb_all.append(rb)

    segw = None
    if SEG_WINDOWED:
        # shared per-chunk structure windows: batch is sorted so each
        # 128-atom chunk only spans a few structures; bake [w0, w0+W)
        m0 = np.full(C, np.inf)
        m1 = np.full(C, -np.inf)
        for rb in rb_all:
            r2 = rb.reshape(C, CHUNK)
            mask = r2 >= 0
            has = mask.any(axis=1)
            lo = np.where(has, np.where(mask, r2, np.inf).min(axis=1), np.inf)
            hi = np.where(has, np.where(mask, r2, -np.inf).max(axis=1), -np.inf)
            m0 = np.minimum(m0, lo)
            m1 = np.maximum(m1, hi)
        w0 = np.where(np.isfinite(m0), m0, 0.0)
        span = np.where(np.isfinite(m1), m1 - w0 + 1, 1.0)
        W = int(max(1, span.max()))
        W = min((W + 3) // 4 * 4, SMAX)
        w0 = np.clip(w0, 0, SMAX - W).astype(np.int64)
        segw = (W, tuple(int(x) for x in w0))

    key = (Ta, C, tuple(np.round(poly, 12)), MODE, PSL_MODE, segw)
    if key not in _BUILD_CACHE:
        _BUILD_CACHE.clear()
        _BUILD_CACHE[key] = _build(Ta, C, poly, MODE, PSL_MODE, segw)
    nc = _BUILD_CACHE[key]

    psq = np.asarray(ps, dtype=DT)  # quantize once, slice per shard
    ones = np.ones((97, 1), dtype=np.float16)
    if fp8 and PSL_MODE == "hilo8":
        ps2 = (ps.astype(np.float32) - psq.astype(np.float32)).astype(F8)
    elif fp8 and PSL_MODE == "e3":
        # scale ps and wpsl into e3m4's normal range (subnormals would
        # cost ~8% error on small values); the 1/(s*t) unscale rides in
        # the colize ones vector rows 0:33 (psl partial rows 0/32)
        s_ps = 15.5 / max(float(np.abs(ps).max()), 1e-30)
        ps2 = np.asarray(
            np.clip(ps.astype(np.float32) * s_ps, -15.5, 15.5),
            dtype=ml_dtypes.float8_e3m4,
        )
        ones[0:97] = np.float16(1.0 / (s_ps * _WPSL_T[0]))
        wout = np.ascontiguousarray(
            (W_out[0] * (s_ps * _WPSL_T[0])).reshape(2, 128).T.reshape(128, 2, 1)
        ).astype(np.float16)
    elif fp8:
        ps2 = ps.astype(np.float16)

    in_maps = []
    for si, (s_lo, s_hi, a_lo, a_hi, n_at, n_st) in enumerate(shards):
        pss = np.zeros((Ta, N_FEAT), dtype=DT)
        pss[:n_at] = psq[a_lo:a_hi]
        rb = rb_all[si]
        if segw is not None:
            rb = rb.reshape(C, CHUNK) - np.asarray(segw[1], dtype=np.float32)[:, None]
            rb = rb.reshape(Ta)
        nums = np.zeros(Ta, dtype=np.float32)
        nums[:n_at] = numbers[a_lo:a_hi].astype(np.float32)
        im = {
            "psT": _to_psT(pss, Ta),
            "relb": np.ascontiguousarray(rb.reshape(C, CHUNK).T),
            "nums": np.ascontiguousarray(nums.reshape(C, CHUNK).T),
            "w1": w1,
            "w2": w2,
            "wout": wout,
            "ones": ones,
            "iota": iota,
        }
        if fp8 and PSL_MODE == "hilo8":
            pss2 = np.zeros((Ta, N_FEAT), dtype=F8)
            pss2[:n_at] = ps2[a_lo:a_hi]
            im["psT2"] = _to_psT(pss2, Ta)
            im["wpsl_hi"] = wpsl_hi
            im["wpsl_lo"] = wpsl_lo
        elif fp8:
            pss2 = np.zeros((Ta, N_FEAT), dtype=ps2.dtype)
            pss2[:n_at] = ps2[a_lo:a_hi]
            im["psT2"] = _to_psT(pss2, Ta)
            im["wpsl"] = wpsl16
        else:
            im["wpsl"] = wpsl16
        in_maps.append(im)

    res = run_bass_kernel_spmd(nc, in_maps, list(range(N_CORES)), trace=TRACE)
    LAST_EXEC_NS = res.exec_time_ns
    LAST_RESULTS = res

    out = np.zeros((N_STRUCT, 1), dtype=np.float32)
    for i, (s_lo, s_hi, a_lo, a_hi, n_at, n_st) in enumerate(shards):
        vals = res.results[i]["out"][0, :n_st].astype(np.float32)
        empty = counts[s_lo:s_hi] == 0
        if empty.any():
            vals = np.where(empty, 0.0, vals)
        out[s_lo:s_hi, 0] = vals
    return out


# revision 36
# speedup vs baseline: 1.3414x; 1.0039x over previous
"""PowerSpectrumModel Trainium2 kernel (8 NeuronCores, SPMD).

Strategy (data-parallel over atoms, structures disjoint per shard):
 - Host: cut the atom axis at structure boundaries into 8 balanced shards;
   pre-TRANSPOSE ps to feature-major [128, nT, 8, TILE] (so every tile
   load is one contiguous stride per partition — no on-device transpose
   descriptors); fp32 PSUM accumulation on device; replicate the small
   weight matrices.
 - Precision (fp8mix mode): the big h1 GEMM runs fp8e4m3 DoubleRow (2x
   PE rate; w1-quant + ps-quant contribute only ~6e-3 rel err because
   errors decorrelate across atoms); h2/psnn stay fp16 (their weight
   quantization biases the per-atom energy coherently across a structure
   -> 3.6e-2 if fp8); psl gets hi+lo fp8 splits of both ps and W_psl
   (or an extra fp16 psT copy, PSL_MODE).
 - Device, per 512-atom tile:
     psT   <- contiguous loads [128, 8, 512]                      [DMA]
     h1    = W_h1 @ psT  (fp8 DoubleRow k-pairs / fp16 k-chunks)  [PE]
     psl   = W_psl row, M=1 matmuls -> e partial rows 0/32/64/96  [PE]
     sil1  = silu(h1)                                             [ACT]
     h2    = W_h2 @ sil1; sil2 = silu(h2)                         [PE/ACT]
     psnn  M=1 matmuls accumulated onto the psl PSUM rows         [PE]
     per 128-chunk: e column via K=97 ones matmul, + species
     energy, one-hot(struct) matmul accumulates into a [1,256]
     PSUM row holding this core's per-structure energies.         [PE/DVE]
 - Host: slice per-core structure ranges, concat -> [2000, 1].
"""

import numpy as np
import ml_dtypes

N_ATOMS = 200000
N_FEAT = 1024
N_SPECIES = 4
N_STRUCT = 2000
H1 = 256
H2 = 256
SCALE = 1.0
N_CORES = 8
TILE = 512
CHUNK = 128
SMAX = 256  # per-core structure capacity (PSUM row)

MODE = "fp8mix"  # "fp16" | "fp8mix"
PSL_MODE = "e3"  # "e3" | "hilo8" | "fp16"
SEG_WINDOWED = True  # bake per-chunk structure windows into the seg matmul

_BUILD_CACHE = {}
TRACE = False
LAST_EXEC_NS = None
LAST_RESULTS = None

F8 = ml_dtypes.float8_e4m3


def _split_waits(nc, mybir, maxw=1):
    """walrus on this build rejects >1 sync wait per instruction; move
    overflow waits onto preceding same-engine NoOps."""
    cnt = 0
    for f in nc.m.functions:
        for blk in f.blocks:
            if not hasattr(blk, "instructions"):
                continue
            out = []
            changed = False
            for inst in blk.instructions:
                si = getattr(inst, "sync_info", None)
                if si is not None and si.on_wait and len(si.on_wait) > maxw:
                    waits = list(si.on_wait)
                    keep = waits[-maxw:]
                    extra = waits[:-maxw]
                    while extra:
                        chunk, extra = extra[:maxw], extra[maxw:]
                        cnt += 1
                        out.append(
                            mybir.InstNoOp(
                                name=f"waitfix-{cnt}",
                                engine=inst.engine,
                                text_hint="waitfix",
                                bass_nofuse=True,
                                ins=[],
                                outs=[],
                                sync_info=mybir.SyncInfo(on_wait=chunk, on_update=[]),
                            )
                        )
                    si.on_wait = keep
                    changed = True
                out.append(inst)
            if changed:
                blk.instructions[:] = out
    return cnt


def _build(Ta, C, poly, mode, psl_mode, segw=None):
    import concourse.bass as bass
    import concourse.tile as tile
    import concourse.mybir as mybir
    from contextlib import ExitStack

    fp8 = mode == "fp8mix"
    f8 = mybir.dt.float8e4
    f16 = mybir.dt.float16
    f32 = mybir.dt.float32
    fd = f8 if fp8 else f16  # h1 GEMM dtype
    AF = mybir.ActivationFunctionType
    ALU = mybir.AluOpType
    DR = mybir.MatmulPerfMode.DoubleRow if fp8 else None
    PSUM = bass.MemorySpace.PSUM
    nT = Ta // TILE
    c0, c1, c2, c3 = (float(x) for x in poly)
    KG = 4 if fp8 else 8  # h1 k-groups (DoubleRow contracts pairs)
    KS = 2 if fp8 else 1  # k-chunks per h1 matmul

    nc = bass.Bass("TRN2", target_bir_lowering=False, debug=False)

    # pre-transposed ps: [p, t, k, a] = ps[t*512+a, k*128+p]
    psT_d = nc.dram_tensor("psT", [128, nT, 8, TILE], fd, kind="ExternalInput").ap()
    f8e3 = mybir.dt.float8e3
    psl2_d = None
    if fp8:
        d2 = {"hilo8": f8, "e3": f8e3, "fp16": f16}[psl_mode]
        psl2_d = nc.dram_tensor("psT2", [128, nT, 8, TILE], d2, kind="ExternalInput").ap()
    relb_d = nc.dram_tensor("relb", [CHUNK, C], f32, kind="ExternalInput").ap()
    nums_d = nc.dram_tensor("nums", [CHUNK, C], f32, kind="ExternalInput").ap()
    # w1: [p, j, m, i, c] = W_h1[m*128+c, (KS*j+i)*128+p]
    w1_d = nc.dram_tensor("w1", [128, KG, 2, KS, 128], fd, kind="ExternalInput").ap()
    # w2: [p, m, i, c] = W_h2[m*128+c, i*128+p]  (fp16)
    w2_d = nc.dram_tensor("w2", [128, 2, 2, 128], f16, kind="ExternalInput").ap()
    # wout: [p, i] = W_out[0, i*128+p]  (fp16)
    wout_d = nc.dram_tensor("wout", [128, 2, 1], f16, kind="ExternalInput").ap()
    # psl weights
    if fp8 and psl_mode == "hilo8":
        # k-subtile dim padded to step 16 (ISA: dual-fp8 LW step%16==0)
        wpa_d = nc.dram_tensor("wpsl_hi", [128, 4, 2, 16], f8, kind="ExternalInput").ap()
        wpb_d = nc.dram_tensor("wpsl_lo", [128, 4, 2, 16], f8, kind="ExternalInput").ap()
    elif fp8 and psl_mode == "e3":
        wpa_d = nc.dram_tensor("wpsl", [128, 8, 1, 1], f8e3, kind="ExternalInput").ap()
        wpb_d = None
    else:
        wpa_d = nc.dram_tensor("wpsl", [128, 8, 1, 1], f16, kind="ExternalInput").ap()
        wpb_d = None
    ones_d = nc.dram_tensor("ones", [97, 1], f16, kind="ExternalInput").ap()
    iota_d = nc.dram_tensor("iota", [128, SMAX], f16, kind="ExternalInput").ap()
    out_d = nc.dram_tensor("out", [1, SMAX], f32, kind="ExternalOutput").ap()

    with tile.TileContext(nc) as tc, ExitStack() as ctx:
        const = ctx.enter_context(tc.tile_pool(name="const", bufs=1))
        psTp = ctx.enter_context(tc.tile_pool(name="psT", bufs=8))
        psTp2 = ctx.enter_context(tc.tile_pool(name="psT2", bufs=6)) if fp8 else None
        silp = ctx.enter_context(tc.tile_pool(name="sil", bufs=6))
        rowp = ctx.enter_context(tc.tile_pool(name="row", bufs=6))
        pp_h1 = ctx.enter_context(tc.tile_pool(name="pph1", bufs=1, space=PSUM))
        pp_h2 = ctx.enter_context(tc.tile_pool(name="pph2", bufs=1, space=PSUM))
        pp_e = ctx.enter_context(tc.tile_pool(name="ppe", bufs=1, space=PSUM))
        pp_ec = ctx.enter_context(tc.tile_pool(name="ppec", bufs=1, space=PSUM))
        pp_seg = ctx.enter_context(tc.tile_pool(name="ppseg", bufs=1, space=PSUM))

        # ---- constants ----
        w1_sb = const.tile([128, KG, 2, KS, 128], fd, tag="w1")
        nc.sync.dma_start(w1_sb[:], w1_d[:])
        w2_sb = const.tile([128, 2, 2, 128], f16, tag="w2")
        nc.sync.dma_start(w2_sb[:], w2_d[:])
        wout_sb = const.tile([128, 2, 1], f16, tag="wout")
        nc.sync.dma_start(wout_sb[:], wout_d[:])
        if fp8 and psl_mode == "hilo8":
            wpa_sb = const.tile([128, 4, 2, 16], f8, tag="wpa")
            nc.sync.dma_start(wpa_sb[:], wpa_d[:])
            wpb_sb = const.tile([128, 4, 2, 16], f8, tag="wpb")
            nc.sync.dma_start(wpb_sb[:], wpb_d[:])
        else:
            wpa_sb = const.tile([128, 8, 1, 1], f8e3 if (fp8 and psl_mode == "e3") else f16, tag="wpa")
            nc.sync.dma_start(wpa_sb[:], wpa_d[:])
            wpb_sb = None
        iota_sb = const.tile([128, SMAX], f16, tag="iota")
        nc.sync.dma_start(iota_sb[:], iota_d[:])
        relb_sb = const.tile([CHUNK, C], f32, tag="relb")
        nc.sync.dma_start(relb_sb[:], relb_d[:])
        nums_sb = const.tile([CHUNK, C], f32, tag="nums")
        nc.sync.dma_start(nums_sb[:], nums_d[:])
        ones_sb = const.tile([97, 1], f16, tag="ones")
        nc.sync.dma_start(ones_sb[:], ones_d[:])

        # species energy per atom: cubic through W_comp[0, 0..3]
        # comp = (c1*n + c0) + n*n*(c3*n + c2)
        t_n2 = const.tile([CHUNK, C], f32, tag="t_n2")
        nc.vector.tensor_mul(t_n2[:], nums_sb[:], nums_sb[:])
        t_a = const.tile([CHUNK, C], f32, tag="t_a")
        nc.vector.tensor_scalar(t_a[:], nums_sb[:], c3, c2, ALU.mult, ALU.add)
        t_b = const.tile([CHUNK, C], f32, tag="t_b")
        nc.vector.tensor_mul(t_b[:], t_n2[:], t_a[:])
        t_c = const.tile([CHUNK, C], f32, tag="t_c")
        nc.vector.tensor_scalar(t_c[:], nums_sb[:], c1, c0, ALU.mult, ALU.add)
        comp_sb = const.tile([CHUNK, C], f32, tag="comp")
        nc.vector.tensor_add(comp_sb[:], t_b[:], t_c[:])

        seg_ps = pp_seg.tile([1, SMAX], f32, tag="seg")
        nc.vector.memset(seg_ps[:], 0.0)
        seg_ps2 = None
        if segw is not None:
            # second accumulator so consecutive seg matmuls alternate
            # PSUM banks instead of read-modify-writing the same one
            seg_ps2 = pp_seg.tile([1, SMAX], f32, tag="seg2")
            nc.vector.memset(seg_ps2[:], 0.0)
        # e-partials bank: psl rows 0/32, psnn rows 64/96, with a +16
        # partition offset on odd tiles (same PSUM quadrant, so the
        # matmul dst stays valid) — in-bank double buffering that breaks
        # the psl(t+1) <- e_row-copy(t) WAR. Rows in between stay 0 from
        # this one-time clear; the K=113 ones-matmul sums the partials.
        e_ps = pp_e.tile([128, TILE], f32, tag="e")
        nc.vector.memset(e_ps[:], 0.0)


        ohp = ctx.enter_context(tc.tile_pool(name="ohp", bufs=8))

        def gen_ohs(tn):
            ohs = []
            for cc in range(4):
                ch = tn * 4 + cc
                if segw is not None:
                    W = segw[0]
                    oh = ohp.tile([128, W], f16, tag="oh", name=f"oh{tn}_{cc}")
                    nc.vector.tensor_scalar(
                        oh[:], iota_sb[:, 0:W], relb_sb[:, ch : ch + 1], None,
                        ALU.is_equal,
                    )
                else:
                    oh = ohp.tile([128, SMAX], f16, tag="oh", name=f"oh{tn}_{cc}")
                    nc.vector.tensor_scalar(
                        oh[:], iota_sb[:], relb_sb[:, ch : ch + 1], None,
                        ALU.is_equal,
                    )
                ohs.append(oh)
            return ohs

        def tail(tp, e_row_p, oh_p):
            # column-ize all 4 chunks into one PSUM tile, add species
            # energy on the DVE, then the 4 windowed seg matmuls
            ec4 = pp_ec.tile([128, 4], f32, tag="ec", name=f"ec4_{tp}")
            ecols = []
            for cc in range(4):
                ch = tp * 4 + cc
                nc.tensor.matmul(
                    ec4[:, cc : cc + 1],
                    e_row_p[0:97, cc * 128 : (cc + 1) * 128],
                    ones_sb[:],
                    start=True,
                    stop=True,
                )
                e_col = rowp.tile([128, 1], f16, tag="ecol", name=f"ecol{tp}_{cc}")
                nc.vector.tensor_add(e_col[:], ec4[:, cc : cc + 1], comp_sb[:, ch : ch + 1])
                ecols.append(e_col)
            for cc in range(4):
                ch = tp * 4 + cc
                if segw is not None:
                    W, w0s = segw
                    w0 = w0s[ch]
                    acc = seg_ps if cc % 2 == 0 else seg_ps2
                    nc.tensor.matmul(
                        acc[0:1, w0 : w0 + W], ecols[cc][:], oh_p[cc][:],
                        start=False, stop=(ch >= C - 2), skip_group_check=True,
                    )
                else:
                    nc.tensor.matmul(
                        seg_ps[:], ecols[cc][:], oh_p[cc][:],
                        start=(ch == 0), stop=(ch == C - 1),
                    )

        # Software-pipelined main loop: tile t-1's colize/seg run between
        # tile t's h2 and psnn (filling the sil2 wait on the PE), with the
        # one-hot windows generated a tile ahead on the DVE.
        prev = None  # (tp, e_row, ohs) ready for tail
        pend = None  # (tp, sil2, ohs) awaiting deferred psnn kj=1 + copy
        ohs_next = gen_ohs(0)

        def psnn_mm(kj, sil2_p, st):
            g = (64 + 32 * kj) if (fp8 and psl_mode == "hilo8") else (32 * kj)
            nc.tensor.matmul(
                e_ps[g : g + 1, :],
                wout_sb[:, kj],
                sil2_p[:, kj],
                start=st,
                stop=(kj == 1),
                tile_position=(0, g),
            )

        def finish_tile(pend_t):
            # deferred psnn kj=1 (sil2 long ready) + e_row copy
            tp, sil2_p, ohs_p = pend_t
            psnn_mm(1, sil2_p, fp8 and psl_mode == "hilo8")
            e_row = rowp.tile([97, TILE], f16, tag="erow", name=f"erow{tp}")
            nc.vector.tensor_copy(e_row[:], e_ps[0:97, :])
            return (tp, e_row, ohs_p)

        for t in range(nT):
            if pend is not None:
                prev = finish_tile(pend)
                pend = None

            # ---- contiguous loads: [128, 8, 512]
            big = psTp.tile([128, 8, TILE], fd, tag="psT", name=f"psT{t}")
            nc.sync.dma_start(big[:], psT_d[:, t])
            if fp8:
                d2 = {"hilo8": f8, "e3": f8e3, "fp16": f16}[psl_mode]
                big2 = psTp2.tile([128, 8, TILE], d2, tag="psT2", name=f"psT2_{t}")
                nc.sync.dma_start(big2[:], psl2_d[:, t])

            # ---- h1: KG k-groups x 2 M-chunks (m-outer so sil1[0]
            # starts early)
            h1ps = [pp_h1.tile([128, TILE], f32, tag=f"h1m{m}", name=f"h1ps{t}_{m}") for m in range(2)]
            sil1 = silp.tile([128, 2, TILE], f16, tag="sil1")
            for m in range(2):
                for j in range(KG):
                    nc.tensor.matmul(
                        h1ps[m][:],
                        w1_sb[:, j, m],
                        big[:, KS * j : KS * (j + 1)],
                        start=(j == 0),
                        stop=(j == KG - 1),
                        perf_mode=DR,
                    )
                nc.scalar.activation(sil1[:, m], h1ps[m][:], AF.Silu)

            # ---- psl: M=1 matmuls spread over the 4 column groups
            # (fills the PE while sil1 runs on ACT)
            if fp8 and psl_mode == "hilo8":
                for cn, (wp, mv) in enumerate(
                    [(wpa_sb, big), (wpa_sb, big2), (wpb_sb, big)]
                ):
                    for j in range(4):
                        nc.tensor.matmul(
                            e_ps[0:1, :],
                            wp[:, j, :, 0:1],
                            mv[:, 2 * j : 2 * j + 2],
                            start=(cn == 0 and j == 0),
                            stop=False,
                            perf_mode=DR,
                        )
            elif fp8:
                for k in range(8):
                    g = 32 * (k % 4)
                    nc.tensor.matmul(
                        e_ps[g : g + 1, :],
                        wpa_sb[:, k],
                        big2[:, k],
                        start=(k < 4),
                        stop=False,
                        tile_position=(0, g),
                    )
            else:
                for k in range(8):
                    g = 32 * (k % 4)
                    nc.tensor.matmul(
                        e_ps[g : g + 1, :],
                        wpa_sb[:, k],
                        big[:, k],
                        start=(k < 4),
                        stop=False,
                        tile_position=(0, g),
                    )

            # ---- h2 (fp16)
            h2ps = [pp_h2.tile([128, TILE], f32, tag=f"h2m{m}", name=f"h2ps{t}_{m}") for m in range(2)]
            for kj in range(2):
                for m in range(2):
                    nc.tensor.matmul(
                        h2ps[m][:],
                        w2_sb[:, m, kj],
                        sil1[:, kj],
                        start=(kj == 0),
                        stop=(kj == 1),
                    )
            sil2 = silp.tile([128, 2, TILE], f16, tag="sil2")
            for m in range(2):
                nc.scalar.activation(sil2[:, m], h2ps[m][:], AF.Silu)

            # ---- previous tile's tail fills the sil2 wait
            if prev is not None:
                tail(*prev)
                prev = None

            # psnn kj=0 (sil2[0] ready soonest); kj=1 is deferred to the
            # next tile body where its dependency is ancient
            psnn_mm(0, sil2, False)

            ohs_cur = ohs_next
            if t + 1 < nT:
                ohs_next = gen_ohs(t + 1)
            pend = (t, sil2, ohs_cur)

        tail(*finish_tile(pend))

        out_sb = rowp.tile([1, SMAX], f32, tag="outsb")
        if seg_ps2 is not None:
            # DVE may read only one PSUM operand: stage one bank via SBUF
            tmp_sb = rowp.tile([1, SMAX], f32, tag="outtmp")
            nc.scalar.activation(tmp_sb[:], seg_ps[:], AF.Copy)
            nc.vector.tensor_add(out_sb[:], tmp_sb[:], seg_ps2[:])
        else:
            nc.scalar.activation(out_sb[:], seg_ps[:], AF.Copy)
        nc.sync.dma_start(out_d[:], out_sb[:])

    _split_waits(nc, mybir)
    return nc


def _install_ntff_hook():
    """Register the axon NTFF profile hook (missing antenv.axon_hooks in
    this image) so run_bass_kernel_spmd(trace=True) can report exec_time_ns."""
    import sys
    import types

    try:
        import antenv.axon_hooks  # noqa: F401

        return
    except ImportError:
        pass
    from trn_agent_boot.trn_boot import _ntff_profile_via_ctypes

    hook = _ntff_profile_via_ctypes("/opt/axon/libaxon_pjrt.so")
    mod = types.ModuleType("antenv.axon_hooks")
    mod.get_axon_ntff_profile_hook = lambda: hook
    mod.set_axon_ntff_profile_hook = lambda h: None
    sys.modules["antenv.axon_hooks"] = mod
    import antenv

    antenv.axon_hooks = mod
    import concourse.bass_utils as bu

    bu.upload_artifacts = lambda tmpdir: tmpdir


def _to_psT(pss, Ta):
    """[Ta, 1024] -> [128, nT, 8, TILE] with [p,t,k,a] = pss[t*TILE+a, k*128+p]"""
    nT = Ta // TILE
    return np.ascontiguousarray(pss.reshape(nT, TILE, 8, 128).transpose(3, 0, 2, 1))


def _to_psT_i(pss, Ta):
    """[Ta, 1024] -> [128, nT, 4, TILE, 2] byte-interleaved k-pairs:
    [p, t, j, a, i] = pss[t*TILE+a, (2j+i)*128+p]"""
    nT = Ta // TILE
    return np.ascontiguousarray(
        pss.reshape(nT, TILE, 4, 2, 128).transpose(4, 0, 2, 1, 3)
    )


def kernel(ps, numbers, batch, W_comp, W_psl, W_h1, W_h2, W_out):
    global LAST_EXEC_NS, LAST_RESULTS
    from concourse.bass_utils import run_bass_kernel_spmd

    if TRACE:
        _install_ntff_hook()

    fp8 = MODE == "fp8mix"
    DT = F8 if fp8 else np.float16
    KG = 4 if fp8 else 8
    KS = 2 if fp8 else 1

    ps = np.asarray(ps)
    numbers = np.asarray(numbers)
    batch = np.asarray(batch)
    W_comp = np.asarray(W_comp, dtype=np.float32)
    W_psl = np.asarray(W_psl, dtype=np.float32)
    W_h1 = np.asarray(W_h1, dtype=np.float32)
    W_h2 = np.asarray(W_h2, dtype=np.float32)
    W_out = np.asarray(W_out, dtype=np.float32)

    counts = np.bincount(batch, minlength=N_STRUCT)
    cum = np.zeros(N_STRUCT + 1, dtype=np.int64)
    np.cumsum(counts, out=cum[1:])

    # equal-structure shard cuts (atoms balance to ~1-2% by CLT; keeps
    # per-core structure count fixed at N_STRUCT/N_CORES <= SMAX)
    s_cut = [i * N_STRUCT // N_CORES for i in range(N_CORES + 1)]

    shards = []
    for i in range(N_CORES):
        s_lo, s_hi = s_cut[i], s_cut[i + 1]
        a_lo, a_hi = int(cum[s_lo]), int(cum[s_hi])
        n_at, n_st = a_hi - a_lo, s_hi - s_lo
        assert n_st <= SMAX, f"shard {i}: {n_st} structs > {SMAX}"
        shards.append((s_lo, s_hi, a_lo, a_hi, n_at, n_st))

    Ta = max(s[4] for s in shards)
    Ta = (Ta + TILE - 1) // TILE * TILE
    nT = Ta // TILE
    C = Ta // CHUNK

    # replicated weights: w1[p, j, m, i, c] = W_h1[m*128+c, (KS*j+i)*128+p]
    w1 = np.ascontiguousarray(
        W_h1.T.reshape(KG, KS, 128, 2, 128).transpose(2, 0, 3, 1, 4)
    ).astype(DT)
    # w2[p, m, i, c] = W_h2[m*128+c, i*128+p]  (fp16)
    w2 = np.ascontiguousarray(
        W_h2.T.reshape(2, 128, 2, 128).transpose(1, 2, 0, 3)
    ).astype(np.float16)
    # wout[p, i] = W_out[0, i*128+p]  (fp16)
    wout = np.ascontiguousarray(
        W_out[0].reshape(2, 128).T.reshape(128, 2, 1)
    ).astype(np.float16)
    # psl weights
    if fp8 and PSL_MODE == "hilo8":
        wp32 = W_psl[0].astype(np.float32)
        wp_hi8 = wp32.astype(F8)
        wp_lo8 = (wp32 - wp_hi8.astype(np.float32)).astype(F8)
        def pack(w):
            out = np.zeros((128, 4, 2, 16), dtype=F8)
            out[..., 0] = np.asarray(w).reshape(4, 2, 128).transpose(2, 0, 1)
            return out

        wpsl_hi, wpsl_lo = pack(wp_hi8), pack(wp_lo8)
    elif fp8 and PSL_MODE == "e3":
        t_w = 15.5 / max(float(np.abs(W_psl).max()), 1e-30)
        _WPSL_T = [t_w]
        wpsl16 = np.asarray(
            np.clip(W_psl[0].astype(np.float32) * t_w, -15.5, 15.5)
            .reshape(8, 128).T.reshape(128, 8, 1, 1),
            dtype=ml_dtypes.float8_e3m4,
        ).copy()
    else:
        wpsl16 = np.ascontiguousarray(
            W_psl[0].reshape(8, 128).T.reshape(128, 8, 1, 1)
        ).astype(np.float16)
    iota = np.ascontiguousarray(
        np.tile(np.arange(SMAX, dtype=np.float16), (128, 1))
    )
    # exact cubic through the 4 species energies
    V = np.vander(np.arange(N_SPECIES, dtype=np.float64), 4, increasing=True)
    poly = np.linalg.solve(V, W_comp[0, :N_SPECIES].astype(np.float64))

    # per-shard relative batch ids (padding = -1)
    rb_all = []
    for s_lo, s_hi, a_lo, a_hi, n_at, n_st in shards:
        rb = np.full(Ta, -1.0, dtype=np.float32)
        rb[:n_at] = (batch[a_lo:a_hi] - s_lo).astype(np.float32)
        rb_all.append(rb)

    segw = None
    if SEG_WINDOWED:
        # shared per-chunk structure windows: batch is sorted so each
        # 128-atom chunk only spans a few structures; bake [w0, w0+W)
        m0 = np.full(C, np.inf)
        m1 = np.full(C, -np.inf)
        for rb in rb_all:
            r2 = rb.reshape(C, CHUNK)
            mask = r2 >= 0
            has = mask.any(axis=1)
            lo = np.where(has, np.where(mask, r2, np.inf).min(axis=1), np.inf)
            hi = np.where(has, np.where(mask, r2, -np.inf).max(axis=1), -np.inf)
            m0 = np.minimum(m0, lo)
            m1 = np.maximum(m1, hi)
        w0 = np.where(np.isfinite(m0), m0, 0.0)
        span = np.where(np.isfinite(m1), m1 - w0 + 1, 1.0)
        W = int(max(1, span.max()))
        W = min((W + 3) // 4 * 4, SMAX)
        w0 = np.clip(w0, 0, SMAX - W).astype(np.int64)
        segw = (W, tuple(int(x) for x in w0))

    key = (Ta, C, tuple(np.round(poly, 12)), MODE, PSL_MODE, segw)
    if key not in _BUILD_CACHE:
        _BUILD_CACHE.clear()
        _BUILD_CACHE[key] = _build(Ta, C, poly, MODE, PSL_MODE, segw)
    nc = _BUILD_CACHE[key]

    psq = np.asarray(ps, dtype=DT)  # quantize once, slice per shard
    ones = np.ones((97, 1), dtype=np.float16)
    if fp8 and PSL_MODE == "hilo8":
        ps2 = (ps.astype(np.float32) - psq.astype(np.float32)).astype(F8)
    elif fp8 and PSL_MODE == "e3":
        # scale ps and wpsl into e3m4's normal range (subnormals would
        # cost ~8% error on small values); the 1/(s*t) unscale rides in
        # the colize ones vector rows 0:33 (psl partial rows 0/32)
        s_ps = 15.5 / max(float(np.abs(ps).max()), 1e-30)
        ps2 = np.asarray(
            np.clip(ps.astype(np.float32) * s_ps, -15.5, 15.5),
            dtype=ml_dtypes.float8_e3m4,
        )
        ones[0:97] = np.float16(1.0 / (s_ps * _WPSL_T[0]))
        wout = np.ascontiguousarray(
            (W_out[0] * (s_ps * _WPSL_T[0])).reshape(2, 128).T.reshape(128, 2, 1)
        ).astype(np.float16)
    elif fp8:
        ps2 = ps.astype(np.float16)

    in_maps = []
    for si, (s_lo, s_hi, a_lo, a_hi, n_at, n_st) in enumerate(shards):
        pss = np.zeros((Ta, N_FEAT), dtype=DT)
        pss[:n_at] = psq[a_lo:a_hi]
        rb = rb_all[si]
        if segw is not None:
            rb = rb.reshape(C, CHUNK) - np.asarray(segw[1], dtype=np.float32)[:, None]
            rb = rb.reshape(Ta)
        nums = np.zeros(Ta, dtype=np.float32)
        nums[:n_at] = numbers[a_lo:a_hi].astype(np.float32)
        im = {
            "psT": _to_psT(pss, Ta),
            "relb": np.ascontiguousarray(rb.reshape(C, CHUNK).T),
            "nums": np.ascontiguousarray(nums.reshape(C, CHUNK).T),
            "w1": w1,
            "w2": w2,
            "wout": wout,
            "ones": ones,
            "iota": iota,
        }
        if fp8 and PSL_MODE == "hilo8":
            pss2 = np.zeros((Ta, N_FEAT), dtype=F8)
            pss2[:n_at] = ps2[a_lo:a_hi]
            im["psT2"] = _to_psT(pss2, Ta)
            im["wpsl_hi"] = wpsl_hi
            im["wpsl_lo"] = wpsl_lo
        elif fp8:
            pss2 = np.zeros((Ta, N_FEAT), dtype=ps2.dtype)
            pss2[:n_at] = ps2[a_lo:a_hi]
            im["psT2"] = _to_psT(pss2, Ta)
            im["wpsl"] = wpsl16
        else:
            im["wpsl"] = wpsl16
        in_maps.append(im)

    res = run_bass_kernel_spmd(nc, in_maps, list(range(N_CORES)), trace=TRACE)
    LAST_EXEC_NS = res.exec_time_ns
    LAST_RESULTS = res

    out = np.zeros((N_STRUCT, 1), dtype=np.float32)
    for i, (s_lo, s_hi, a_lo, a_hi, n_at, n_st) in enumerate(shards):
        vals = res.results[i]["out"][0, :n_st].astype(np.float32)
        empty = counts[s_lo:s_hi] == 0
        if empty.any():
            vals = np.where(empty, 0.0, vals)
        out[s_lo:s_hi, 0] = vals
    return out
